# revision 1
# baseline (speedup 1.0000x reference)
"""Trainium2 Bass kernel for a 2-layer GAT + MLP (nn_MemoryGNN).

Strategy (8 NeuronCores, SPMD):
  - Destination-node partition: core k owns dst nodes [k*6250, (k+1)*6250).
  - Every core computes the FULL H1 = x @ [W1|U1|V1] table (x is broadcast by
    the host for free), so layer-1 message gathers are purely local.
  - Per-edge gathers use gpsimd.dma_gather from padded per-dst-tile slot
    tables (host-precomputed int16 index blobs).  Padding slots point at a
    dummy row whose attention-score column is -1e30, so exp() gives them zero
    weight with no masking ops.
  - Softmax is computed unnormalized (exp without segment-max subtraction;
    scores are O(10) so this is safe) and num/den partials are accumulated.
  - Layer 2 needs remote H2 rows: a chunked AllGather of the per-core H2
    shards overlaps with layer-1 compute.
  - Output rows are produced in a degree-sorted permuted order; the host
    applies the inverse permutation (free).
"""

import sys
import numpy as np

for _p in ("/opt/trn_rl_repo", "/root/.axon_site/_ro/trn_rl_repo"):
    if _p not in sys.path:
        sys.path.insert(0, _p)

import concourse.bass as bass
import concourse.bacc as bacc
import concourse.mybir as mybir
import concourse.tile as tile
from concourse import library_config
from concourse.bass_utils import run_bass_kernel_spmd

F32 = mybir.dt.float32
I16 = mybir.dt.int16
AF = mybir.ActivationFunctionType
OP = mybir.AluOpType
AX = mybir.AxisListType

NEG_SLOPE = 0.2


def make_cfg(N=50000, E=1000000, IN_DIM=256, HID=64, HEADS=4, OUT_DIM=128,
             NC=8, CHT=7, KCAP1=32, KCAP2=32):
    cfg = dict(N=N, E=E, IN_DIM=IN_DIM, HID=HID, HEADS=HEADS, OUT_DIM=OUT_DIM,
               NC=NC, CHT=CHT, KCAP1=KCAP1, KCAP2=KCAP2)
    cfg["SHARD"] = N // NC
    assert N % NC == 0
    TP = 128
    cfg["TP"] = TP
    NT = -(-cfg["SHARD"] // TP)
    cfg["NT"] = NT
    assert NT % CHT == 0, (NT, CHT)
    cfg["NCH"] = NT // CHT
    cfg["ROWS"] = NT * TP
    cfg["CHROWS"] = CHT * TP          # SH2 rows per AllGather chunk
    # layer-1 table: row n -> n + (n >= LO1); 2 dummy rows
    cfg["D1"] = IN_DIM + 2 * HEADS    # used row width (H1 | ssrc | sdst)
    cfg["W1R"] = -(-cfg["D1"] // 64) * 64
    cfg["LO1"] = (N // 2 + 63) // 64 * 64
    assert cfg["LO1"] + 1 <= 32767 and N - cfg["LO1"] + 1 <= 32767
    cfg["HX1_ROWS"] = N + 2
    # layer-2 table (chunk-major): [chunks 0..L-1 | dum | chunks L..NCH-1 | dum]
    cfg["D2"] = OUT_DIM + 2
    cfg["W2R"] = -(-cfg["D2"] // 64) * 64
    CH_ALL = cfg["CHROWS"] * NC       # global rows per chunk
    cfg["CH_ALL"] = CH_ALL
    LOCH = NC * cfg["ROWS"] // 2 // CH_ALL   # chunks in the lo half
    LOCH = max(1, min(cfg["NCH"] - 1, LOCH))
    cfg["LOCH"] = LOCH
    cfg["LO2ROWS"] = LOCH * CH_ALL
    assert cfg["LO2ROWS"] + 1 <= 32767
    assert (cfg["NCH"] - LOCH) * CH_ALL + 1 <= 32767
    cfg["HX2_ROWS"] = cfg["NCH"] * CH_ALL + 2
    return cfg


# ----------------------------------------------------------------- host prep

def _wrap16(flat):
    """flat int array (len divisible by 16) -> wrapped [128, n/16] int16."""
    w = flat.reshape(-1, 16).T.astype(np.int16)
    return np.tile(w, (8, 1))


def _pack_core(cfg, srcs_by_dst, row_of_src, lo_limit, dum_lo, dum_hi, kcap):
    """For one core: sort dsts by (lo,hi) counts, tile, build index blob.

    srcs_by_dst: list over local dst ids of arrays of table rows (already
    mapped through row_of_src).  Returns (perm, vtiles, blob_cols) where
    vtiles is a list per real tile of [(kl, kh), ...] sub-iterations and
    blob_cols the per-tile wrapped int16 column blocks (as arrays).
    """
    SHARD, TP, NT = cfg["SHARD"], cfg["TP"], cfg["NT"]
    lo_cnt = np.array([int((s < lo_limit).sum()) for s in srcs_by_dst])
    hi_cnt = np.array([len(s) for s in srcs_by_dst]) - lo_cnt
    order = np.lexsort((-hi_cnt, -lo_cnt))
    perm = np.full(NT * TP, -1, dtype=np.int64)
    perm[:SHARD] = order
    kl_t = np.zeros(NT, dtype=np.int64)
    kh_t = np.zeros(NT, dtype=np.int64)
    for t in range(NT):
        rows = perm[t * TP:(t + 1) * TP]
        real = rows[rows >= 0]
        if len(real):
            kl_t[t] = lo_cnt[real].max()
            kh_t[t] = hi_cnt[real].max()
    return perm, lo_cnt, hi_cnt, kl_t, kh_t


def _build_blobs(cfg, perm, srcs_by_dst, lo_limit, dum_lo, dum_hi,
                 kl_t, kh_t, kcap, dst_rows):
    """Build the per-core int16 index blob.

    Per tile layout: [dst-lo idx (8 cols) | dst-hi idx (8 cols) |
                      per-vtile (lo slots kl_v*8 | hi slots kh_v*8) ...]
    dst_rows: table row of each local dst (for the sdst gather).
    Returns (blob [128, C] int16, vtiles list, col offsets dict).
    """
    TP, NT = cfg["TP"], cfg["NT"]
    cols = []
    meta = []
    for t in range(NT):
        rows = perm[t * TP:(t + 1) * TP]
        # dst gathers (lo/hi split with additive dummy)
        dlo = np.full(TP, dum_lo, dtype=np.int64)
        dhi = np.full(TP, dum_hi - lo_limit, dtype=np.int64)
        for p, r in enumerate(rows):
            if r >= 0:
                dr = dst_rows[r]
                if dr < lo_limit:
                    dlo[p] = dr
                else:
                    dhi[p] = dr - lo_limit
        tile_cols = [_wrap16(dlo), _wrap16(dhi)]
        # slot tables
        lo_mat = np.full((TP, max(1, kl_t[t])), dum_lo, dtype=np.int64)
        hi_mat = np.full((TP, max(1, kh_t[t])), dum_hi - lo_limit, dtype=np.int64)
        for p, r in enumerate(rows):
            if r >= 0:
                s = srcs_by_dst[r]
                lo = s[s < lo_limit]
                hi = s[s >= lo_limit] - lo_limit
                lo_mat[p, :len(lo)] = lo
                hi_mat[p, :len(hi)] = hi
        # split into per-half vtiles of slot count <= kcap
        vt = []
        la, ha = int(kl_t[t]), int(kh_t[t])
        off = 0
        while off < la:
            kv = min(kcap, la - off)
            vt.append((0, kv))
            tile_cols.append(_wrap16(lo_mat[:, off:off + kv].T.reshape(-1)))
            off += kv
        off = 0
        while off < ha:
            kv = min(kcap, ha - off)
            vt.append((1, kv))
            tile_cols.append(_wrap16(hi_mat[:, off:off + kv].T.reshape(-1)))
            off += kv
        meta.append(vt)
        cols.append(np.concatenate(tile_cols, axis=1))
    blob = np.concatenate(cols, axis=1)
    return blob, meta


def ho_remaining(ha, hi_off):
    return hi_off < ha


def _prep(cfg, edge_index):
    """Host preprocessing. Returns per-core blobs + tile structure + perms."""
    N, NC, SHARD = cfg["N"], cfg["NC"], cfg["SHARD"]
    TP, NT, ROWS = cfg["TP"], cfg["NT"], cfg["ROWS"]
    src = np.concatenate([np.asarray(edge_index[0]), np.arange(N)]).astype(np.int64)
    dst = np.concatenate([np.asarray(edge_index[1]), np.arange(N)]).astype(np.int64)
    order = np.argsort(dst, kind="stable")
    src_s, dst_s = src[order], dst[order]
    deg = np.bincount(dst, minlength=N)
    rp = np.zeros(N + 1, dtype=np.int64)
    np.cumsum(deg, out=rp[1:])

    LO1 = cfg["LO1"]
    row1_of = lambda n: n + (n >= LO1)          # noqa: E731
    dum1_lo, dum1_hi = LO1, N + 1               # table rows (hi is global row)

    cores = []
    for k in range(NC):
        g0 = k * SHARD
        srcs_by_dst = [src_s[rp[g0 + r]:rp[g0 + r + 1]] for r in range(SHARD)]
        rows1 = [row1_of(s) for s in srcs_by_dst]
        perm1, lo1, hi1, kl1, kh1 = _pack_core(
            cfg, rows1, None, LO1, dum1_lo, dum1_hi, cfg["KCAP1"])
        cores.append(dict(srcs_by_dst=srcs_by_dst, rows1=rows1, perm1=perm1,
                          kl1=kl1, kh1=kh1))

    # uniform tile sizes across cores
    KL1 = np.max([c["kl1"] for c in cores], axis=0)
    KH1 = np.max([c["kh1"] for c in cores], axis=0)

    # HX2 chunk-major row of each global node (needs perm1 of its owner)
    pos1 = np.empty(N, dtype=np.int64)
    for k in range(NC):
        p = cores[k]["perm1"]
        for q in range(ROWS):
            if p[q] >= 0:
                pos1[k * SHARD + p[q]] = q
    CHROWS, CH_ALL = cfg["CHROWS"], cfg["CH_ALL"]
    c_of = pos1 // CHROWS
    r_of = pos1 % CHROWS
    owner = np.arange(N) // SHARD
    cm = c_of * CH_ALL + owner * CHROWS + r_of
    LO2 = cfg["LO2ROWS"]
    row2_of_node = cm + (cm >= LO2)             # dummy inserted at LO2
    dum2_lo = LO2
    dum2_hi = cfg["HX2_ROWS"] - 1

    for k in range(NC):
        c = cores[k]
        rows2 = [row2_of_node[s] for s in c["srcs_by_dst"]]
        perm2, lo2, hi2, kl2, kh2 = _pack_core(
            cfg, rows2, None, LO2 + 1, dum2_lo, dum2_hi, cfg["KCAP2"])
        c.update(rows2=rows2, perm2=perm2, kl2=kl2, kh2=kh2)

    KL2 = np.max([c["kl2"] for c in cores], axis=0)
    KH2 = np.max([c["kh2"] for c in cores], axis=0)

    # build blobs with the uniform sizes
    blobs1, blobs2 = [], []
    meta1 = meta2 = None
    for k in range(NC):
        c = cores[k]
        d1rows = np.array([row1_of(k * SHARD + r) for r in range(SHARD)])
        b1, m1 = _build_blobs(cfg, c["perm1"], c["rows1"], LO1 + 1, dum1_lo,
                              dum1_hi, KL1, KH1, cfg["KCAP1"], d1rows)
        d2rows = np.array([row2_of_node[k * SHARD + r] for r in range(SHARD)])
        b2, m2 = _build_blobs(cfg, c["perm2"], c["rows2"], LO2 + 1, dum2_lo,
                              dum2_hi, KL2, KH2, cfg["KCAP2"], d2rows)
        blobs1.append(b1)
        blobs2.append(b2)
        meta1, meta2 = m1, m2   # identical structure across cores
    return dict(cores=cores, blobs1=blobs1, blobs2=blobs2,
                vt1=meta1, vt2=meta2, KL1=KL1, KH1=KH1, KL2=KL2, KH2=KH2)


def _pack_consts(cfg, W1, a1_src, a1_dst, b1, W2, a2_src, a2_dst, b2,
                 Wm1, bm1, Wm2, bm2):
    IN_DIM, HID, HEADS, OUT_DIM = (cfg["IN_DIM"], cfg["HID"], cfg["HEADS"],
                                   cfg["OUT_DIM"])
    D1, D2, W2R = cfg["D1"], cfg["D2"], cfg["W2R"]
    U1 = np.einsum("khc,hc->kh", W1.reshape(IN_DIM, HEADS, HID), a1_src)
    V1 = np.einsum("khc,hc->kh", W1.reshape(IN_DIM, HEADS, HID), a1_dst)
    W1R = cfg["W1R"]
    W1X = np.zeros((IN_DIM, W1R), dtype=np.float32)
    W1X[:, :D1] = np.concatenate([W1, U1, V1], axis=1)
    u2 = W2 @ a2_src[0]
    v2 = W2 @ a2_dst[0]
    W2X = np.zeros((HEADS * HID, W2R), dtype=np.float32)
    W2X[:, :OUT_DIM] = W2
    W2X[:, OUT_DIM] = u2
    W2X[:, OUT_DIM + 1] = v2
    P = 128
    blocks = {}
    cols = 0
    def add(name, arr):
        nonlocal cols
        a = np.zeros((P, arr.shape[1]), dtype=np.float32)
        a[:arr.shape[0]] = arr
        blocks[name] = (cols, arr.shape[1])
        cols += arr.shape[1]
        return a
    parts = []
    parts.append(add("w1x0", W1X[0:P]))
    parts.append(add("w1x1", W1X[P:2 * P]))
    parts.append(add("w2x0", W2X[0:P]))
    parts.append(add("w2x1", W2X[P:2 * P]))
    parts.append(add("wm1", Wm1.astype(np.float32)))
    parts.append(add("wm2", Wm2.astype(np.float32)))
    parts.append(add("b1r", np.tile(b1.astype(np.float32), (P, 1))))
    parts.append(add("b2r", np.tile(b2.astype(np.float32), (P, 1))))
    parts.append(add("bm1r", np.tile(bm1.astype(np.float32), (P, 1))))
    parts.append(add("bm2r", np.tile(bm2.astype(np.float32), (P, 1))))
    parts.append(add("ident", np.eye(P, dtype=np.float32)))
    consts = np.concatenate(parts, axis=1)
    # dummy rows: [hx1_lo, hx1_hi, hx2_lo, hx2_hi] in a [4, W1R] array
    dums = np.zeros((4, cfg["W1R"]), dtype=np.float32)
    dums[0:2, IN_DIM:IN_DIM + HEADS] = -1e30        # ssrc1
    dums[2:4, OUT_DIM] = -1e30                      # ssrc2
    return consts, blocks, dums


# ------------------------------------------------------------- device build

def _build(cfg, prep, cblocks, CW, phase="full"):
    N, NC = cfg["N"], cfg["NC"]
    IN_DIM, HID, HEADS, OUT_DIM = (cfg["IN_DIM"], cfg["HID"], cfg["HEADS"],
                                   cfg["OUT_DIM"])
    TP, NT, ROWS, SHARD = cfg["TP"], cfg["NT"], cfg["ROWS"], cfg["SHARD"]
    D1, W1R, LO1 = cfg["D1"], cfg["W1R"], cfg["LO1"]
    D2, W2R, LO2 = cfg["D2"], cfg["W2R"], cfg["LO2ROWS"]
    CHT, NCH, CHROWS, CH_ALL = cfg["CHT"], cfg["NCH"], cfg["CHROWS"], cfg["CH_ALL"]
    HX1R, HX2R = cfg["HX1_ROWS"], cfg["HX2_ROWS"]
    KL1, KH1, KL2, KH2 = prep["KL1"], prep["KH1"], prep["KL2"], prep["KH2"]
    vt1, vt2 = prep["vt1"], prep["vt2"]
    C1 = prep["blobs1"][0].shape[1]
    C2 = prep["blobs2"][0].shape[1]
    NH2 = HEADS * HID

    nc = bacc.Bacc("TRN2", target_bir_lowering=False, debug=False,
                   num_devices=NC)
    xT = nc.dram_tensor("xT", [IN_DIM, N], F32, kind="ExternalInput")
    consts = nc.dram_tensor("consts", [128, CW], F32, kind="ExternalInput")
    dums = nc.dram_tensor("dums", [4, W1R], F32, kind="ExternalInput")
    idx1 = nc.dram_tensor("idx1", [128, C1], I16, kind="ExternalInput")
    idx2 = nc.dram_tensor("idx2", [128, C2], I16, kind="ExternalInput")
    out = nc.dram_tensor("out", [ROWS, OUT_DIM], F32, kind="ExternalOutput")
    dbg = nc.dram_tensor("dbg", [ROWS, W1R], F32, kind="ExternalOutput")

    HX1 = nc.dram_tensor("HX1", [HX1R, W1R], F32)
    HX2 = nc.dram_tensor("HX2", [HX2R, W2R], F32)
    SH2 = nc.dram_tensor("SH2", [ROWS, W2R], F32)

    hx1_lo = HX1[0:LO1 + 1, :]
    hx1_hi = HX1[LO1 + 1:HX1R, :]
    hx2_lo = HX2[0:LO2 + 1, :]
    hx2_hi = HX2[LO2 + 1:HX2R, :]

    P = 128

    with tile.TileContext(nc) as tc:
        nc.gpsimd.load_library(library_config.mlp)
        with tc.tile_pool(name="cp", bufs=1) as cp:
            cb = cp.tile([128, CW], F32, tag="consts")
            nc.sync.dma_start(cb[:, :], consts[:, :])

            def C(name):
                off, w = cblocks[name]
                return cb[:, off:off + w]

            # dummy rows (DRAM -> DRAM)
            nc.sync.dma_start(HX1[LO1:LO1 + 1, :], dums[0:1, :])
            nc.sync.dma_start(HX1[HX1R - 1:HX1R, :], dums[1:2, :])
            nc.sync.dma_start(HX2[LO2:LO2 + 1, :], dums[2:3, 0:W2R])
            nc.sync.dma_start(HX2[HX2R - 1:HX2R, :], dums[3:4, 0:W2R])

            # ---------------- P0: full HX1 table -----------------------
            SB = 512
            nsb = -(-N // SB) if cfg.get("P0", 1) else 0
            with (
                tc.tile_pool(name="p0", bufs=2) as p0,
                tc.tile_pool(name="p0ps", bufs=8, space="PSUM") as p0ps,
            ):
                for sb in range(nsb):
                    base = sb * SB
                    cnt = min(SB, N - base)
                    nq = -(-cnt // P)
                    xt0 = p0.tile([P, cnt], F32, tag="xt0")
                    xt1 = p0.tile([P, cnt], F32, tag="xt1")
                    nc.sync.dma_start(xt0[:, :], xT[0:P, base:base + cnt])
                    nc.sync.dma_start(xt1[:, :], xT[P:2 * P, base:base + cnt])
                    hx4 = p0.tile([P, nq * W1R], F32, tag="hx4")
                    for q in range(nq):
                        pb = min(P, cnt - q * P)
                        ps = p0ps.tile([P, W1R], F32, tag="p0ps")
                        nc.tensor.matmul(ps[0:pb, :], xt0[:, q * P:q * P + pb],
                                         C("w1x0"), start=True, stop=False)
                        nc.tensor.matmul(ps[0:pb, :], xt1[:, q * P:q * P + pb],
                                         C("w1x1"), start=False, stop=True)
                        nc.scalar.copy(hx4[0:pb, q * W1R:(q + 1) * W1R],
                                       ps[0:pb, :])
                    # write rows [base, base+cnt) -> HX1 (split at LO1)
                    hx4v = hx4[:, :].rearrange("p (q w) -> p q w", q=nq)
                    def wr(a, b):   # node range [a, b) within this superblock
                        if a >= b:
                            return
                        ra = base + a + (1 if base + a >= LO1 else 0)
                        dv = HX1[ra:ra + (b - a), :]
                        qa, pa = divmod(a, P)
                        qb, pb_ = divmod(b - 1, P)
                        if qa == qb:
                            nc.sync.dma_start(
                                dv, hx4v[pa:pb_ + 1, qa, :])
                        else:
                            n0 = P - pa
                            nc.sync.dma_start(dv[0:n0, :], hx4v[pa:P, qa, :])
                            off = n0
                            for q in range(qa + 1, qb):
                                nc.sync.dma_start(dv[off:off + P, :],
                                                  hx4v[0:P, q, :])
                                off += P
                            nc.sync.dma_start(dv[off:, :],
                                              hx4v[0:pb_ + 1, qb, :])
                    if base < LO1 < base + cnt:
                        wr(0, LO1 - base)
                        wr(LO1 - base, cnt)
                    else:
                        wr(0, cnt)

            if phase == "p0":
                nc.sync.dma_start(dbg[0:128, :], HX1[0:128, :])
                nc.sync.dma_start(dbg[128:256, :],
                                  HX1[LO1 + 1:LO1 + 129, :])
            # ---------------- L1 + H2 prep + chunked AllGather ----------
            with (
                tc.tile_pool(name="l1", bufs=2) as l1,
                tc.tile_pool(name="l1b", bufs=2) as l1b,
                tc.tile_pool(name="l1ps", bufs=2, space="PSUM") as l1ps,
            ):
                col = [0]

                def idx_tile(pool, blob, ncols, tag):
                    it = pool.tile([128, ncols], I16, tag=tag)
                    nc.sync.dma_start(it[:, :],
                                      blob[:, col[0]:col[0] + ncols])
                    col[0] += ncols
                    return it

                L1T = cfg.get("L1T", NT)
                L1S = cfg.get("L1S", 3)
                for t in (range(L1T) if phase != "p0" else range(0)):
                    # sdst gathers
                    itl = idx_tile(l1b, idx1, 8, "it_dl")
                    ith = idx_tile(l1b, idx1, 8, "it_dh")
                    if L1S >= 1:
                        sdl = l1b.tile([P, 64], F32, tag="sdl")
                        sdh = l1b.tile([P, 64], F32, tag="sdh")
                        nc.gpsimd.dma_gather(
                            sdl[:, :].rearrange("p (j w) -> p j w", j=1),
                            hx1_lo[:, IN_DIM:IN_DIM + 64], itl[:, :], P, P,
                            64, elem_step=W1R, single_packet=False)
                        nc.gpsimd.dma_gather(
                            sdh[:, :].rearrange("p (j w) -> p j w", j=1),
                            hx1_hi[:, IN_DIM:IN_DIM + 64], ith[:, :], P, P,
                            64, elem_step=W1R, single_packet=False)
                        sd4 = l1b.tile([P, HEADS], F32, tag="sd4")
                        nc.vector.tensor_tensor(
                            sd4[:, :], sdl[:, HEADS:2 * HEADS],
                            sdh[:, HEADS:2 * HEADS], op=OP.add)
                    if L1S >= 2:
                        num = l1b.tile([P, NH2], F32, tag="num")
                        den = l1b.tile([P, HEADS], F32, tag="den")
                    L1G = cfg.get("L1G", 99)
                    for v, (half, kv) in enumerate(vt1[t]):
                        itv = idx_tile(l1b, idx1, kv * 8, "it_sl")
                        if v >= L1G:
                            nc.sync.dma_start(dbg[0:128, 0:kv * 8].bitcast(I16),
                                              itv[:, :]) if False else None
                            continue
                        hg = l1.tile([P, kv * W1R], F32, tag="hg")
                        nc.gpsimd.dma_gather(
                            hg[:, :].rearrange("p (j w) -> p j w", j=kv),
                            (hx1_lo if half == 0 else hx1_hi)[:, :],
                            itv[:, :], P * kv, P * kv, W1R,
                            single_packet=False)
                        if L1S == 0:
                            if t == 0 and v == 0:
                                nc.sync.dma_start(
                                    dbg[0:P, 0:min(W1R, kv * W1R)],
                                    hg[:, 0:min(W1R, kv * W1R)])
                            continue
                        hgv = hg[:, :].rearrange("p (j w) -> p j w", j=kv)
                        s = l1b.tile([P, kv * HEADS], F32, tag="s")
                        sv = s[:, :].rearrange("p (j h) -> p j h", j=kv)
                        nc.vector.tensor_tensor(
                            sv, hgv[:, :, IN_DIM:IN_DIM + HEADS],
                            sd4[:, :].unsqueeze(1).broadcast_to(
                                [P, kv, HEADS]), op=OP.add)
                        s2t = l1b.tile([P, kv * HEADS], F32, tag="s2t")
                        nc.vector.tensor_scalar_mul(s2t[:, :], s[:, :],
                                                    NEG_SLOPE)
                        w0 = l1b.tile([P, kv * HEADS], F32, tag="w0")
                        nc.vector.tensor_tensor(w0[:, :], s[:, :], s2t[:, :],
                                                op=OP.max)
                        w = l1b.tile([P, kv * HEADS], F32, tag="w")
                        nc.scalar.activation(w[:, :], w0[:, :], AF.Exp)
                        if L1S == 1:
                            if t == 0 and v == 0:
                                nc.sync.dma_start(dbg[0:P, 0:kv * HEADS],
                                                  w[:, :])
                            continue
                        wv = w[:, :].rearrange("p (j h) -> p j h", j=kv)
                        if v == 0:
                            dv = den[:, :]
                        else:
                            denv = l1b.tile([P, HEADS], F32, tag="denv")
                            dv = denv[:, :]
                        nc.vector.tensor_reduce(
                            dv, w[:, :].rearrange("p (j h) -> p h j", j=kv),
                            axis=AX.X, op=OP.add)
                        if v > 0:
                            nc.vector.tensor_tensor(den[:, :], den[:, :], dv,
                                                    op=OP.add)
                        tmp = l1.tile([P, kv * NH2], F32, tag="tmp")
                        tmpv = tmp[:, :].rearrange(
                            "p (j h c) -> p j h c", j=kv, h=HEADS)
                        nc.vector.tensor_tensor(
                            tmpv,
                            hgv[:, :, 0:IN_DIM].rearrange(
                                "p j (h c) -> p j h c", h=HEADS),
                            wv.unsqueeze(3).broadcast_to([P, kv, HEADS, HID]),
                            op=OP.mult)
                        if v == 0:
                            nv = num[:, :]
                        else:
                            numv = l1b.tile([P, NH2], F32, tag="numv")
                            nv = numv[:, :]
                        nc.vector.tensor_reduce(
                            nv, tmp[:, :].rearrange("p (j c) -> p c j", j=kv),
                            axis=AX.X, op=OP.add)
                        if v > 0:
                            nc.vector.tensor_tensor(num[:, :], num[:, :], nv,
                                                    op=OP.add)
                    if L1S < 3:
                        if L1S == 2 and t == 0:
                            nc.sync.dma_start(dbg[0:P, 0:NH2], num[:, :])
                            nc.sync.dma_start(dbg[0:P, NH2:NH2 + HEADS],
                                              den[:, :])
                        continue
                    dinv = l1b.tile([P, HEADS], F32, tag="dinv")
                    nc.vector.tensor_scalar_max(dinv[:, :], den[:, :], 1e-6)
                    nc.vector.reciprocal(dinv[:, :], dinv[:, :])
                    o = l1b.tile([P, NH2], F32, tag="o")
                    nc.vector.tensor_tensor(
                        o[:, :].rearrange("p (h c) -> p h c", h=HEADS),
                        num[:, :].rearrange("p (h c) -> p h c", h=HEADS),
                        dinv[:, :].unsqueeze(2).broadcast_to([P, HEADS, HID]),
                        op=OP.mult)
                    nc.vector.tensor_tensor(o[:, :], o[:, :], C("b1r"),
                                            op=OP.add)
                    # elu
                    m0 = l1b.tile([P, NH2], F32, tag="m0")
                    nc.vector.tensor_scalar_min(m0[:, :], o[:, :], 0.0)
                    em = l1b.tile([P, NH2], F32, tag="em")
                    nc.scalar.activation(em[:, :], m0[:, :], AF.Exp)
                    p1 = l1b.tile([P, NH2], F32, tag="p1")
                    nc.vector.tensor_scalar(p1[:, :], o[:, :], 0.0, -1.0,
                                            op0=OP.max, op1=OP.add)
                    eo = l1b.tile([P, NH2], F32, tag="eo")
                    nc.vector.tensor_tensor(eo[:, :], em[:, :], p1[:, :],
                                            op=OP.add)
                    # transpose + H2 matmul
                    o1T = l1b.tile([P, NH2], F32, tag="o1T")
                    for cchunk in range(NH2 // P):
                        pt = l1ps.tile([P, P], F32, tag="pt")
                        nc.tensor.transpose(
                            pt[:, :], eo[:, cchunk * P:(cchunk + 1) * P],
                            C("ident"))
                        nc.scalar.copy(o1T[:, cchunk * P:(cchunk + 1) * P],
                                       pt[:, :])
                    h2p = l1ps.tile([P, W2R], F32, tag="h2p")
                    nc.tensor.matmul(h2p[:, :], o1T[:, 0:P], C("w2x0"),
                                     start=True, stop=False)
                    nc.tensor.matmul(h2p[:, :], o1T[:, P:2 * P], C("w2x1"),
                                     start=False, stop=True)
                    sh2 = l1b.tile([P, W2R], F32, tag="sh2")
                    nc.scalar.copy(sh2[:, :], h2p[:, :])
                    nc.sync.dma_start(SH2[t * P:(t + 1) * P, :], sh2[:, :])

                    if (t + 1) % CHT == 0 and phase not in ("l1",):
                        c = t // CHT
                        base = c * CH_ALL + (1 if c >= cfg["LOCH"] else 0)
                        nc.gpsimd.collective_compute(
                            "AllGather", OP.bypass,
                            replica_groups=[list(range(NC))],
                            ins=[SH2[c * CHROWS:(c + 1) * CHROWS, :].opt()],
                            outs=[HX2[base:base + CH_ALL, :].opt()],
                        )

            if phase in ("l1", "ag"):
                for _t in range(NT):
                    nc.sync.dma_start(
                        dbg[_t * P:(_t + 1) * P, 0:W2R],
                        SH2[_t * P:(_t + 1) * P, :])
            # ---------------- L2 + MLP + normalize ----------------------
            with (
                tc.tile_pool(name="l2", bufs=2) as l2,
                tc.tile_pool(name="l2b", bufs=2) as l2b,
                tc.tile_pool(name="l2ps", bufs=2, space="PSUM") as l2ps,
            ):
                col2 = [0]

                def idx_tile2(ncols, tag):
                    it = l2b.tile([128, ncols], I16, tag=tag)
                    nc.sync.dma_start(it[:, :],
                                      idx2[:, col2[0]:col2[0] + ncols])
                    col2[0] += ncols
                    return it

                for t in (range(NT) if phase == "full" else range(0)):
                    itl = idx_tile2(8, "it_dl")
                    ith = idx_tile2(8, "it_dh")
                    sdl = l2b.tile([P, 64], F32, tag="sdl")
                    sdh = l2b.tile([P, 64], F32, tag="sdh")
                    nc.gpsimd.dma_gather(
                        sdl[:, :].rearrange("p (j w) -> p j w", j=1),
                        hx2_lo[:, OUT_DIM:OUT_DIM + 64], itl[:, :], P, P, 64,
                        elem_step=W2R, single_packet=False)
                    nc.gpsimd.dma_gather(
                        sdh[:, :].rearrange("p (j w) -> p j w", j=1),
                        hx2_hi[:, OUT_DIM:OUT_DIM + 64], ith[:, :], P, P, 64,
                        elem_step=W2R, single_packet=False)
                    sd1 = l2b.tile([P, 1], F32, tag="sd1")
                    nc.vector.tensor_tensor(sd1[:, :], sdl[:, 1:2],
                                            sdh[:, 1:2], op=OP.add)

                    num = l2b.tile([P, OUT_DIM], F32, tag="num")
                    den = l2b.tile([P, 1], F32, tag="den")
                    for v, (half, kv) in enumerate(vt2[t]):
                        hg = l2.tile([P, kv * W2R], F32, tag="hg")
                        itv = idx_tile2(kv * 8, "it_sl")
                        nc.gpsimd.dma_gather(
                            hg[:, :].rearrange("p (j w) -> p j w", j=kv),
                            (hx2_lo if half == 0 else hx2_hi)[:, :],
                            itv[:, :], P * kv, P * kv, W2R,
                            single_packet=False)
                        hgv = hg[:, :].rearrange("p (j w) -> p j w", j=kv)
                        s = l2b.tile([P, kv], F32, tag="s")
                        nc.vector.tensor_tensor(
                            s[:, :].unsqueeze(2),
                            hgv[:, :, OUT_DIM:OUT_DIM + 1],
                            sd1[:, :].unsqueeze(1).broadcast_to([P, kv, 1]),
                            op=OP.add)
                        s2t = l2b.tile([P, kv], F32, tag="s2t")
                        nc.vector.tensor_scalar_mul(s2t[:, :], s[:, :],
                                                    NEG_SLOPE)
                        w0 = l2b.tile([P, kv], F32, tag="w0")
                        nc.vector.tensor_tensor(w0[:, :], s[:, :], s2t[:, :],
                                                op=OP.max)
                        w = l2b.tile([P, kv], F32, tag="w")
                        nc.scalar.activation(w[:, :], w0[:, :], AF.Exp)
                        if v == 0:
                            dv = den[:, :]
                        else:
                            denv = l2b.tile([P, 1], F32, tag="denv")
                            dv = denv[:, :]
                        nc.vector.tensor_reduce(dv, w[:, :], axis=AX.X,
                                                op=OP.add)
                        if v > 0:
                            nc.vector.tensor_tensor(den[:, :], den[:, :], dv,
                                                    op=OP.add)
                        tmp = l2.tile([P, kv * OUT_DIM], F32, tag="tmp")
                        nc.vector.tensor_tensor(
                            tmp[:, :].rearrange("p (j c) -> p j c", j=kv),
                            hgv[:, :, 0:OUT_DIM],
                            w[:, :].unsqueeze(2).broadcast_to(
                                [P, kv, OUT_DIM]),
                            op=OP.mult)
                        if v == 0:
                            nv = num[:, :]
                        else:
                            numv = l2b.tile([P, OUT_DIM], F32, tag="numv")
                            nv = numv[:, :]
                        nc.vector.tensor_reduce(
                            nv, tmp[:, :].rearrange("p (j c) -> p c j", j=kv),
                            axis=AX.X, op=OP.add)
                        if v > 0:
                            nc.vector.tensor_tensor(num[:, :], num[:, :], nv,
                                                    op=OP.add)
                    dinv = l2b.tile([P, 1], F32, tag="dinv")
                    nc.vector.tensor_scalar_max(dinv[:, :], den[:, :], 1e-6)
                    nc.vector.reciprocal(dinv[:, :], dinv[:, :])
                    o2 = l2b.tile([P, OUT_DIM], F32, tag="o2")
                    nc.vector.tensor_scalar(o2[:, :], num[:, :], dinv[:, 0:1],
                                            None, op0=OP.mult)
                    nc.vector.tensor_tensor(o2[:, :], o2[:, :], C("b2r"),
                                            op=OP.add)
                    # MLP
                    pt2 = l2ps.tile([P, P], F32, tag="pt2")
                    nc.tensor.transpose(pt2[:, :], o2[:, :], C("ident"))
                    o2T = l2b.tile([P, P], F32, tag="o2T")
                    nc.scalar.copy(o2T[:, :], pt2[:, :])
                    h3p = l2ps.tile([P, HID], F32, tag="h3p")
                    nc.tensor.matmul(h3p[:, :], o2T[:, :], C("wm1"),
                                     start=True, stop=True)
                    h3 = l2b.tile([P, HID], F32, tag="h3")
                    nc.vector.tensor_tensor(h3[:, :], h3p[:, :], C("bm1r"),
                                            op=OP.add)
                    nc.scalar.activation(h3[:, :], h3[:, :], AF.Relu)
                    pt3 = l2ps.tile([HID, P], F32, tag="pt3")
                    nc.tensor.transpose(pt3[:, :], h3[:, :], C("ident"))
                    h3T = l2b.tile([HID, P], F32, tag="h3T")
                    nc.scalar.copy(h3T[:, :], pt3[:, :])
                    h4p = l2ps.tile([P, OUT_DIM], F32, tag="h4p")
                    nc.tensor.matmul(h4p[:, :], h3T[0:HID, :],
                                     C("wm2")[0:HID, :], start=True, stop=True)
                    h4 = l2b.tile([P, OUT_DIM], F32, tag="h4")
                    nc.vector.tensor_tensor(h4[:, :], h4p[:, :], C("bm2r"),
                                            op=OP.add)
                    hsq = l2b.tile([P, OUT_DIM], F32, tag="hsq")
                    nc.scalar.activation(hsq[:, :], h4[:, :], AF.Square)
                    n2 = l2b.tile([P, 1], F32, tag="n2")
                    nc.vector.tensor_reduce(n2[:, :], hsq[:, :], axis=AX.X,
                                            op=OP.add)
                    nin = l2b.tile([P, 1], F32, tag="nin")
                    nc.vector.tensor_scalar_max(nin[:, :], n2[:, :], 1e-12)
                    nc.scalar.activation(nin[:, :], nin[:, :], AF.Sqrt)
                    nc.vector.reciprocal(nin[:, :], nin[:, :])
                    of = l2b.tile([P, OUT_DIM], F32, tag="of")
                    nc.vector.tensor_scalar(of[:, :], h4[:, :], nin[:, 0:1],
                                            None, op0=OP.mult)
                    nc.sync.dma_start(out[t * P:(t + 1) * P, :], of[:, :])

    nc.compile()
    return nc


# ------------------------------------------------------------------ driver

def run(cfg, inputs, trace=False, phase="full"):
    x = np.asarray(inputs["x"], dtype=np.float32)
    edge_index = np.asarray(inputs["edge_index"])
    prep = _prep(cfg, edge_index)
    consts, cblocks, dums = _pack_consts(
        cfg, *[np.asarray(inputs[k], dtype=np.float32) for k in
               ("W1", "a1_src", "a1_dst", "b1", "W2", "a2_src", "a2_dst",
                "b2", "Wm1", "bm1", "Wm2", "bm2")])
    nc = _build(cfg, prep, cblocks, consts.shape[1], phase=phase)
    xT = np.ascontiguousarray(x.T)
    in_maps = []
    for k in range(cfg["NC"]):
        in_maps.append({
            "xT": xT, "consts": consts, "dums": dums,
            "idx1": np.ascontiguousarray(prep["blobs1"][k]),
            "idx2": np.ascontiguousarray(prep["blobs2"][k]),
        })
    res = run_bass_kernel_spmd(nc, in_maps, list(range(cfg["NC"])),
                               trace=trace)
    N, NC, SHARD = cfg["N"], cfg["NC"], cfg["SHARD"]
    full = np.zeros((N, cfg["OUT_DIM"]), dtype=np.float32)
    for k in range(NC):
        o = res.results[k]["out"]
        perm2 = prep["cores"][k]["perm2"]
        for r in range(cfg["ROWS"]):
            if perm2[r] >= 0:
                full[k * SHARD + perm2[r]] = o[r]
    return full, res


def kernel(**inputs):
    cfg = make_cfg()
    full, _ = run(cfg, inputs, trace=False)
    return full



# revision 15
# speedup vs baseline: 1.8396x; 1.8396x over previous
"""Trainium2 Bass kernel for a 2-layer GAT + MLP (nn_MemoryGNN).

Strategy (8 NeuronCores, SPMD):
  - Destination-node partition: core k owns dst nodes [k*6250, (k+1)*6250).
  - Every core computes the FULL HX1 table (fp16): row n -> [h1 (256,
    head-minor c*4+h) | ssrc (4) | sdst (4) | pad] = 384 cols (768B rows),
    so layer-1 message gathers are purely local.
  - Per-edge gathers use gpsimd.dma_gather from per-dst-tile slot tables
    (host-precomputed int16 index blobs, degree-sorted tiles).
  - Softmax is computed unnormalized with a global exp-shift of -2 (folded
    into the Act-engine exp bias); the shift cancels in num/den.
  - fp16 everywhere in the hot loops (DVE 2x eligible); accumulation of the
    weighted message sum is an fp16 add-chain (err ~0.1% << 2e-2 tol).
  - Layer 2 (fp16 512B rows) needs remote H2 rows: a chunked AllGather of
    per-core SH2 shards into HX2 (Shared) overlaps with layer-1 compute.
  - Output rows are produced in a degree-sorted permuted order; the host
    applies the inverse permutation (free).
"""

import sys
import numpy as np

for _p in ("/opt/trn_rl_repo", "/root/.axon_site/_ro/trn_rl_repo"):
    if _p not in sys.path:
        sys.path.insert(0, _p)

import concourse.bass as bass
import concourse.bacc as bacc
import concourse.mybir as mybir
import concourse.tile as tile
from concourse import library_config
from concourse.bass_utils import run_bass_kernel_spmd

F32 = mybir.dt.float32
F16 = mybir.dt.float16
I16 = mybir.dt.int16
I32 = mybir.dt.int32
AF = mybir.ActivationFunctionType
OP = mybir.AluOpType
AX = mybir.AxisListType

NEG_SLOPE = 0.2
SHIFT = 2.0          # global exp shift (cancels in num/den)
PAD_BIAS = -30000.0  # fp16-safe "minus infinity" for masked slots


def make_cfg(N=50000, E=1000000, IN_DIM=256, HID=64, HEADS=4, OUT_DIM=128,
             NC=8, CHT=7, KCAP1=48, KCAP2=64):
    cfg = dict(N=N, E=E, IN_DIM=IN_DIM, HID=HID, HEADS=HEADS, OUT_DIM=OUT_DIM,
               NC=NC, CHT=CHT, KCAP1=KCAP1, KCAP2=KCAP2)
    cfg["SHARD"] = N // NC
    assert N % NC == 0
    TP = 128
    cfg["TP"] = TP
    NT = -(-cfg["SHARD"] // TP)
    cfg["NT"] = NT
    assert NT % CHT == 0, (NT, CHT)
    cfg["NCH"] = NT // CHT
    cfg["ROWS"] = NT * TP
    cfg["CHROWS"] = CHT * TP          # SH2 rows per AllGather chunk
    # layer-1 table: row n -> n + (n >= LO1); dummies at LO1 and last row
    cfg["D1"] = IN_DIM + 2 * HEADS
    cfg["W1R"] = 384
    assert cfg["W1R"] * 2 % 256 == 0
    cfg["LO1"] = 25024
    assert cfg["LO1"] + 1 <= 32767 and N - cfg["LO1"] + 1 <= 32767
    cfg["HX1_ROWS"] = N + 2
    # layer-2 table (chunk-major, no dummies; masked pads instead)
    cfg["D2"] = OUT_DIM + 2
    cfg["W2R"] = 256
    CH_ALL = cfg["CHROWS"] * NC       # global rows per chunk
    cfg["CH_ALL"] = CH_ALL
    LOCH = 32768 // CH_ALL            # chunks fully addressable as lo
    LOCH = max(1, min(cfg["NCH"] - 1, LOCH))
    cfg["LOCH"] = LOCH
    cfg["LO2ROWS"] = LOCH * CH_ALL
    assert cfg["LO2ROWS"] <= 32767
    assert (cfg["NCH"] - LOCH) * CH_ALL <= 32767
    cfg["HX2_ROWS"] = cfg["NCH"] * CH_ALL
    return cfg


# ----------------------------------------------------------------- host prep

def _wrap16(flat):
    """flat int array (len divisible by 16) -> wrapped [128, n/16] int16."""
    w = flat.reshape(-1, 16).T.astype(np.int16)
    return np.tile(w, (8, 1))


def _chunks(total, cap):
    out = []
    off = 0
    while off < total:
        kv = min(cap, total - off)
        out.append(kv)
        off += kv
    return out


def _sort_core(cfg, lo_cnt, hi_cnt):
    """Degree-sort local dsts into tiles; return perm and per-tile maxima."""
    SHARD, TP, NT = cfg["SHARD"], cfg["TP"], cfg["NT"]
    order = np.lexsort((-hi_cnt, -(lo_cnt // 6)))
    perm = np.full(NT * TP, -1, dtype=np.int64)
    perm[:SHARD] = order
    kl_t = np.zeros(NT, dtype=np.int64)
    kh_t = np.zeros(NT, dtype=np.int64)
    for t in range(NT):
        rows = perm[t * TP:(t + 1) * TP]
        real = rows[rows >= 0]
        if len(real):
            kl_t[t] = lo_cnt[real].max()
            kh_t[t] = hi_cnt[real].max()
    return perm, kl_t, kh_t


def _prep(cfg, edge_index):
    """Host preprocessing (structure only). Returns per-core blobs + meta."""
    N, NC, SHARD = cfg["N"], cfg["NC"], cfg["SHARD"]
    TP, NT, ROWS = cfg["TP"], cfg["NT"], cfg["ROWS"]
    KCAP1, KCAP2 = cfg["KCAP1"], cfg["KCAP2"]
    LO1 = cfg["LO1"]
    src = np.concatenate([np.asarray(edge_index[0]),
                          np.arange(N)]).astype(np.int64)
    dst = np.concatenate([np.asarray(edge_index[1]),
                          np.arange(N)]).astype(np.int64)
    order = np.argsort(dst, kind="stable")
    src_s = src[order]
    deg = np.bincount(dst, minlength=N)
    rp = np.zeros(N + 1, dtype=np.int64)
    np.cumsum(deg, out=rp[1:])

    # L1 table indexing: lo rows = node n (< LO1), dummy at LO1;
    # hi rows = node n - LO1 (n >= LO1), dummy at N - LO1 + 1 - 1
    dum1_lo = LO1
    dum1_hi = cfg["HX1_ROWS"] - 1 - (LO1 + 1)

    cores = []
    for k in range(NC):
        g0 = k * SHARD
        srcs = [src_s[rp[g0 + r]:rp[g0 + r + 1]] for r in range(SHARD)]
        lo_cnt = np.array([int((s < LO1).sum()) for s in srcs])
        hi_cnt = np.array([len(s) for s in srcs]) - lo_cnt
        perm1, kl1, kh1 = _sort_core(cfg, lo_cnt, hi_cnt)
        cores.append(dict(srcs=srcs, perm1=perm1, kl1=kl1, kh1=kh1))

    KL1 = np.max([c["kl1"] for c in cores], axis=0)
    KH1 = np.max([c["kh1"] for c in cores], axis=0)
    vt1 = [[(0, kv) for kv in _chunks(int(KL1[t]), KCAP1)] +
           [(1, kv) for kv in _chunks(int(KH1[t]), KCAP1)] for t in range(NT)]

    # position of each global node in its owner's tile order
    pos1 = np.empty(N, dtype=np.int64)
    for k in range(NC):
        p = cores[k]["perm1"]
        q = np.arange(ROWS)
        real = p >= 0
        pos1[k * SHARD + p[real]] = q[real]
    CHROWS, CH_ALL = cfg["CHROWS"], cfg["CH_ALL"]
    c_of = pos1 // CHROWS
    r_of = pos1 % CHROWS
    owner = np.arange(N) // SHARD
    cm = c_of * CH_ALL + owner * CHROWS + r_of      # HX2 row of node
    LO2 = cfg["LO2ROWS"]

    for k in range(NC):
        c = cores[k]
        rows2 = [cm[s] for s in c["srcs"]]
        lo2 = np.array([int((r < LO2).sum()) for r in rows2])
        hi2 = np.array([len(r) for r in rows2]) - lo2
        perm2, kl2, kh2 = _sort_core(cfg, lo2, hi2)
        c.update(rows2=rows2, perm2=perm2, kl2=kl2, kh2=kh2)

    KL2 = np.max([c["kl2"] for c in cores], axis=0)
    KH2 = np.max([c["kh2"] for c in cores], axis=0)
    vt2 = [[(0, kv) for kv in _chunks(int(KL2[t]), KCAP2)] +
           [(1, kv) for kv in _chunks(int(KH2[t]), KCAP2)] for t in range(NT)]

    pad_bits = np.float16(PAD_BIAS).view(np.int16)

    blobs1, blobs2 = [], []
    for k in range(NC):
        c = cores[k]
        g0 = k * SHARD
        # ---- blob1: [sd-lo idx | sd-hi idx | per-tile slot idx] ----
        cols = []
        for t in range(NT):
            rows = c["perm1"][t * TP:(t + 1) * TP]
            dlo = np.full(TP, dum1_lo, dtype=np.int64)
            dhi = np.full(TP, dum1_hi, dtype=np.int64)
            for p, r in enumerate(rows):
                if r >= 0:
                    n = g0 + r
                    if n < LO1:
                        dlo[p] = n
                    else:
                        dhi[p] = n - LO1
            cols.append(_wrap16(dlo))
            cols.append(_wrap16(dhi))
        sd_cols = [np.concatenate(cols[0::2], axis=1),
                   np.concatenate(cols[1::2], axis=1)]
        tile_cols = []
        for t in range(NT):
            rows = c["perm1"][t * TP:(t + 1) * TP]
            la, ha = int(KL1[t]), int(KH1[t])
            lo_mat = np.full((TP, max(1, la)), dum1_lo, dtype=np.int64)
            hi_mat = np.full((TP, max(1, ha)), dum1_hi, dtype=np.int64)
            for p, r in enumerate(rows):
                if r >= 0:
                    s = c["srcs"][r]
                    lo = s[s < LO1]
                    hi = s[s >= LO1] - LO1
                    lo_mat[p, :len(lo)] = lo
                    hi_mat[p, :len(hi)] = hi
            off_l = off_h = 0
            for half, kv in vt1[t]:
                if half == 0:
                    m = lo_mat[:, off_l:off_l + kv]
                    off_l += kv
                else:
                    m = hi_mat[:, off_h:off_h + kv]
                    off_h += kv
                tile_cols.append(_wrap16(m.T.reshape(-1)))
        blobs1.append(np.concatenate(sd_cols + tile_cols, axis=1))

        # ---- blob2: [sd2 idx (SH2 rows) | per-tile (slot idx + bias)] ----
        pos1_local = np.zeros(SHARD, dtype=np.int64)
        p1 = c["perm1"]
        for q in range(ROWS):
            if p1[q] >= 0:
                pos1_local[p1[q]] = q
        first_real = int(c["perm2"][0])
        cols = []
        for t in range(NT):
            rows = c["perm2"][t * TP:(t + 1) * TP]
            d2 = np.empty(TP, dtype=np.int64)
            for p, r in enumerate(rows):
                d2[p] = pos1_local[r if r >= 0 else first_real]
            cols.append(_wrap16(d2))
        tile_cols = [np.concatenate(cols, axis=1)]
        for t in range(NT):
            rows = c["perm2"][t * TP:(t + 1) * TP]
            la, ha = int(KL2[t]), int(KH2[t])
            lo_mat = np.zeros((TP, max(1, la)), dtype=np.int64)
            hi_mat = np.zeros((TP, max(1, ha)), dtype=np.int64)
            lo_msk = np.zeros((TP, max(1, la)), dtype=np.int16)
            hi_msk = np.zeros((TP, max(1, ha)), dtype=np.int16)
            lo_msk[:] = pad_bits
            hi_msk[:] = pad_bits
            for p, r in enumerate(rows):
                if r >= 0:
                    s = c["rows2"][r]
                    lo = s[s < LO2]
                    hi = s[s >= LO2] - LO2
                    lo_mat[p, :len(lo)] = lo
                    hi_mat[p, :len(hi)] = hi
                    lo_msk[p, :len(lo)] = 0
                    hi_msk[p, :len(hi)] = 0
            off_l = off_h = 0
            for half, kv in vt2[t]:
                if half == 0:
                    m, b = lo_mat[:, off_l:off_l + kv], lo_msk[:, off_l:off_l + kv]
                    off_l += kv
                else:
                    m, b = hi_mat[:, off_h:off_h + kv], hi_msk[:, off_h:off_h + kv]
                    off_h += kv
                tile_cols.append(_wrap16(m.T.reshape(-1)))
                tile_cols.append(b.astype(np.int16))   # [128, kv] plain layout
            # bias bits are fp16 PAD_BIAS for masked slots, 0 for real
        blobs2.append(np.concatenate(tile_cols, axis=1))

    return dict(cores=cores, blobs1=blobs1, blobs2=blobs2,
                vt1=vt1, vt2=vt2, KL1=KL1, KH1=KH1, KL2=KL2, KH2=KH2)


def _hm(mat, HEADS, HID):
    """Reorder columns from head-major (h*HID+c) to head-minor (c*HEADS+h)."""
    n = mat.shape[1] if mat.ndim == 2 else mat.shape[0]
    assert n == HEADS * HID
    idx = np.empty(n, dtype=np.int64)
    for h in range(HEADS):
        for c in range(HID):
            idx[c * HEADS + h] = h * HID + c
    return mat[:, idx] if mat.ndim == 2 else mat[idx]


def _pack_consts(cfg, W1, a1_src, a1_dst, b1, W2, a2_src, a2_dst, b2,
                 Wm1, bm1, Wm2, bm2):
    IN_DIM, HID, HEADS, OUT_DIM = (cfg["IN_DIM"], cfg["HID"], cfg["HEADS"],
                                   cfg["OUT_DIM"])
    W1R, W2R = cfg["W1R"], cfg["W2R"]
    U1 = np.einsum("khc,hc->kh", W1.reshape(IN_DIM, HEADS, HID), a1_src)
    V1 = np.einsum("khc,hc->kh", W1.reshape(IN_DIM, HEADS, HID), a1_dst)
    W1X = np.zeros((IN_DIM, W1R), dtype=np.float32)
    W1X[:, :IN_DIM] = _hm(W1, HEADS, HID)
    W1X[:, IN_DIM:IN_DIM + HEADS] = U1
    W1X[:, IN_DIM + HEADS:IN_DIM + 2 * HEADS] = V1
    u2 = W2 @ a2_src[0]
    v2 = W2 @ a2_dst[0]
    # rows of W2 permuted to head-minor order (o1T rows are head-minor)
    W2X = np.zeros((HEADS * HID, W2R), dtype=np.float32)
    W2X[:, :OUT_DIM] = W2
    W2X[:, OUT_DIM] = u2
    W2X[:, OUT_DIM + 1] = v2
    W2Xp = _hm(W2X.T, HEADS, HID).T
    P = 128
    blocks = {}
    cols = 0

    def add(name, arr):
        nonlocal cols
        a = np.zeros((P, arr.shape[1]), dtype=np.float16)
        a[:arr.shape[0]] = arr.astype(np.float16)
        blocks[name] = (cols, arr.shape[1])
        cols += arr.shape[1]
        return a

    parts = []
    parts.append(add("w1x0", W1X[0:P]))
    parts.append(add("w1x1", W1X[P:2 * P]))
    parts.append(add("w2x0", W2Xp[0:P]))
    parts.append(add("w2x1", W2Xp[P:2 * P]))
    parts.append(add("wm1", Wm1.astype(np.float32)))
    parts.append(add("wm2", Wm2.astype(np.float32)))
    parts.append(add("ident", np.eye(P, dtype=np.float32)))
    parts.append(add("b1r", np.tile(_hm(b1.astype(np.float32)[None, :],
                                        HEADS, HID), (P, 1))))
    parts.append(add("b2r", np.tile(b2.astype(np.float32), (P, 1))))
    parts.append(add("bm1r", np.tile(bm1.astype(np.float32), (P, 1))))
    parts.append(add("bm2r", np.tile(bm2.astype(np.float32), (P, 1))))
    parts.append(add("nshift", np.full((P, 1), -SHIFT, dtype=np.float32)))
    consts = np.concatenate(parts, axis=1)
    # dummy rows: [hx1_lo, hx1_hi] in a [2, W1R] array; ssrc cols poisoned
    dums = np.zeros((2, W1R), dtype=np.float16)
    dums[:, IN_DIM:IN_DIM + HEADS] = PAD_BIAS
    return consts, blocks, dums


# ------------------------------------------------------------- device build

def _build(cfg, prep, cblocks, CW, phase="full", sim_collective=False):
    N, NC = cfg["N"], cfg["NC"]
    IN_DIM, HID, HEADS, OUT_DIM = (cfg["IN_DIM"], cfg["HID"], cfg["HEADS"],
                                   cfg["OUT_DIM"])
    TP, NT, ROWS, SHARD = cfg["TP"], cfg["NT"], cfg["ROWS"], cfg["SHARD"]
    W1R, LO1 = cfg["W1R"], cfg["LO1"]
    W2R, LO2 = cfg["W2R"], cfg["LO2ROWS"]
    CHT, NCH, CHROWS, CH_ALL = (cfg["CHT"], cfg["NCH"], cfg["CHROWS"],
                                cfg["CH_ALL"])
    HX1R, HX2R = cfg["HX1_ROWS"], cfg["HX2_ROWS"]
    vt1, vt2 = prep["vt1"], prep["vt2"]
    C1 = prep["blobs1"][0].shape[1]
    C2 = prep["blobs2"][0].shape[1]
    NH2 = HEADS * HID
    P = 128

    nc = bacc.Bacc("TRN2", target_bir_lowering=False, debug=False,
                   num_devices=NC)
    xT = nc.dram_tensor("xT", [IN_DIM, N], F16, kind="ExternalInput")
    consts = nc.dram_tensor("consts", [128, CW], F16, kind="ExternalInput")
    dums = nc.dram_tensor("dums", [2, W1R], F16, kind="ExternalInput")
    idx1 = nc.dram_tensor("idx1", [128, C1], I16, kind="ExternalInput")
    idx2 = nc.dram_tensor("idx2", [128, C2], I16, kind="ExternalInput")
    out = nc.dram_tensor("out", [ROWS, OUT_DIM], F32, kind="ExternalOutput")
    dbg = nc.dram_tensor("dbg", [128, 512], F32, kind="ExternalOutput")

    HX1 = nc.dram_tensor("HX1", [HX1R, W1R], F16)
    SH2 = nc.dram_tensor("SH2", [ROWS, W2R], F16)
    HX2 = nc.dram_tensor("HX2", [HX2R, W2R], F16, addr_space="Shared")

    hx1_lo = HX1[0:LO1 + 1, :]
    hx1_hi = HX1[LO1 + 1:HX1R, :]
    hx2_lo = HX2[0:LO2, :]
    hx2_hi = HX2[LO2:HX2R, :]

    with tile.TileContext(nc) as tc:
        nc.gpsimd.load_library(library_config.mlp)
        with tc.tile_pool(name="cp", bufs=1) as cp:
            cb = cp.tile([128, CW], F16, tag="consts")
            nc.sync.dma_start(cb[:, :], consts[:, :])

            def C(name):
                off, w = cblocks[name]
                return cb[:, off:off + w]

            nc.sync.dma_start(HX1[LO1:LO1 + 1, :], dums[0:1, :])
            nc.sync.dma_start(HX1[HX1R - 1:HX1R, :], dums[1:2, :])

            # ---------------- P0: full HX1 table (fp16) -----------------
            SB = 1024
            nsb = -(-N // SB)
            with (
                tc.tile_pool(name="p0", bufs=2) as p0,
                tc.tile_pool(name="p0ps", bufs=1, space="PSUM") as p0ps,
            ):
                for sb in range(nsb):
                    base = sb * SB
                    cnt = min(SB, N - base)
                    nq = -(-cnt // P)
                    xt = p0.tile([P, 2 * cnt], F16, tag="xt")
                    xtv = xt[:, :].rearrange("p (q c) -> p q c", q=2)
                    nc.sync.dma_start(
                        xtv,
                        xT[:, base:base + cnt].rearrange(
                            "(q p) c -> p q c", p=P))
                    for g in range(2):
                        q0 = g * 4
                        gq = min(4, nq - q0)
                        if gq <= 0:
                            continue
                        ps = p0ps.tile([P, 4 * 512], F32, tag=f"ps{g}")
                        psv = ps[:, :].rearrange("p (q c) -> p q c", q=4)
                        for qi in range(gq):
                            q = q0 + qi
                            pb = min(P, cnt - q * P)
                            nc.tensor.matmul(psv[0:pb, qi, 0:W1R],
                                             xtv[:, 0, q * P:q * P + pb],
                                             C("w1x0"), start=True, stop=False)
                            nc.tensor.matmul(psv[0:pb, qi, 0:W1R],
                                             xtv[:, 1, q * P:q * P + pb],
                                             C("w1x1"), start=False, stop=True)
                        hx4 = p0.tile([P, gq * W1R], F16, tag=f"hx4{g}")
                        hx4v = hx4[:, :].rearrange("p (q w) -> p q w", q=gq)
                        hq = (gq + 1) // 2
                        nc.scalar.copy(hx4v[:, 0:hq, :], psv[:, 0:hq, 0:W1R])
                        if gq > hq:
                            nc.vector.tensor_scalar(
                                hx4v[:, hq:gq, :], psv[:, hq:gq, 0:W1R],
                                0.0, None, op0=OP.add)
                        gb = base + q0 * P
                        gcnt = min(4 * P, cnt - q0 * P)

                        def wr(a, b):   # node range [a, b) within group
                            if a >= b:
                                return
                            ra = gb + a + (1 if gb + a >= LO1 else 0)
                            dv = HX1[ra:ra + (b - a), :]
                            qa, pa = divmod(a, P)
                            qb, pb_ = divmod(b - 1, P)
                            if qa == qb:
                                nc.sync.dma_start(dv, hx4v[pa:pb_ + 1, qa, :])
                                return
                            if pa == 0 and pb_ == P - 1:
                                nc.sync.dma_start(
                                    dv.rearrange("(q p) w -> p q w", p=P),
                                    hx4v[:, qa:qb + 1, :])
                                return
                            n0 = P - pa
                            nc.sync.dma_start(dv[0:n0, :], hx4v[pa:P, qa, :])
                            off = n0
                            for q in range(qa + 1, qb):
                                nc.sync.dma_start(dv[off:off + P, :],
                                                  hx4v[0:P, q, :])
                                off += P
                            nc.sync.dma_start(dv[off:, :],
                                              hx4v[0:pb_ + 1, qb, :])

                        if gb < LO1 < gb + gcnt:
                            wr(0, LO1 - gb)
                            wr(LO1 - gb, gcnt)
                        else:
                            wr(0, gcnt)

            if phase == "p0":
                nc.sync.dma_start(dbg[0:128, 0:W1R].bitcast(F16),
                                  HX1[0:128, :])

            # ---------------- L1 + H2 + chunked AllGather ----------------
            _l2r_cm = tc.tile_pool(name="l2r", bufs=1)
            l2r = _l2r_cm.__enter__()
            IT2 = l2r.tile([128, C2], I16, tag="it2")
            nc.sync.dma_start(IT2[:, :], idx2[:, :])
            with (
                tc.tile_pool(name="l1r", bufs=1) as l1r,
                tc.tile_pool(name="l1", bufs=3) as l1,
                tc.tile_pool(name="l1b", bufs=3) as l1b,
                tc.tile_pool(name="l1ps", bufs=2, space="PSUM") as l1ps,
            ):
                IT1 = l1r.tile([128, C1], I16, tag="it1")
                nc.sync.dma_start(IT1[:, :], idx1[:, :])
                sd4 = l1r.tile([P, NT * HEADS], F16, tag="sd4")
                with tc.tile_pool(name="sdp", bufs=1) as sdp:
                    sdl = sdp.tile([P, NT * 128], F16, tag="sdl")
                    sdh = sdp.tile([P, NT * 128], F16, tag="sdh")
                    nc.gpsimd.dma_gather(
                        sdl[:, :].rearrange("p (t w) -> p t w", t=NT),
                        hx1_lo[:, IN_DIM:IN_DIM + 128], IT1[:, 0:NT * 8],
                        NT * P, NT * P, 128, elem_step=W1R,
                        single_packet=False)
                    nc.gpsimd.dma_gather(
                        sdh[:, :].rearrange("p (t w) -> p t w", t=NT),
                        hx1_hi[:, IN_DIM:IN_DIM + 128], IT1[:, NT * 8:NT * 16],
                        NT * P, NT * P, 128, elem_step=W1R,
                        single_packet=False)
                    sdlv = sdl[:, :].rearrange("p (t w) -> p t w", t=NT)
                    sdhv = sdh[:, :].rearrange("p (t w) -> p t w", t=NT)
                    nc.vector.tensor_tensor(
                        sd4[:, :].rearrange("p (t h) -> p t h", t=NT),
                        sdlv[:, :, HEADS:2 * HEADS],
                        sdhv[:, :, HEADS:2 * HEADS], op=OP.add)

                col = [2 * NT * 8]
                for t in (range(NT) if phase != "p0" else range(0)):
                    num = l1b.tile([P, NH2], F16, tag="num")
                    nc.vector.memset(num[:, :], 0.0)
                    den = l1b.tile([P, HEADS], F32, tag="den")
                    nc.vector.memset(den[:, :], 0.0)
                    def l1_mac(hg, hgv, w, kv):
                        dv = l1b.tile([P, HEADS], F32, tag="dv")
                        nc.vector.tensor_reduce(
                            dv[:, :],
                            w[:, :].rearrange("p (j h) -> p h j", j=kv),
                            axis=AX.X, op=OP.add)
                        nc.vector.tensor_tensor(den[:, :], den[:, :],
                                                dv[:, :], op=OP.add)
                        nc.vector.tensor_tensor(
                            hgv[:, :, 0:NH2].rearrange(
                                "p j (c h) -> p j c h", h=HEADS),
                            hgv[:, :, 0:NH2].rearrange(
                                "p j (c h) -> p j c h", h=HEADS),
                            w[:, :].rearrange("p (j h) -> p j h", j=kv)
                            .unsqueeze(2).broadcast_to([P, kv, HID, HEADS]),
                            op=OP.mult)
                        n = kv
                        while n > 1:
                            if n % 2 == 1:
                                nc.vector.tensor_tensor(
                                    num[:, :], num[:, :],
                                    hgv[:, n - 1, 0:NH2], op=OP.add)
                                n -= 1
                            pairs = n // 2
                            hp = hg[:, 0:pairs * 2 * W1R].rearrange(
                                "p (j two w) -> p j two w", two=2, w=W1R)
                            nc.vector.tensor_tensor(
                                hgv[:, 0:pairs, 0:NH2],
                                hp[:, :, 0, 0:NH2], hp[:, :, 1, 0:NH2],
                                op=OP.add)
                            n = pairs
                        nc.vector.tensor_tensor(num[:, :], num[:, :],
                                                hgv[:, 0, 0:NH2], op=OP.add)

                    pend = None
                    for v, (half, kv) in enumerate(vt1[t]):
                        itv = IT1[:, col[0]:col[0] + kv * 8]
                        col[0] += kv * 8
                        hg = l1.tile([P, kv * W1R], F16, tag="hg")
                        nc.gpsimd.dma_gather(
                            hg[:, :].rearrange("p (j w) -> p j w", j=kv),
                            (hx1_lo if half == 0 else hx1_hi)[:, :],
                            itv, P * kv, P * kv, W1R, single_packet=False)
                        hgv = hg[:, :].rearrange("p (j w) -> p j w", j=kv)
                        s = l1b.tile([P, kv * HEADS], F16, tag="s")
                        nc.vector.tensor_tensor(
                            s[:, :].rearrange("p (j h) -> p j h", j=kv),
                            hgv[:, :, IN_DIM:IN_DIM + HEADS],
                            sd4[:, t * HEADS:(t + 1) * HEADS].unsqueeze(1)
                            .broadcast_to([P, kv, HEADS]), op=OP.add)
                        w0 = l1b.tile([P, kv * HEADS], F16, tag="w0")
                        nc.vector.scalar_tensor_tensor(
                            w0[:, :], s[:, :], NEG_SLOPE, s[:, :],
                            op0=OP.mult, op1=OP.max)
                        w = l1b.tile([P, kv * HEADS], F16, tag="w")
                        nc.scalar.activation(w[:, :], w0[:, :], AF.Exp,
                                             bias=C("nshift"))
                        if pend is not None:
                            l1_mac(*pend)
                        pend = (hg, hgv, w, kv)
                    if pend is not None:
                        l1_mac(*pend)
                    dinv = l1b.tile([P, HEADS], F32, tag="dinv")
                    nc.vector.tensor_scalar_max(dinv[:, :], den[:, :], 1e-6)
                    nc.vector.reciprocal(dinv[:, :], dinv[:, :])
                    o = l1b.tile([P, NH2], F16, tag="o")
                    nc.vector.tensor_tensor(
                        o[:, :].rearrange("p (c h) -> p c h", h=HEADS),
                        num[:, :].rearrange("p (c h) -> p c h", h=HEADS),
                        dinv[:, :].unsqueeze(1).broadcast_to(
                            [P, HID, HEADS]), op=OP.mult)
                    nc.vector.tensor_tensor(o[:, :], o[:, :], C("b1r"),
                                            op=OP.add)
                    # elu
                    m0 = l1b.tile([P, NH2], F16, tag="m0")
                    nc.vector.tensor_scalar_min(m0[:, :], o[:, :], 0.0)
                    em = l1b.tile([P, NH2], F16, tag="em")
                    nc.scalar.activation(em[:, :], m0[:, :], AF.Exp)
                    p1 = l1b.tile([P, NH2], F16, tag="p1")
                    nc.vector.tensor_scalar(p1[:, :], o[:, :], 0.0, -1.0,
                                            op0=OP.max, op1=OP.add)
                    eo = l1b.tile([P, NH2], F16, tag="eo")
                    nc.vector.tensor_tensor(eo[:, :], em[:, :], p1[:, :],
                                            op=OP.add)
                    # transpose + H2 matmul
                    pt = l1ps.tile([P, NH2], F16, tag="pt")
                    nc.tensor.transpose(pt[:, 0:P], eo[:, 0:P], C("ident"))
                    nc.tensor.transpose(pt[:, P:NH2], eo[:, P:NH2],
                                        C("ident"))
                    o1T = l1b.tile([P, NH2], F16, tag="o1T")
                    nc.scalar.copy(o1T[:, :], pt[:, :])
                    h2p = l1ps.tile([P, W2R], F32, tag="h2p")
                    nc.tensor.matmul(h2p[:, :], o1T[:, 0:P], C("w2x0"),
                                     start=True, stop=False)
                    nc.tensor.matmul(h2p[:, :], o1T[:, P:NH2], C("w2x1"),
                                     start=False, stop=True)
                    sh2 = l1b.tile([P, W2R], F16, tag="sh2")
                    nc.scalar.copy(sh2[:, :], h2p[:, :])
                    nc.sync.dma_start(SH2[t * P:(t + 1) * P, :], sh2[:, :])

                    if (t + 1) % CHT == 0 and phase == "full":
                        c = t // CHT
                        base = c * CH_ALL
                        if sim_collective:
                            for r in range(NC):
                                nc.sync.dma_start(
                                    HX2[base + r * CHROWS:
                                        base + (r + 1) * CHROWS, :],
                                    SH2[c * CHROWS:(c + 1) * CHROWS, :])
                        else:
                            nc.gpsimd.collective_compute(
                                "AllGather", OP.bypass,
                                replica_groups=[list(range(NC))],
                                ins=[SH2[c * CHROWS:(c + 1) * CHROWS, :].opt()],
                                outs=[HX2[base:base + CH_ALL, :].opt()],
                            )

            if phase == "l1":
                nc.sync.dma_start(dbg[0:128, 0:W2R].bitcast(F16),
                                  SH2[0:128, :])
            # ---------------- L2 + MLP + normalize ----------------------
            with (
                tc.tile_pool(name="l2", bufs=3) as l2,
                tc.tile_pool(name="l2b", bufs=4) as l2b,
                tc.tile_pool(name="l2ps", bufs=4, space="PSUM") as l2ps,
            ):
                sd2g = l2r.tile([P, NT * 128], F16, tag="sd2g")
                nc.gpsimd.dma_gather(
                    sd2g[:, :].rearrange("p (t w) -> p t w", t=NT),
                    SH2[:, OUT_DIM:OUT_DIM + 128], IT2[:, 0:NT * 8],
                    NT * P, NT * P, 128, elem_step=W2R, single_packet=False)
                sd2v = sd2g[:, :].rearrange("p (t w) -> p t w", t=NT)
                sd2f = l2r.tile([P, NT], F32, tag="sd2f")
                nc.vector.tensor_scalar(sd2f[:, :].unsqueeze(2),
                                        sd2v[:, :, 1:2], 0.0, None,
                                        op0=OP.add)

                o2a = l2r.tile([P, NT * OUT_DIM], F16, tag="o2a")
                magic = l2r.tile([P, 1], I32, tag="magic")
                nc.vector.memset(magic[:, :], 0x5F3759DF)
                col2 = [NT * 8]
                for t in (range(NT) if phase == "full" else range(0)):
                    num2 = l2b.tile([P, OUT_DIM], F16, tag="num2")
                    nc.vector.memset(num2[:, :], 0.0)
                    den2 = l2b.tile([P, 1], F32, tag="den2")
                    nc.vector.memset(den2[:, :], 0.0)
                    def l2_mac(hg, hgv, w, dv, kv):
                        nc.vector.tensor_tensor(den2[:, :], den2[:, :],
                                                dv[:, :], op=OP.add)
                        for j in range(kv):
                            nc.vector.tensor_scalar(
                                hgv[:, j, 0:OUT_DIM], hgv[:, j, 0:OUT_DIM],
                                w[:, j:j + 1], None, op0=OP.mult)
                        n = kv
                        while n > 1:
                            if n % 2 == 1:
                                nc.vector.tensor_tensor(
                                    num2[:, :], num2[:, :],
                                    hgv[:, n - 1, 0:OUT_DIM], op=OP.add)
                                n -= 1
                            pairs = n // 2
                            hp = hg[:, 0:pairs * 2 * W2R].rearrange(
                                "p (j two w) -> p j two w", two=2, w=W2R)
                            nc.vector.tensor_tensor(
                                hgv[:, 0:pairs, 0:OUT_DIM],
                                hp[:, :, 0, 0:OUT_DIM], hp[:, :, 1, 0:OUT_DIM],
                                op=OP.add)
                            n = pairs
                        nc.vector.tensor_tensor(num2[:, :], num2[:, :],
                                                hgv[:, 0, 0:OUT_DIM], op=OP.add)

                    pend = None
                    for v, (half, kv) in enumerate(vt2[t]):
                        itv = IT2[:, col2[0]:col2[0] + kv * 8]
                        col2[0] += kv * 8
                        ib = IT2[:, col2[0]:col2[0] + kv].bitcast(F16)
                        col2[0] += kv
                        hg = l2.tile([P, kv * W2R], F16, tag="hg2")
                        nc.gpsimd.dma_gather(
                            hg[:, :].rearrange("p (j w) -> p j w", j=kv),
                            (hx2_lo if half == 0 else hx2_hi)[:, :],
                            itv, P * kv, P * kv, W2R, single_packet=False)
                        hgv = hg[:, :].rearrange("p (j w) -> p j w", j=kv)
                        s = l2b.tile([P, kv], F16, tag="s2")
                        nc.vector.tensor_scalar(
                            s[:, :].unsqueeze(2),
                            hgv[:, :, OUT_DIM:OUT_DIM + 1],
                            sd2f[:, t:t + 1], None, op0=OP.add)
                        nc.vector.tensor_tensor(s[:, :], s[:, :], ib,
                                                op=OP.add)
                        w0 = l2b.tile([P, kv], F16, tag="w02")
                        nc.vector.scalar_tensor_tensor(
                            w0[:, :], s[:, :], NEG_SLOPE, s[:, :],
                            op0=OP.mult, op1=OP.max)
                        w = l2b.tile([P, kv], F32, tag="w2")
                        dv = l2b.tile([P, 1], F32, tag="dv2")
                        nc.scalar.activation(w[:, :], w0[:, :], AF.Exp,
                                             bias=C("nshift"),
                                             accum_out=dv[:, :])
                        if pend is not None:
                            l2_mac(*pend)
                        pend = (hg, hgv, w, dv, kv)
                    if pend is not None:
                        l2_mac(*pend)
                    dinv = l2b.tile([P, 1], F32, tag="dinv2")
                    nc.vector.tensor_scalar_max(dinv[:, :], den2[:, :], 1e-6)
                    nc.vector.reciprocal(dinv[:, :], dinv[:, :])
                    o2 = o2a[:, t * OUT_DIM:(t + 1) * OUT_DIM]
                    nc.vector.tensor_scalar(o2, num2[:, :],
                                            dinv[:, 0:1], None, op0=OP.mult)
                    nc.vector.tensor_tensor(o2, o2, C("b2r"), op=OP.add)

                for t in (range(NT) if phase == "full" else range(0)):
                    o2 = o2a[:, t * OUT_DIM:(t + 1) * OUT_DIM]
                    # MLP (all PSUM packed into one bank per tile)
                    mlp = l2ps.tile([P, 512], F32, tag="mlp")
                    pt2 = mlp[:, 0:64].bitcast(F16)
                    nc.tensor.transpose(pt2, o2, C("ident"))
                    o2T = l2b.tile([P, P], F16, tag="o2T")
                    nc.scalar.copy(o2T[:, :], pt2)
                    h3p = mlp[:, 64:128]
                    nc.tensor.matmul(h3p, o2T[:, :], C("wm1"),
                                     start=True, stop=True)
                    h3 = l2b.tile([P, HID], F16, tag="h3")
                    nc.vector.tensor_tensor(h3[:, :], h3p, C("bm1r"),
                                            op=OP.add)
                    nc.vector.tensor_scalar_max(h3[:, :], h3[:, :], 0.0)
                    pt3 = mlp[0:HID, 128:192].bitcast(F16)
                    nc.tensor.transpose(pt3, h3[:, :], C("ident"))
                    h3T = l2b.tile([HID, P], F16, tag="h3T")
                    nc.scalar.copy(h3T[:, :], pt3)
                    h4p = mlp[:, 256:384]
                    nc.tensor.matmul(h4p, h3T[0:HID, :],
                                     C("wm2")[0:HID, :], start=True, stop=True)
                    h4 = l2b.tile([P, OUT_DIM], F32, tag="h4")
                    nc.vector.tensor_tensor(h4[:, :], h4p, C("bm2r"),
                                            op=OP.add)
                    hsq = l2b.tile([P, OUT_DIM], F32, tag="hsq")
                    nc.vector.tensor_tensor(hsq[:, :], h4[:, :], h4[:, :],
                                            op=OP.mult)
                    n2 = l2b.tile([P, 1], F32, tag="n2")
                    nc.vector.tensor_reduce(n2[:, :], hsq[:, :], axis=AX.X,
                                            op=OP.add)
                    nin = l2b.tile([P, 1], F32, tag="nin")
                    nc.vector.tensor_scalar_max(nin[:, :], n2[:, :], 1e-12)
                    # Newton rsqrt (keeps Sqrt off the Act engine: avoids
                    # act-table thrash between Exp and Sqrt function sets)
                    yr = l2b.tile([P, 1], F32, tag="yr")
                    tn = l2b.tile([P, 1], F32, tag="tn")
                    nc.vector.tensor_scalar(yr[:, :].bitcast(I32),
                                            nin[:, :].bitcast(I32), 1, None,
                                            op0=OP.arith_shift_right)
                    nc.vector.tensor_tensor(yr[:, :].bitcast(I32),
                                            magic[:, :],
                                            yr[:, :].bitcast(I32),
                                            op=OP.subtract)
                    for _ in range(3):
                        nc.vector.tensor_tensor(tn[:, :], yr[:, :], yr[:, :],
                                                op=OP.mult)
                        nc.vector.tensor_tensor(tn[:, :], tn[:, :], nin[:, :],
                                                op=OP.mult)
                        nc.vector.tensor_scalar(tn[:, :], tn[:, :], -0.5, 1.5,
                                                op0=OP.mult, op1=OP.add)
                        nc.vector.tensor_tensor(yr[:, :], yr[:, :], tn[:, :],
                                                op=OP.mult)
                    nin = yr
                    of = l2b.tile([P, OUT_DIM], F32, tag="of")
                    nc.vector.tensor_scalar(of[:, :], h4[:, :], nin[:, 0:1],
                                            None, op0=OP.mult)
                    nc.sync.dma_start(out[t * P:(t + 1) * P, :], of[:, :])
            _l2r_cm.__exit__(None, None, None)

    nc.compile()
    return nc


# ------------------------------------------------------------------ driver

def run(cfg, inputs, trace=False, phase="full", sim_collective=False,
        prep=None):
    x = np.asarray(inputs["x"], dtype=np.float32)
    edge_index = np.asarray(inputs["edge_index"])
    if prep is None:
        prep = _prep(cfg, edge_index)
    consts, cblocks, dums = _pack_consts(
        cfg, *[np.asarray(inputs[k], dtype=np.float32) for k in
               ("W1", "a1_src", "a1_dst", "b1", "W2", "a2_src", "a2_dst",
                "b2", "Wm1", "bm1", "Wm2", "bm2")])
    nc = _build(cfg, prep, cblocks, consts.shape[1], phase=phase,
                sim_collective=sim_collective)
    xT = np.ascontiguousarray(x.T.astype(np.float16))
    in_maps = []
    for k in range(cfg["NC"]):
        in_maps.append({
            "xT": xT, "consts": consts, "dums": dums,
            "idx1": np.ascontiguousarray(prep["blobs1"][k]),
            "idx2": np.ascontiguousarray(prep["blobs2"][k]),
        })
    res = run_bass_kernel_spmd(nc, in_maps, list(range(cfg["NC"])),
                               trace=trace)
    N, NC, SHARD = cfg["N"], cfg["NC"], cfg["SHARD"]
    full = np.zeros((N, cfg["OUT_DIM"]), dtype=np.float32)
    for k in range(NC):
        o = res.results[k]["out"]
        perm2 = prep["cores"][k]["perm2"]
        real = perm2 >= 0
        full[k * SHARD + perm2[real]] = o[real]
    return full, res


def kernel(**inputs):
    cfg = make_cfg()
    full, _ = run(cfg, inputs, trace=False)
    return full


# revision 18
# speedup vs baseline: 1.9086x; 1.0375x over previous
"""Trainium2 Bass kernel for a 2-layer GAT + MLP (nn_MemoryGNN).

Strategy (8 NeuronCores, SPMD):
  - Destination-node partition: core k owns dst nodes [k*6250, (k+1)*6250).
  - Every core computes the FULL HX1 table (fp16): row n -> [h1 (256,
    head-minor c*4+h) | ssrc (4) | sdst (4) | pad] = 384 cols (768B rows),
    so layer-1 message gathers are purely local.
  - Per-edge gathers use gpsimd.dma_gather from per-dst-tile slot tables
    (host-precomputed int16 index blobs, degree-sorted tiles).
  - Softmax is computed unnormalized with a global exp-shift of -2 (folded
    into the Act-engine exp bias); the shift cancels in num/den.
  - fp16 everywhere in the hot loops (DVE 2x eligible); accumulation of the
    weighted message sum is an fp16 add-chain (err ~0.1% << 2e-2 tol).
  - Layer 2 (fp16 512B rows) needs remote H2 rows: a chunked AllGather of
    per-core SH2 shards into HX2 (Shared) overlaps with layer-1 compute.
  - Output rows are produced in a degree-sorted permuted order; the host
    applies the inverse permutation (free).
"""

import sys
import numpy as np

for _p in ("/opt/trn_rl_repo", "/root/.axon_site/_ro/trn_rl_repo"):
    if _p not in sys.path:
        sys.path.insert(0, _p)

import concourse.bass as bass
import concourse.bacc as bacc
import concourse.mybir as mybir
import concourse.tile as tile
from concourse import library_config
from concourse.bass_utils import run_bass_kernel_spmd

F32 = mybir.dt.float32
F16 = mybir.dt.float16
I16 = mybir.dt.int16
I32 = mybir.dt.int32
AF = mybir.ActivationFunctionType
OP = mybir.AluOpType
AX = mybir.AxisListType

NEG_SLOPE = 0.2
SHIFT = 2.0          # global exp shift (cancels in num/den)
PAD_BIAS = -30000.0  # fp16-safe "minus infinity" for masked slots


def make_cfg(N=50000, E=1000000, IN_DIM=256, HID=64, HEADS=4, OUT_DIM=128,
             NC=8, CHT=7, KCAP1=48, KCAP2=64):
    cfg = dict(N=N, E=E, IN_DIM=IN_DIM, HID=HID, HEADS=HEADS, OUT_DIM=OUT_DIM,
               NC=NC, CHT=CHT, KCAP1=KCAP1, KCAP2=KCAP2)
    cfg["SHARD"] = N // NC
    assert N % NC == 0
    TP = 128
    cfg["TP"] = TP
    NT = -(-cfg["SHARD"] // TP)
    cfg["NT"] = NT
    assert NT % CHT == 0, (NT, CHT)
    cfg["NCH"] = NT // CHT
    cfg["ROWS"] = NT * TP
    cfg["CHROWS"] = CHT * TP          # SH2 rows per AllGather chunk
    # layer-1 table: row n -> n + (n >= LO1); dummies at LO1 and last row
    cfg["D1"] = IN_DIM + 2 * HEADS
    cfg["W1R"] = 384
    assert cfg["W1R"] * 2 % 256 == 0
    cfg["LO1"] = 25024
    assert cfg["LO1"] + 1 <= 32767 and N - cfg["LO1"] + 1 <= 32767
    cfg["HX1_ROWS"] = N + 2
    # layer-2 table (chunk-major, no dummies; masked pads instead)
    cfg["D2"] = OUT_DIM + 2
    cfg["W2R"] = 256
    CH_ALL = cfg["CHROWS"] * NC       # global rows per chunk
    cfg["CH_ALL"] = CH_ALL
    LOCH = 32768 // CH_ALL            # chunks fully addressable as lo
    LOCH = max(1, min(cfg["NCH"] - 1, LOCH))
    cfg["LOCH"] = LOCH
    cfg["LO2ROWS"] = LOCH * CH_ALL
    assert cfg["LO2ROWS"] <= 32767
    assert (cfg["NCH"] - LOCH) * CH_ALL <= 32767
    cfg["HX2_ROWS"] = cfg["NCH"] * CH_ALL
    return cfg


# ----------------------------------------------------------------- host prep

def _wrap16(flat):
    """flat int array (len divisible by 16) -> wrapped [128, n/16] int16."""
    w = flat.reshape(-1, 16).T.astype(np.int16)
    return np.tile(w, (8, 1))


def _chunks(total, cap):
    out = []
    off = 0
    while off < total:
        kv = min(cap, total - off)
        out.append(kv)
        off += kv
    return out


def _sort_core(cfg, lo_cnt, hi_cnt):
    """Degree-sort local dsts into tiles; return perm and per-tile maxima."""
    SHARD, TP, NT = cfg["SHARD"], cfg["TP"], cfg["NT"]
    order = np.lexsort((-hi_cnt, -(lo_cnt // 6)))
    perm = np.full(NT * TP, -1, dtype=np.int64)
    perm[:SHARD] = order
    kl_t = np.zeros(NT, dtype=np.int64)
    kh_t = np.zeros(NT, dtype=np.int64)
    for t in range(NT):
        rows = perm[t * TP:(t + 1) * TP]
        real = rows[rows >= 0]
        if len(real):
            kl_t[t] = lo_cnt[real].max()
            kh_t[t] = hi_cnt[real].max()
    return perm, kl_t, kh_t


def _prep(cfg, edge_index):
    """Host preprocessing (structure only). Returns per-core blobs + meta."""
    N, NC, SHARD = cfg["N"], cfg["NC"], cfg["SHARD"]
    TP, NT, ROWS = cfg["TP"], cfg["NT"], cfg["ROWS"]
    KCAP1, KCAP2 = cfg["KCAP1"], cfg["KCAP2"]
    LO1 = cfg["LO1"]
    src = np.concatenate([np.asarray(edge_index[0]),
                          np.arange(N)]).astype(np.int64)
    dst = np.concatenate([np.asarray(edge_index[1]),
                          np.arange(N)]).astype(np.int64)
    order = np.argsort(dst, kind="stable")
    src_s = src[order]
    deg = np.bincount(dst, minlength=N)
    rp = np.zeros(N + 1, dtype=np.int64)
    np.cumsum(deg, out=rp[1:])

    # L1 table indexing: lo rows = node n (< LO1), dummy at LO1;
    # hi rows = node n - LO1 (n >= LO1), dummy at N - LO1 + 1 - 1
    dum1_lo = LO1
    dum1_hi = cfg["HX1_ROWS"] - 1 - (LO1 + 1)

    cores = []
    for k in range(NC):
        g0 = k * SHARD
        srcs = [src_s[rp[g0 + r]:rp[g0 + r + 1]] for r in range(SHARD)]
        lo_cnt = np.array([int((s < LO1).sum()) for s in srcs])
        hi_cnt = np.array([len(s) for s in srcs]) - lo_cnt
        perm1, kl1, kh1 = _sort_core(cfg, lo_cnt, hi_cnt)
        cores.append(dict(srcs=srcs, perm1=perm1, kl1=kl1, kh1=kh1))

    KL1 = np.max([c["kl1"] for c in cores], axis=0)
    KH1 = np.max([c["kh1"] for c in cores], axis=0)
    vt1 = [[(0, kv) for kv in _chunks(int(KL1[t]), KCAP1)] +
           [(1, kv) for kv in _chunks(int(KH1[t]), KCAP1)] for t in range(NT)]

    # position of each global node in its owner's tile order
    pos1 = np.empty(N, dtype=np.int64)
    for k in range(NC):
        p = cores[k]["perm1"]
        q = np.arange(ROWS)
        real = p >= 0
        pos1[k * SHARD + p[real]] = q[real]
    CHROWS, CH_ALL = cfg["CHROWS"], cfg["CH_ALL"]
    c_of = pos1 // CHROWS
    r_of = pos1 % CHROWS
    owner = np.arange(N) // SHARD
    cm = c_of * CH_ALL + owner * CHROWS + r_of      # HX2 row of node
    LO2 = cfg["LO2ROWS"]

    for k in range(NC):
        c = cores[k]
        rows2 = [cm[s] for s in c["srcs"]]
        lo2 = np.array([int((r < LO2).sum()) for r in rows2])
        hi2 = np.array([len(r) for r in rows2]) - lo2
        perm2, kl2, kh2 = _sort_core(cfg, lo2, hi2)
        c.update(rows2=rows2, perm2=perm2, kl2=kl2, kh2=kh2)

    KL2 = np.max([c["kl2"] for c in cores], axis=0)
    KH2 = np.max([c["kh2"] for c in cores], axis=0)
    vt2 = [[(0, kv) for kv in _chunks(int(KL2[t]), KCAP2)] +
           [(1, kv) for kv in _chunks(int(KH2[t]), KCAP2)] for t in range(NT)]

    pad_bits = np.float16(PAD_BIAS).view(np.int16)

    blobs1, blobs2 = [], []
    for k in range(NC):
        c = cores[k]
        g0 = k * SHARD
        # ---- blob1: [sd-lo idx | sd-hi idx | per-tile slot idx] ----
        cols = []
        for t in range(NT):
            rows = c["perm1"][t * TP:(t + 1) * TP]
            dlo = np.full(TP, dum1_lo, dtype=np.int64)
            dhi = np.full(TP, dum1_hi, dtype=np.int64)
            for p, r in enumerate(rows):
                if r >= 0:
                    n = g0 + r
                    if n < LO1:
                        dlo[p] = n
                    else:
                        dhi[p] = n - LO1
            cols.append(_wrap16(dlo))
            cols.append(_wrap16(dhi))
        sd_cols = [np.concatenate(cols[0::2], axis=1),
                   np.concatenate(cols[1::2], axis=1)]
        tile_cols = []
        for t in range(NT):
            rows = c["perm1"][t * TP:(t + 1) * TP]
            la, ha = int(KL1[t]), int(KH1[t])
            lo_mat = np.full((TP, max(1, la)), dum1_lo, dtype=np.int64)
            hi_mat = np.full((TP, max(1, ha)), dum1_hi, dtype=np.int64)
            for p, r in enumerate(rows):
                if r >= 0:
                    s = c["srcs"][r]
                    lo = s[s < LO1]
                    hi = s[s >= LO1] - LO1
                    lo_mat[p, :len(lo)] = lo
                    hi_mat[p, :len(hi)] = hi
            off_l = off_h = 0
            for half, kv in vt1[t]:
                if half == 0:
                    m = lo_mat[:, off_l:off_l + kv]
                    off_l += kv
                else:
                    m = hi_mat[:, off_h:off_h + kv]
                    off_h += kv
                tile_cols.append(_wrap16(m.T.reshape(-1)))
        blobs1.append(np.concatenate(sd_cols + tile_cols, axis=1))

        # ---- blob2: [sd2 idx (SH2 rows) | per-tile (slot idx + bias)] ----
        pos1_local = np.zeros(SHARD, dtype=np.int64)
        p1 = c["perm1"]
        for q in range(ROWS):
            if p1[q] >= 0:
                pos1_local[p1[q]] = q
        first_real = int(c["perm2"][0])
        cols = []
        for t in range(NT):
            rows = c["perm2"][t * TP:(t + 1) * TP]
            d2 = np.empty(TP, dtype=np.int64)
            for p, r in enumerate(rows):
                d2[p] = pos1_local[r if r >= 0 else first_real]
            cols.append(_wrap16(d2))
        tile_cols = [np.concatenate(cols, axis=1)]
        for t in range(NT):
            rows = c["perm2"][t * TP:(t + 1) * TP]
            la, ha = int(KL2[t]), int(KH2[t])
            lo_mat = np.zeros((TP, max(1, la)), dtype=np.int64)
            hi_mat = np.zeros((TP, max(1, ha)), dtype=np.int64)
            lo_msk = np.zeros((TP, max(1, la)), dtype=np.int16)
            hi_msk = np.zeros((TP, max(1, ha)), dtype=np.int16)
            lo_msk[:] = pad_bits
            hi_msk[:] = pad_bits
            for p, r in enumerate(rows):
                if r >= 0:
                    s = c["rows2"][r]
                    lo = s[s < LO2]
                    hi = s[s >= LO2] - LO2
                    lo_mat[p, :len(lo)] = lo
                    hi_mat[p, :len(hi)] = hi
                    lo_msk[p, :len(lo)] = 0
                    hi_msk[p, :len(hi)] = 0
            off_l = off_h = 0
            for half, kv in vt2[t]:
                if half == 0:
                    m, b = lo_mat[:, off_l:off_l + kv], lo_msk[:, off_l:off_l + kv]
                    off_l += kv
                else:
                    m, b = hi_mat[:, off_h:off_h + kv], hi_msk[:, off_h:off_h + kv]
                    off_h += kv
                tile_cols.append(_wrap16(m.T.reshape(-1)))
                tile_cols.append(b.astype(np.int16))   # [128, kv] plain layout
            # bias bits are fp16 PAD_BIAS for masked slots, 0 for real
        blobs2.append(np.concatenate(tile_cols, axis=1))

    return dict(cores=cores, blobs1=blobs1, blobs2=blobs2,
                vt1=vt1, vt2=vt2, KL1=KL1, KH1=KH1, KL2=KL2, KH2=KH2)


def _hm(mat, HEADS, HID):
    """Reorder columns from head-major (h*HID+c) to head-minor (c*HEADS+h)."""
    n = mat.shape[1] if mat.ndim == 2 else mat.shape[0]
    assert n == HEADS * HID
    idx = np.empty(n, dtype=np.int64)
    for h in range(HEADS):
        for c in range(HID):
            idx[c * HEADS + h] = h * HID + c
    return mat[:, idx] if mat.ndim == 2 else mat[idx]


def _pack_consts(cfg, W1, a1_src, a1_dst, b1, W2, a2_src, a2_dst, b2,
                 Wm1, bm1, Wm2, bm2):
    IN_DIM, HID, HEADS, OUT_DIM = (cfg["IN_DIM"], cfg["HID"], cfg["HEADS"],
                                   cfg["OUT_DIM"])
    W1R, W2R = cfg["W1R"], cfg["W2R"]
    U1 = np.einsum("khc,hc->kh", W1.reshape(IN_DIM, HEADS, HID), a1_src)
    V1 = np.einsum("khc,hc->kh", W1.reshape(IN_DIM, HEADS, HID), a1_dst)
    W1X = np.zeros((IN_DIM, W1R), dtype=np.float32)
    W1X[:, :IN_DIM] = _hm(W1, HEADS, HID)
    W1X[:, IN_DIM:IN_DIM + HEADS] = U1
    W1X[:, IN_DIM + HEADS:IN_DIM + 2 * HEADS] = V1
    u2 = W2 @ a2_src[0]
    v2 = W2 @ a2_dst[0]
    # rows of W2 permuted to head-minor order (o1T rows are head-minor)
    W2X = np.zeros((HEADS * HID, W2R), dtype=np.float32)
    W2X[:, :OUT_DIM] = W2
    W2X[:, OUT_DIM] = u2
    W2X[:, OUT_DIM + 1] = v2
    W2Xp = _hm(W2X.T, HEADS, HID).T
    P = 128
    blocks = {}
    cols = 0

    def add(name, arr):
        nonlocal cols
        a = np.zeros((P, arr.shape[1]), dtype=np.float16)
        a[:arr.shape[0]] = arr.astype(np.float16)
        blocks[name] = (cols, arr.shape[1])
        cols += arr.shape[1]
        return a

    parts = []
    parts.append(add("w1x0", W1X[0:P]))
    parts.append(add("w1x1", W1X[P:2 * P]))
    parts.append(add("w2x0", W2Xp[0:P]))
    parts.append(add("w2x1", W2Xp[P:2 * P]))
    parts.append(add("wm1", Wm1.astype(np.float32)))
    parts.append(add("wm2", Wm2.astype(np.float32)))
    parts.append(add("ident", np.eye(P, dtype=np.float32)))
    parts.append(add("b1r", np.tile(_hm(b1.astype(np.float32)[None, :],
                                        HEADS, HID), (P, 1))))
    parts.append(add("b2r", np.tile(b2.astype(np.float32), (P, 1))))
    parts.append(add("bm1r", np.tile(bm1.astype(np.float32), (P, 1))))
    parts.append(add("bm2r", np.tile(bm2.astype(np.float32), (P, 1))))
    parts.append(add("nshift", np.full((P, 1), -SHIFT, dtype=np.float32)))
    consts = np.concatenate(parts, axis=1)
    # dummy rows: [hx1_lo, hx1_hi] in a [2, W1R] array; ssrc cols poisoned
    dums = np.zeros((2, W1R), dtype=np.float16)
    dums[:, IN_DIM:IN_DIM + HEADS] = PAD_BIAS
    return consts, blocks, dums


# ------------------------------------------------------------- device build

def _build(cfg, prep, cblocks, CW, phase="full", sim_collective=False):
    N, NC = cfg["N"], cfg["NC"]
    IN_DIM, HID, HEADS, OUT_DIM = (cfg["IN_DIM"], cfg["HID"], cfg["HEADS"],
                                   cfg["OUT_DIM"])
    TP, NT, ROWS, SHARD = cfg["TP"], cfg["NT"], cfg["ROWS"], cfg["SHARD"]
    W1R, LO1 = cfg["W1R"], cfg["LO1"]
    W2R, LO2 = cfg["W2R"], cfg["LO2ROWS"]
    CHT, NCH, CHROWS, CH_ALL = (cfg["CHT"], cfg["NCH"], cfg["CHROWS"],
                                cfg["CH_ALL"])
    HX1R, HX2R = cfg["HX1_ROWS"], cfg["HX2_ROWS"]
    vt1, vt2 = prep["vt1"], prep["vt2"]
    C1 = prep["blobs1"][0].shape[1]
    C2 = prep["blobs2"][0].shape[1]
    NH2 = HEADS * HID
    P = 128

    nc = bacc.Bacc("TRN2", target_bir_lowering=False, debug=False,
                   num_devices=NC)
    xT = nc.dram_tensor("xT", [IN_DIM, N], F16, kind="ExternalInput")
    consts = nc.dram_tensor("consts", [128, CW], F16, kind="ExternalInput")
    dums = nc.dram_tensor("dums", [2, W1R], F16, kind="ExternalInput")
    idx1 = nc.dram_tensor("idx1", [128, C1], I16, kind="ExternalInput")
    idx2 = nc.dram_tensor("idx2", [128, C2], I16, kind="ExternalInput")
    out = nc.dram_tensor("out", [ROWS, OUT_DIM], F32, kind="ExternalOutput")
    dbg = nc.dram_tensor("dbg", [128, 512], F32, kind="ExternalOutput")

    HX1 = nc.dram_tensor("HX1", [HX1R, W1R], F16)
    SH2 = nc.dram_tensor("SH2", [ROWS, W2R], F16)
    HX2 = nc.dram_tensor("HX2", [HX2R, W2R], F16, addr_space="Shared")

    hx1_lo = HX1[0:LO1 + 1, :]
    hx1_hi = HX1[LO1 + 1:HX1R, :]
    hx2_lo = HX2[0:LO2, :]
    hx2_hi = HX2[LO2:HX2R, :]

    with tile.TileContext(nc) as tc:
        nc.gpsimd.load_library(library_config.mlp)
        with tc.tile_pool(name="cp", bufs=1) as cp:
            cb = cp.tile([128, CW], F16, tag="consts")
            nc.sync.dma_start(cb[:, :], consts[:, :])

            def C(name):
                off, w = cblocks[name]
                return cb[:, off:off + w]

            nc.sync.dma_start(HX1[LO1:LO1 + 1, :], dums[0:1, :])
            nc.sync.dma_start(HX1[HX1R - 1:HX1R, :], dums[1:2, :])

            # ---------------- P0: full HX1 table (fp16) -----------------
            SB = 1024
            nsb = -(-N // SB)
            with (
                tc.tile_pool(name="p0", bufs=3) as p0,
                tc.tile_pool(name="p0ps", bufs=1, space="PSUM") as p0ps,
            ):
                for sb in range(nsb):
                    base = sb * SB
                    cnt = min(SB, N - base)
                    nq = -(-cnt // P)
                    xt = p0.tile([P, 2 * cnt], F16, tag="xt")
                    xtv = xt[:, :].rearrange("p (q c) -> p q c", q=2)
                    nc.sync.dma_start(
                        xtv,
                        xT[:, base:base + cnt].rearrange(
                            "(q p) c -> p q c", p=P))
                    for g in range(2):
                        q0 = g * 4
                        gq = min(4, nq - q0)
                        if gq <= 0:
                            continue
                        ps = p0ps.tile([P, 4 * 512], F32, tag=f"ps{g}")
                        psv = ps[:, :].rearrange("p (q c) -> p q c", q=4)
                        for qi in range(gq):
                            q = q0 + qi
                            pb = min(P, cnt - q * P)
                            nc.tensor.matmul(psv[0:pb, qi, 0:W1R],
                                             xtv[:, 0, q * P:q * P + pb],
                                             C("w1x0"), start=True, stop=False)
                            nc.tensor.matmul(psv[0:pb, qi, 0:W1R],
                                             xtv[:, 1, q * P:q * P + pb],
                                             C("w1x1"), start=False, stop=True)
                        hx4 = p0.tile([P, gq * W1R], F16, tag=f"hx4{g}")
                        hx4v = hx4[:, :].rearrange("p (q w) -> p q w", q=gq)
                        hq = (gq + 1) // 2
                        nc.scalar.copy(hx4v[:, 0:hq, :], psv[:, 0:hq, 0:W1R])
                        if gq > hq:
                            nc.vector.tensor_scalar(
                                hx4v[:, hq:gq, :], psv[:, hq:gq, 0:W1R],
                                0.0, None, op0=OP.add)
                        gb = base + q0 * P
                        gcnt = min(4 * P, cnt - q0 * P)

                        def wr(a, b):   # node range [a, b) within group
                            if a >= b:
                                return
                            ra = gb + a + (1 if gb + a >= LO1 else 0)
                            dv = HX1[ra:ra + (b - a), :]
                            qa, pa = divmod(a, P)
                            qb, pb_ = divmod(b - 1, P)
                            if qa == qb:
                                nc.sync.dma_start(dv, hx4v[pa:pb_ + 1, qa, :])
                                return
                            if pa == 0 and pb_ == P - 1:
                                nc.sync.dma_start(
                                    dv.rearrange("(q p) w -> p q w", p=P),
                                    hx4v[:, qa:qb + 1, :])
                                return
                            n0 = P - pa
                            nc.sync.dma_start(dv[0:n0, :], hx4v[pa:P, qa, :])
                            off = n0
                            for q in range(qa + 1, qb):
                                nc.sync.dma_start(dv[off:off + P, :],
                                                  hx4v[0:P, q, :])
                                off += P
                            nc.sync.dma_start(dv[off:, :],
                                              hx4v[0:pb_ + 1, qb, :])

                        if gb < LO1 < gb + gcnt:
                            wr(0, LO1 - gb)
                            wr(LO1 - gb, gcnt)
                        else:
                            wr(0, gcnt)

            if phase == "p0":
                nc.sync.dma_start(dbg[0:128, 0:W1R].bitcast(F16),
                                  HX1[0:128, :])

            # ---------------- L1 + H2 + chunked AllGather ----------------
            _l2r_cm = tc.tile_pool(name="l2r", bufs=1)
            l2r = _l2r_cm.__enter__()
            IT2 = l2r.tile([128, C2], I16, tag="it2")
            nc.sync.dma_start(IT2[:, :], idx2[:, :])
            with (
                tc.tile_pool(name="l1r", bufs=1) as l1r,
                tc.tile_pool(name="l1", bufs=3) as l1,
                tc.tile_pool(name="l1b", bufs=3) as l1b,
                tc.tile_pool(name="l1ps", bufs=2, space="PSUM") as l1ps,
            ):
                IT1 = l1r.tile([128, C1], I16, tag="it1")
                nc.sync.dma_start(IT1[:, :], idx1[:, :])
                sd4 = l1r.tile([P, NT * HEADS], F16, tag="sd4")
                with tc.tile_pool(name="sdp", bufs=1) as sdp:
                    sdl = sdp.tile([P, NT * 128], F16, tag="sdl")
                    sdh = sdp.tile([P, NT * 128], F16, tag="sdh")
                    nc.gpsimd.dma_gather(
                        sdl[:, :].rearrange("p (t w) -> p t w", t=NT),
                        hx1_lo[:, IN_DIM:IN_DIM + 128], IT1[:, 0:NT * 8],
                        NT * P, NT * P, 128, elem_step=W1R,
                        single_packet=False)
                    nc.gpsimd.dma_gather(
                        sdh[:, :].rearrange("p (t w) -> p t w", t=NT),
                        hx1_hi[:, IN_DIM:IN_DIM + 128], IT1[:, NT * 8:NT * 16],
                        NT * P, NT * P, 128, elem_step=W1R,
                        single_packet=False)
                    sdlv = sdl[:, :].rearrange("p (t w) -> p t w", t=NT)
                    sdhv = sdh[:, :].rearrange("p (t w) -> p t w", t=NT)
                    nc.vector.tensor_tensor(
                        sd4[:, :].rearrange("p (t h) -> p t h", t=NT),
                        sdlv[:, :, HEADS:2 * HEADS],
                        sdhv[:, :, HEADS:2 * HEADS], op=OP.add)

                col = [2 * NT * 8]
                for t in (range(NT) if phase != "p0" else range(0)):
                    num = l1b.tile([P, NH2], F16, tag="num")
                    den = l1b.tile([P, HEADS], F32, tag="den")
                    def l1_mac(hg, hgv, w, kv, first):
                        dv = den if first else l1b.tile([P, HEADS], F32,
                                                        tag="dv")
                        nc.vector.tensor_reduce(
                            dv[:, :],
                            w[:, :].rearrange("p (j h) -> p h j", j=kv),
                            axis=AX.X, op=OP.add)
                        if not first:
                            nc.vector.tensor_tensor(den[:, :], den[:, :],
                                                    dv[:, :], op=OP.add)
                        nc.vector.tensor_tensor(
                            hgv[:, :, 0:NH2].rearrange(
                                "p j (c h) -> p j c h", h=HEADS),
                            hgv[:, :, 0:NH2].rearrange(
                                "p j (c h) -> p j c h", h=HEADS),
                            w[:, :].rearrange("p (j h) -> p j h", j=kv)
                            .unsqueeze(2).broadcast_to([P, kv, HID, HEADS]),
                            op=OP.mult)
                        n = kv
                        while n > 1:
                            if n % 2 == 1:
                                nc.vector.tensor_tensor(
                                    hgv[:, 0, 0:NH2], hgv[:, 0, 0:NH2],
                                    hgv[:, n - 1, 0:NH2], op=OP.add)
                                n -= 1
                            pairs = n // 2
                            hp = hg[:, 0:pairs * 2 * W1R].rearrange(
                                "p (j two w) -> p j two w", two=2, w=W1R)
                            nc.vector.tensor_tensor(
                                hgv[:, 0:pairs, 0:NH2],
                                hp[:, :, 0, 0:NH2], hp[:, :, 1, 0:NH2],
                                op=OP.add)
                            n = pairs
                        if first:
                            nc.vector.tensor_scalar(
                                num[:, :], hgv[:, 0, 0:NH2], 0.0, None,
                                op0=OP.add)
                        else:
                            nc.vector.tensor_tensor(num[:, :], num[:, :],
                                                    hgv[:, 0, 0:NH2],
                                                    op=OP.add)

                    pend = None
                    for v, (half, kv) in enumerate(vt1[t]):
                        itv = IT1[:, col[0]:col[0] + kv * 8]
                        col[0] += kv * 8
                        hg = l1.tile([P, kv * W1R], F16, tag="hg")
                        nc.gpsimd.dma_gather(
                            hg[:, :].rearrange("p (j w) -> p j w", j=kv),
                            (hx1_lo if half == 0 else hx1_hi)[:, :],
                            itv, P * kv, P * kv, W1R, single_packet=False)
                        hgv = hg[:, :].rearrange("p (j w) -> p j w", j=kv)
                        s = l1b.tile([P, kv * HEADS], F16, tag="s")
                        nc.vector.tensor_tensor(
                            s[:, :].rearrange("p (j h) -> p j h", j=kv),
                            hgv[:, :, IN_DIM:IN_DIM + HEADS],
                            sd4[:, t * HEADS:(t + 1) * HEADS].unsqueeze(1)
                            .broadcast_to([P, kv, HEADS]), op=OP.add)
                        w0 = l1b.tile([P, kv * HEADS], F16, tag="w0")
                        nc.vector.scalar_tensor_tensor(
                            w0[:, :], s[:, :], NEG_SLOPE, s[:, :],
                            op0=OP.mult, op1=OP.max)
                        w = l1b.tile([P, kv * HEADS], F16, tag="w")
                        nc.scalar.activation(w[:, :], w0[:, :], AF.Exp,
                                             bias=C("nshift"))
                        if pend is not None:
                            l1_mac(*pend)
                        pend = (hg, hgv, w, kv, v == 0)
                    if pend is not None:
                        l1_mac(*pend)
                    dinv = l1b.tile([P, HEADS], F32, tag="dinv")
                    nc.vector.tensor_scalar_max(dinv[:, :], den[:, :], 1e-6)
                    nc.vector.reciprocal(dinv[:, :], dinv[:, :])
                    o = l1b.tile([P, NH2], F16, tag="o")
                    nc.vector.tensor_tensor(
                        o[:, :].rearrange("p (c h) -> p c h", h=HEADS),
                        num[:, :].rearrange("p (c h) -> p c h", h=HEADS),
                        dinv[:, :].unsqueeze(1).broadcast_to(
                            [P, HID, HEADS]), op=OP.mult)
                    nc.vector.tensor_tensor(o[:, :], o[:, :], C("b1r"),
                                            op=OP.add)
                    # elu
                    m0 = l1b.tile([P, NH2], F16, tag="m0")
                    nc.vector.tensor_scalar_min(m0[:, :], o[:, :], 0.0)
                    em = l1b.tile([P, NH2], F16, tag="em")
                    nc.scalar.activation(em[:, :], m0[:, :], AF.Exp)
                    p1 = l1b.tile([P, NH2], F16, tag="p1")
                    nc.vector.tensor_scalar(p1[:, :], o[:, :], 0.0, -1.0,
                                            op0=OP.max, op1=OP.add)
                    eo = l1b.tile([P, NH2], F16, tag="eo")
                    nc.vector.tensor_tensor(eo[:, :], em[:, :], p1[:, :],
                                            op=OP.add)
                    # transpose + H2 matmul
                    pt = l1ps.tile([P, NH2], F16, tag="pt")
                    nc.tensor.transpose(pt[:, 0:P], eo[:, 0:P], C("ident"))
                    nc.tensor.transpose(pt[:, P:NH2], eo[:, P:NH2],
                                        C("ident"))
                    o1T = l1b.tile([P, NH2], F16, tag="o1T")
                    nc.scalar.copy(o1T[:, :], pt[:, :])
                    h2p = l1ps.tile([P, W2R], F32, tag="h2p")
                    nc.tensor.matmul(h2p[:, :], o1T[:, 0:P], C("w2x0"),
                                     start=True, stop=False)
                    nc.tensor.matmul(h2p[:, :], o1T[:, P:NH2], C("w2x1"),
                                     start=False, stop=True)
                    sh2 = l1b.tile([P, W2R], F16, tag="sh2")
                    nc.scalar.copy(sh2[:, :], h2p[:, :])
                    nc.sync.dma_start(SH2[t * P:(t + 1) * P, :], sh2[:, :])

                    if (t + 1) % CHT == 0 and phase == "full":
                        c = t // CHT
                        base = c * CH_ALL
                        if sim_collective:
                            for r in range(NC):
                                nc.sync.dma_start(
                                    HX2[base + r * CHROWS:
                                        base + (r + 1) * CHROWS, :],
                                    SH2[c * CHROWS:(c + 1) * CHROWS, :])
                        else:
                            nc.gpsimd.collective_compute(
                                "AllGather", OP.bypass,
                                replica_groups=[list(range(NC))],
                                ins=[SH2[c * CHROWS:(c + 1) * CHROWS, :].opt()],
                                outs=[HX2[base:base + CH_ALL, :].opt()],
                            )

            if phase == "l1":
                nc.sync.dma_start(dbg[0:128, 0:W2R].bitcast(F16),
                                  SH2[0:128, :])
            # ---------------- L2 + MLP + normalize ----------------------
            with (
                tc.tile_pool(name="l2", bufs=4) as l2,
                tc.tile_pool(name="l2b", bufs=4) as l2b,
                tc.tile_pool(name="l2ps", bufs=4, space="PSUM") as l2ps,
            ):
                sd2g = l2r.tile([P, NT * 128], F16, tag="sd2g")
                nc.gpsimd.dma_gather(
                    sd2g[:, :].rearrange("p (t w) -> p t w", t=NT),
                    SH2[:, OUT_DIM:OUT_DIM + 128], IT2[:, 0:NT * 8],
                    NT * P, NT * P, 128, elem_step=W2R, single_packet=False)
                sd2v = sd2g[:, :].rearrange("p (t w) -> p t w", t=NT)
                sd2f = l2r.tile([P, NT], F32, tag="sd2f")
                nc.vector.tensor_scalar(sd2f[:, :].unsqueeze(2),
                                        sd2v[:, :, 1:2], 0.0, None,
                                        op0=OP.add)

                o2a = l2r.tile([P, NT * OUT_DIM], F16, tag="o2a")
                magic = l2r.tile([P, 1], I32, tag="magic")
                nc.vector.memset(magic[:, :], 0x5F3759DF)
                col2 = [NT * 8]
                for t in (range(NT) if phase == "full" else range(0)):
                    num2 = l2b.tile([P, OUT_DIM], F16, tag="num2")
                    den2 = l2b.tile([P, 1], F32, tag="den2")
                    def l2_mac(hg, hgv, w, dv, kv, first):
                        if not first:
                            nc.vector.tensor_tensor(den2[:, :], den2[:, :],
                                                    dv[:, :], op=OP.add)
                        for j in range(kv):
                            nc.vector.tensor_scalar(
                                hgv[:, j, 0:OUT_DIM], hgv[:, j, 0:OUT_DIM],
                                w[:, j:j + 1], None, op0=OP.mult)
                        n = kv
                        while n > 1:
                            if n % 2 == 1:
                                nc.vector.tensor_tensor(
                                    hgv[:, 0, 0:OUT_DIM], hgv[:, 0, 0:OUT_DIM],
                                    hgv[:, n - 1, 0:OUT_DIM], op=OP.add)
                                n -= 1
                            pairs = n // 2
                            hp = hg[:, 0:pairs * 2 * W2R].rearrange(
                                "p (j two w) -> p j two w", two=2, w=W2R)
                            nc.vector.tensor_tensor(
                                hgv[:, 0:pairs, 0:OUT_DIM],
                                hp[:, :, 0, 0:OUT_DIM], hp[:, :, 1, 0:OUT_DIM],
                                op=OP.add)
                            n = pairs
                        if first:
                            nc.vector.tensor_scalar(
                                num2[:, :], hgv[:, 0, 0:OUT_DIM], 0.0, None,
                                op0=OP.add)
                        else:
                            nc.vector.tensor_tensor(num2[:, :], num2[:, :],
                                                    hgv[:, 0, 0:OUT_DIM],
                                                    op=OP.add)

                    pend = None
                    for v, (half, kv) in enumerate(vt2[t]):
                        itv = IT2[:, col2[0]:col2[0] + kv * 8]
                        col2[0] += kv * 8
                        ib = IT2[:, col2[0]:col2[0] + kv].bitcast(F16)
                        col2[0] += kv
                        hg = l2.tile([P, kv * W2R], F16, tag="hg2")
                        nc.gpsimd.dma_gather(
                            hg[:, :].rearrange("p (j w) -> p j w", j=kv),
                            (hx2_lo if half == 0 else hx2_hi)[:, :],
                            itv, P * kv, P * kv, W2R, single_packet=False)
                        hgv = hg[:, :].rearrange("p (j w) -> p j w", j=kv)
                        s = l2b.tile([P, kv], F16, tag="s2")
                        nc.vector.tensor_scalar(
                            s[:, :].unsqueeze(2),
                            hgv[:, :, OUT_DIM:OUT_DIM + 1],
                            sd2f[:, t:t + 1], None, op0=OP.add)
                        nc.vector.tensor_tensor(s[:, :], s[:, :], ib,
                                                op=OP.add)
                        w0 = l2b.tile([P, kv], F16, tag="w02")
                        nc.vector.scalar_tensor_tensor(
                            w0[:, :], s[:, :], NEG_SLOPE, s[:, :],
                            op0=OP.mult, op1=OP.max)
                        w = l2b.tile([P, kv], F32, tag="w2")
                        dv = den2 if v == 0 else l2b.tile([P, 1], F32,
                                                          tag="dv2")
                        nc.scalar.activation(w[:, :], w0[:, :], AF.Exp,
                                             bias=C("nshift"),
                                             accum_out=dv[:, :])
                        if pend is not None:
                            l2_mac(*pend)
                        pend = (hg, hgv, w, dv, kv, v == 0)
                    if pend is not None:
                        l2_mac(*pend)
                    dinv = l2b.tile([P, 1], F32, tag="dinv2")
                    nc.vector.tensor_scalar_max(dinv[:, :], den2[:, :], 1e-6)
                    nc.vector.reciprocal(dinv[:, :], dinv[:, :])
                    o2 = o2a[:, t * OUT_DIM:(t + 1) * OUT_DIM]
                    nc.vector.tensor_scalar(o2, num2[:, :],
                                            dinv[:, 0:1], None, op0=OP.mult)
                    nc.vector.tensor_tensor(o2, o2, C("b2r"), op=OP.add)

                for t in (range(NT) if phase == "full" else range(0)):
                    o2 = o2a[:, t * OUT_DIM:(t + 1) * OUT_DIM]
                    # MLP (all PSUM packed into one bank per tile)
                    mlp = l2ps.tile([P, 512], F32, tag="mlp")
                    pt2 = mlp[:, 0:64].bitcast(F16)
                    nc.tensor.transpose(pt2, o2, C("ident"))
                    o2T = l2b.tile([P, P], F16, tag="o2T")
                    nc.scalar.copy(o2T[:, :], pt2)
                    h3p = mlp[:, 64:128]
                    nc.tensor.matmul(h3p, o2T[:, :], C("wm1"),
                                     start=True, stop=True)
                    h3 = l2b.tile([P, HID], F16, tag="h3")
                    nc.vector.tensor_tensor(h3[:, :], h3p, C("bm1r"),
                                            op=OP.add)
                    nc.vector.tensor_scalar_max(h3[:, :], h3[:, :], 0.0)
                    pt3 = mlp[0:HID, 128:192].bitcast(F16)
                    nc.tensor.transpose(pt3, h3[:, :], C("ident"))
                    h3T = l2b.tile([HID, P], F16, tag="h3T")
                    nc.scalar.copy(h3T[:, :], pt3)
                    h4p = mlp[:, 256:384]
                    nc.tensor.matmul(h4p, h3T[0:HID, :],
                                     C("wm2")[0:HID, :], start=True, stop=True)
                    h4 = l2b.tile([P, OUT_DIM], F32, tag="h4")
                    nc.vector.tensor_tensor(h4[:, :], h4p, C("bm2r"),
                                            op=OP.add)
                    hsq = l2b.tile([P, OUT_DIM], F32, tag="hsq")
                    nc.vector.tensor_tensor(hsq[:, :], h4[:, :], h4[:, :],
                                            op=OP.mult)
                    n2 = l2b.tile([P, 1], F32, tag="n2")
                    nc.vector.tensor_reduce(n2[:, :], hsq[:, :], axis=AX.X,
                                            op=OP.add)
                    nin = l2b.tile([P, 1], F32, tag="nin")
                    nc.vector.tensor_scalar_max(nin[:, :], n2[:, :], 1e-12)
                    # Newton rsqrt (keeps Sqrt off the Act engine: avoids
                    # act-table thrash between Exp and Sqrt function sets)
                    yr = l2b.tile([P, 1], F32, tag="yr")
                    tn = l2b.tile([P, 1], F32, tag="tn")
                    nc.vector.tensor_scalar(yr[:, :].bitcast(I32),
                                            nin[:, :].bitcast(I32), 1, None,
                                            op0=OP.arith_shift_right)
                    nc.vector.tensor_tensor(yr[:, :].bitcast(I32),
                                            magic[:, :],
                                            yr[:, :].bitcast(I32),
                                            op=OP.subtract)
                    for _ in range(3):
                        nc.vector.tensor_tensor(tn[:, :], yr[:, :], yr[:, :],
                                                op=OP.mult)
                        nc.vector.tensor_tensor(tn[:, :], tn[:, :], nin[:, :],
                                                op=OP.mult)
                        nc.vector.tensor_scalar(tn[:, :], tn[:, :], -0.5, 1.5,
                                                op0=OP.mult, op1=OP.add)
                        nc.vector.tensor_tensor(yr[:, :], yr[:, :], tn[:, :],
                                                op=OP.mult)
                    nin = yr
                    of = l2b.tile([P, OUT_DIM], F32, tag="of")
                    nc.vector.tensor_scalar(of[:, :], h4[:, :], nin[:, 0:1],
                                            None, op0=OP.mult)
                    nc.sync.dma_start(out[t * P:(t + 1) * P, :], of[:, :])
            _l2r_cm.__exit__(None, None, None)

    nc.compile()
    return nc


# ------------------------------------------------------------------ driver

def run(cfg, inputs, trace=False, phase="full", sim_collective=False,
        prep=None):
    x = np.asarray(inputs["x"], dtype=np.float32)
    edge_index = np.asarray(inputs["edge_index"])
    if prep is None:
        prep = _prep(cfg, edge_index)
    consts, cblocks, dums = _pack_consts(
        cfg, *[np.asarray(inputs[k], dtype=np.float32) for k in
               ("W1", "a1_src", "a1_dst", "b1", "W2", "a2_src", "a2_dst",
                "b2", "Wm1", "bm1", "Wm2", "bm2")])
    nc = _build(cfg, prep, cblocks, consts.shape[1], phase=phase,
                sim_collective=sim_collective)
    xT = np.ascontiguousarray(x.T.astype(np.float16))
    in_maps = []
    for k in range(cfg["NC"]):
        in_maps.append({
            "xT": xT, "consts": consts, "dums": dums,
            "idx1": np.ascontiguousarray(prep["blobs1"][k]),
            "idx2": np.ascontiguousarray(prep["blobs2"][k]),
        })
    res = run_bass_kernel_spmd(nc, in_maps, list(range(cfg["NC"])),
                               trace=trace)
    N, NC, SHARD = cfg["N"], cfg["NC"], cfg["SHARD"]
    full = np.zeros((N, cfg["OUT_DIM"]), dtype=np.float32)
    for k in range(NC):
        o = res.results[k]["out"]
        perm2 = prep["cores"][k]["perm2"]
        real = perm2 >= 0
        full[k * SHARD + perm2[real]] = o[real]
    return full, res


def kernel(**inputs):
    cfg = make_cfg()
    full, _ = run(cfg, inputs, trace=False)
    return full


# revision 19
# speedup vs baseline: 1.9638x; 1.0289x over previous
"""Trainium2 Bass kernel for a 2-layer GAT + MLP (nn_MemoryGNN).

Strategy (8 NeuronCores, SPMD):
  - Destination-node partition: core k owns dst nodes [k*6250, (k+1)*6250).
  - Every core computes the FULL HX1 table (fp16): row n -> [h1 (256,
    head-minor c*4+h) | ssrc (4) | sdst (4) | pad] = 384 cols (768B rows),
    so layer-1 message gathers are purely local.
  - Per-edge gathers use gpsimd.dma_gather from per-dst-tile slot tables
    (host-precomputed int16 index blobs, degree-sorted tiles).
  - Softmax is computed unnormalized with a global exp-shift of -2 (folded
    into the Act-engine exp bias); the shift cancels in num/den.
  - fp16 everywhere in the hot loops (DVE 2x eligible); accumulation of the
    weighted message sum is an fp16 add-chain (err ~0.1% << 2e-2 tol).
  - Layer 2 (fp16 512B rows) needs remote H2 rows: a chunked AllGather of
    per-core SH2 shards into HX2 (Shared) overlaps with layer-1 compute.
  - Output rows are produced in a degree-sorted permuted order; the host
    applies the inverse permutation (free).
"""

import sys
import numpy as np

for _p in ("/opt/trn_rl_repo", "/root/.axon_site/_ro/trn_rl_repo"):
    if _p not in sys.path:
        sys.path.insert(0, _p)

import concourse.bass as bass
import concourse.bacc as bacc
import concourse.mybir as mybir
import concourse.tile as tile
from concourse import library_config
from concourse.bass_utils import run_bass_kernel_spmd

F32 = mybir.dt.float32
F16 = mybir.dt.float16
I16 = mybir.dt.int16
I32 = mybir.dt.int32
AF = mybir.ActivationFunctionType
OP = mybir.AluOpType
AX = mybir.AxisListType

NEG_SLOPE = 0.2
SHIFT = 2.0          # global exp shift (cancels in num/den)
PAD_BIAS = -30000.0  # fp16-safe "minus infinity" for masked slots


def make_cfg(N=50000, E=1000000, IN_DIM=256, HID=64, HEADS=4, OUT_DIM=128,
             NC=8, CHT=7, KCAP1=48, KCAP2=64):
    cfg = dict(N=N, E=E, IN_DIM=IN_DIM, HID=HID, HEADS=HEADS, OUT_DIM=OUT_DIM,
               NC=NC, CHT=CHT, KCAP1=KCAP1, KCAP2=KCAP2)
    cfg["SHARD"] = N // NC
    assert N % NC == 0
    TP = 128
    cfg["TP"] = TP
    NT = -(-cfg["SHARD"] // TP)
    cfg["NT"] = NT
    assert NT % CHT == 0, (NT, CHT)
    cfg["NCH"] = NT // CHT
    cfg["ROWS"] = NT * TP
    cfg["CHROWS"] = CHT * TP          # SH2 rows per AllGather chunk
    # layer-1 table: row n -> n + (n >= LO1); dummies at LO1 and last row
    cfg["D1"] = IN_DIM + 2 * HEADS
    cfg["W1R"] = 384
    assert cfg["W1R"] * 2 % 256 == 0
    cfg["LO1"] = 25024
    assert cfg["LO1"] + 1 <= 32767 and N - cfg["LO1"] + 1 <= 32767
    cfg["HX1_ROWS"] = N + 2
    # layer-2 table (chunk-major, no dummies; masked pads instead)
    cfg["D2"] = OUT_DIM + 2
    cfg["W2R"] = 256
    CH_ALL = cfg["CHROWS"] * NC       # global rows per chunk
    cfg["CH_ALL"] = CH_ALL
    LOCH = 32768 // CH_ALL            # chunks fully addressable as lo
    LOCH = max(1, min(cfg["NCH"] - 1, LOCH))
    cfg["LOCH"] = LOCH
    cfg["LO2ROWS"] = LOCH * CH_ALL
    assert cfg["LO2ROWS"] <= 32767
    assert (cfg["NCH"] - LOCH) * CH_ALL <= 32767
    cfg["HX2_ROWS"] = cfg["NCH"] * CH_ALL
    return cfg


# ----------------------------------------------------------------- host prep

def _wrap16(flat):
    """flat int array (len divisible by 16) -> wrapped [128, n/16] int16."""
    w = flat.reshape(-1, 16).T.astype(np.int16)
    return np.tile(w, (8, 1))


def _chunks(total, cap):
    out = []
    off = 0
    while off < total:
        kv = min(cap, total - off)
        out.append(kv)
        off += kv
    return out


def _sort_core(cfg, lo_cnt, hi_cnt):
    """Degree-sort local dsts into tiles; return perm and per-tile maxima."""
    SHARD, TP, NT = cfg["SHARD"], cfg["TP"], cfg["NT"]
    order = np.lexsort((-hi_cnt, -(lo_cnt // 6)))
    perm = np.full(NT * TP, -1, dtype=np.int64)
    perm[:SHARD] = order
    kl_t = np.zeros(NT, dtype=np.int64)
    kh_t = np.zeros(NT, dtype=np.int64)
    for t in range(NT):
        rows = perm[t * TP:(t + 1) * TP]
        real = rows[rows >= 0]
        if len(real):
            kl_t[t] = lo_cnt[real].max()
            kh_t[t] = hi_cnt[real].max()
    return perm, kl_t, kh_t


def _prep(cfg, edge_index):
    """Host preprocessing (structure only). Returns per-core blobs + meta."""
    N, NC, SHARD = cfg["N"], cfg["NC"], cfg["SHARD"]
    TP, NT, ROWS = cfg["TP"], cfg["NT"], cfg["ROWS"]
    KCAP1, KCAP2 = cfg["KCAP1"], cfg["KCAP2"]
    LO1 = cfg["LO1"]
    src = np.concatenate([np.asarray(edge_index[0]),
                          np.arange(N)]).astype(np.int64)
    dst = np.concatenate([np.asarray(edge_index[1]),
                          np.arange(N)]).astype(np.int64)
    order = np.argsort(dst, kind="stable")
    src_s = src[order]
    deg = np.bincount(dst, minlength=N)
    rp = np.zeros(N + 1, dtype=np.int64)
    np.cumsum(deg, out=rp[1:])

    # L1 table indexing: lo rows = node n (< LO1), dummy at LO1;
    # hi rows = node n - LO1 (n >= LO1), dummy at N - LO1 + 1 - 1
    dum1_lo = LO1
    dum1_hi = cfg["HX1_ROWS"] - 1 - (LO1 + 1)

    cores = []
    for k in range(NC):
        g0 = k * SHARD
        srcs = [src_s[rp[g0 + r]:rp[g0 + r + 1]] for r in range(SHARD)]
        lo_cnt = np.array([int((s < LO1).sum()) for s in srcs])
        hi_cnt = np.array([len(s) for s in srcs]) - lo_cnt
        perm1, kl1, kh1 = _sort_core(cfg, lo_cnt, hi_cnt)
        cores.append(dict(srcs=srcs, perm1=perm1, kl1=kl1, kh1=kh1))

    KL1 = np.max([c["kl1"] for c in cores], axis=0)
    KH1 = np.max([c["kh1"] for c in cores], axis=0)
    vt1 = [[(0, kv) for kv in _chunks(int(KL1[t]), KCAP1)] +
           [(1, kv) for kv in _chunks(int(KH1[t]), KCAP1)] for t in range(NT)]

    # position of each global node in its owner's tile order
    pos1 = np.empty(N, dtype=np.int64)
    for k in range(NC):
        p = cores[k]["perm1"]
        q = np.arange(ROWS)
        real = p >= 0
        pos1[k * SHARD + p[real]] = q[real]
    CHROWS, CH_ALL = cfg["CHROWS"], cfg["CH_ALL"]
    c_of = pos1 // CHROWS
    r_of = pos1 % CHROWS
    owner = np.arange(N) // SHARD
    cm = c_of * CH_ALL + owner * CHROWS + r_of      # HX2 row of node
    LO2 = cfg["LO2ROWS"]

    for k in range(NC):
        c = cores[k]
        rows2 = [cm[s] for s in c["srcs"]]
        lo2 = np.array([int((r < LO2).sum()) for r in rows2])
        hi2 = np.array([len(r) for r in rows2]) - lo2
        perm2, kl2, kh2 = _sort_core(cfg, lo2, hi2)
        c.update(rows2=rows2, perm2=perm2, kl2=kl2, kh2=kh2)

    KL2 = np.max([c["kl2"] for c in cores], axis=0)
    KH2 = np.max([c["kh2"] for c in cores], axis=0)
    vt2 = [[(0, kv) for kv in _chunks(int(KL2[t]), KCAP2)] +
           [(1, kv) for kv in _chunks(int(KH2[t]), KCAP2)] for t in range(NT)]

    pad_bits = np.float16(PAD_BIAS).view(np.int16)

    blobs1, blobs2 = [], []
    for k in range(NC):
        c = cores[k]
        g0 = k * SHARD
        # ---- blob1: [sd-lo idx | sd-hi idx | per-tile slot idx] ----
        cols = []
        for t in range(NT):
            rows = c["perm1"][t * TP:(t + 1) * TP]
            dlo = np.full(TP, dum1_lo, dtype=np.int64)
            dhi = np.full(TP, dum1_hi, dtype=np.int64)
            for p, r in enumerate(rows):
                if r >= 0:
                    n = g0 + r
                    if n < LO1:
                        dlo[p] = n
                    else:
                        dhi[p] = n - LO1
            cols.append(_wrap16(dlo))
            cols.append(_wrap16(dhi))
        sd_cols = [np.concatenate(cols[0::2], axis=1),
                   np.concatenate(cols[1::2], axis=1)]
        tile_cols = []
        for t in range(NT):
            rows = c["perm1"][t * TP:(t + 1) * TP]
            la, ha = int(KL1[t]), int(KH1[t])
            lo_mat = np.full((TP, max(1, la)), dum1_lo, dtype=np.int64)
            hi_mat = np.full((TP, max(1, ha)), dum1_hi, dtype=np.int64)
            for p, r in enumerate(rows):
                if r >= 0:
                    s = c["srcs"][r]
                    lo = s[s < LO1]
                    hi = s[s >= LO1] - LO1
                    lo_mat[p, :len(lo)] = lo
                    hi_mat[p, :len(hi)] = hi
            off_l = off_h = 0
            for half, kv in vt1[t]:
                if half == 0:
                    m = lo_mat[:, off_l:off_l + kv]
                    off_l += kv
                else:
                    m = hi_mat[:, off_h:off_h + kv]
                    off_h += kv
                tile_cols.append(_wrap16(m.T.reshape(-1)))
        blobs1.append(np.concatenate(sd_cols + tile_cols, axis=1))

        # ---- blob2: [sd2 idx (SH2 rows) | per-tile (slot idx + bias)] ----
        pos1_local = np.zeros(SHARD, dtype=np.int64)
        p1 = c["perm1"]
        for q in range(ROWS):
            if p1[q] >= 0:
                pos1_local[p1[q]] = q
        first_real = int(c["perm2"][0])
        cols = []
        for t in range(NT):
            rows = c["perm2"][t * TP:(t + 1) * TP]
            d2 = np.empty(TP, dtype=np.int64)
            for p, r in enumerate(rows):
                d2[p] = pos1_local[r if r >= 0 else first_real]
            cols.append(_wrap16(d2))
        tile_cols = [np.concatenate(cols, axis=1)]
        for t in range(NT):
            rows = c["perm2"][t * TP:(t + 1) * TP]
            la, ha = int(KL2[t]), int(KH2[t])
            lo_mat = np.zeros((TP, max(1, la)), dtype=np.int64)
            hi_mat = np.zeros((TP, max(1, ha)), dtype=np.int64)
            lo_msk = np.zeros((TP, max(1, la)), dtype=np.int16)
            hi_msk = np.zeros((TP, max(1, ha)), dtype=np.int16)
            lo_msk[:] = pad_bits
            hi_msk[:] = pad_bits
            for p, r in enumerate(rows):
                if r >= 0:
                    s = c["rows2"][r]
                    lo = s[s < LO2]
                    hi = s[s >= LO2] - LO2
                    lo_mat[p, :len(lo)] = lo
                    hi_mat[p, :len(hi)] = hi
                    lo_msk[p, :len(lo)] = 0
                    hi_msk[p, :len(hi)] = 0
            off_l = off_h = 0
            for half, kv in vt2[t]:
                if half == 0:
                    m, b = lo_mat[:, off_l:off_l + kv], lo_msk[:, off_l:off_l + kv]
                    off_l += kv
                else:
                    m, b = hi_mat[:, off_h:off_h + kv], hi_msk[:, off_h:off_h + kv]
                    off_h += kv
                tile_cols.append(_wrap16(m.T.reshape(-1)))
                tile_cols.append(b.astype(np.int16))   # [128, kv] plain layout
            # bias bits are fp16 PAD_BIAS for masked slots, 0 for real
        blobs2.append(np.concatenate(tile_cols, axis=1))

    return dict(cores=cores, blobs1=blobs1, blobs2=blobs2,
                vt1=vt1, vt2=vt2, KL1=KL1, KH1=KH1, KL2=KL2, KH2=KH2)


def _hm(mat, HEADS, HID):
    """Reorder columns from head-major (h*HID+c) to head-minor (c*HEADS+h)."""
    n = mat.shape[1] if mat.ndim == 2 else mat.shape[0]
    assert n == HEADS * HID
    idx = np.empty(n, dtype=np.int64)
    for h in range(HEADS):
        for c in range(HID):
            idx[c * HEADS + h] = h * HID + c
    return mat[:, idx] if mat.ndim == 2 else mat[idx]


def _pack_consts(cfg, W1, a1_src, a1_dst, b1, W2, a2_src, a2_dst, b2,
                 Wm1, bm1, Wm2, bm2):
    IN_DIM, HID, HEADS, OUT_DIM = (cfg["IN_DIM"], cfg["HID"], cfg["HEADS"],
                                   cfg["OUT_DIM"])
    W1R, W2R = cfg["W1R"], cfg["W2R"]
    U1 = np.einsum("khc,hc->kh", W1.reshape(IN_DIM, HEADS, HID), a1_src)
    V1 = np.einsum("khc,hc->kh", W1.reshape(IN_DIM, HEADS, HID), a1_dst)
    W1X = np.zeros((IN_DIM, W1R), dtype=np.float32)
    W1X[:, :IN_DIM] = _hm(W1, HEADS, HID)
    W1X[:, IN_DIM:IN_DIM + HEADS] = U1
    W1X[:, IN_DIM + HEADS:IN_DIM + 2 * HEADS] = V1
    u2 = W2 @ a2_src[0]
    v2 = W2 @ a2_dst[0]
    # rows of W2 permuted to head-minor order (o1T rows are head-minor)
    W2X = np.zeros((HEADS * HID, W2R), dtype=np.float32)
    W2X[:, :OUT_DIM] = W2
    W2X[:, OUT_DIM] = u2
    W2X[:, OUT_DIM + 1] = v2
    W2Xp = _hm(W2X.T, HEADS, HID).T
    P = 128
    blocks = {}
    cols = 0

    def add(name, arr):
        nonlocal cols
        a = np.zeros((P, arr.shape[1]), dtype=np.float16)
        a[:arr.shape[0]] = arr.astype(np.float16)
        blocks[name] = (cols, arr.shape[1])
        cols += arr.shape[1]
        return a

    parts = []
    parts.append(add("w1x0", W1X[0:P]))
    parts.append(add("w1x1", W1X[P:2 * P]))
    parts.append(add("w2x0", W2Xp[0:P]))
    parts.append(add("w2x1", W2Xp[P:2 * P]))
    parts.append(add("wm1", Wm1.astype(np.float32)))
    parts.append(add("wm2", Wm2.astype(np.float32)))
    parts.append(add("ident", np.eye(P, dtype=np.float32)))
    parts.append(add("b1r", np.tile(_hm(b1.astype(np.float32)[None, :],
                                        HEADS, HID), (P, 1))))
    parts.append(add("b2r", np.tile(b2.astype(np.float32), (P, 1))))
    parts.append(add("bm1r", np.tile(bm1.astype(np.float32), (P, 1))))
    parts.append(add("bm2r", np.tile(bm2.astype(np.float32), (P, 1))))
    parts.append(add("nshift", np.full((P, 1), -SHIFT, dtype=np.float32)))
    consts = np.concatenate(parts, axis=1)
    # dummy rows: [hx1_lo, hx1_hi] in a [2, W1R] array; ssrc cols poisoned
    dums = np.zeros((2, W1R), dtype=np.float16)
    dums[:, IN_DIM:IN_DIM + HEADS] = PAD_BIAS
    return consts, blocks, dums


# ------------------------------------------------------------- device build

def _build(cfg, prep, cblocks, CW, phase="full", sim_collective=False):
    N, NC = cfg["N"], cfg["NC"]
    IN_DIM, HID, HEADS, OUT_DIM = (cfg["IN_DIM"], cfg["HID"], cfg["HEADS"],
                                   cfg["OUT_DIM"])
    TP, NT, ROWS, SHARD = cfg["TP"], cfg["NT"], cfg["ROWS"], cfg["SHARD"]
    W1R, LO1 = cfg["W1R"], cfg["LO1"]
    W2R, LO2 = cfg["W2R"], cfg["LO2ROWS"]
    CHT, NCH, CHROWS, CH_ALL = (cfg["CHT"], cfg["NCH"], cfg["CHROWS"],
                                cfg["CH_ALL"])
    HX1R, HX2R = cfg["HX1_ROWS"], cfg["HX2_ROWS"]
    vt1, vt2 = prep["vt1"], prep["vt2"]
    C1 = prep["blobs1"][0].shape[1]
    C2 = prep["blobs2"][0].shape[1]
    NH2 = HEADS * HID
    P = 128

    nc = bacc.Bacc("TRN2", target_bir_lowering=False, debug=False,
                   num_devices=NC)
    xT = nc.dram_tensor("xT", [IN_DIM, N], F16, kind="ExternalInput")
    consts = nc.dram_tensor("consts", [128, CW], F16, kind="ExternalInput")
    dums = nc.dram_tensor("dums", [2, W1R], F16, kind="ExternalInput")
    idx1 = nc.dram_tensor("idx1", [128, C1], I16, kind="ExternalInput")
    idx2 = nc.dram_tensor("idx2", [128, C2], I16, kind="ExternalInput")
    out = nc.dram_tensor("out", [ROWS, OUT_DIM], F32, kind="ExternalOutput")
    dbg = nc.dram_tensor("dbg", [128, 512], F32, kind="ExternalOutput")

    HX1 = nc.dram_tensor("HX1", [HX1R, W1R], F16)
    SH2 = nc.dram_tensor("SH2", [ROWS, W2R], F16)
    HX2 = nc.dram_tensor("HX2", [HX2R, W2R], F16, addr_space="Shared")

    hx1_lo = HX1[0:LO1 + 1, :]
    hx1_hi = HX1[LO1 + 1:HX1R, :]
    hx2_lo = HX2[0:LO2, :]
    hx2_hi = HX2[LO2:HX2R, :]

    with tile.TileContext(nc) as tc:
        nc.gpsimd.load_library(library_config.mlp)
        with tc.tile_pool(name="cp", bufs=1) as cp:
            cb = cp.tile([128, CW], F16, tag="consts")
            nc.sync.dma_start(cb[:, :], consts[:, :])

            def C(name):
                off, w = cblocks[name]
                return cb[:, off:off + w]

            nc.sync.dma_start(HX1[LO1:LO1 + 1, :], dums[0:1, :])
            nc.sync.dma_start(HX1[HX1R - 1:HX1R, :], dums[1:2, :])

            # ---------------- P0: full HX1 table (fp16) -----------------
            SB = 1024
            nsb = -(-N // SB)
            with (
                tc.tile_pool(name="p0", bufs=3) as p0,
                tc.tile_pool(name="p0ps", bufs=1, space="PSUM") as p0ps,
            ):
                for sb in range(nsb):
                    base = sb * SB
                    cnt = min(SB, N - base)
                    nq = -(-cnt // P)
                    xt = p0.tile([P, 2 * cnt], F16, tag="xt")
                    xtv = xt[:, :].rearrange("p (q c) -> p q c", q=2)
                    nc.sync.dma_start(
                        xtv,
                        xT[:, base:base + cnt].rearrange(
                            "(q p) c -> p q c", p=P))
                    for g in range(2):
                        q0 = g * 4
                        gq = min(4, nq - q0)
                        if gq <= 0:
                            continue
                        ps = p0ps.tile([P, 4 * 512], F32, tag=f"ps{g}")
                        psv = ps[:, :].rearrange("p (q c) -> p q c", q=4)
                        for qi in range(gq):
                            q = q0 + qi
                            pb = min(P, cnt - q * P)
                            nc.tensor.matmul(psv[0:pb, qi, 0:W1R],
                                             xtv[:, 0, q * P:q * P + pb],
                                             C("w1x0"), start=True, stop=False)
                            nc.tensor.matmul(psv[0:pb, qi, 0:W1R],
                                             xtv[:, 1, q * P:q * P + pb],
                                             C("w1x1"), start=False, stop=True)
                        hx4 = p0.tile([P, gq * W1R], F16, tag=f"hx4{g}")
                        hx4v = hx4[:, :].rearrange("p (q w) -> p q w", q=gq)
                        hq = (gq + 1) // 2
                        nc.scalar.copy(hx4v[:, 0:hq, :], psv[:, 0:hq, 0:W1R])
                        if gq > hq:
                            nc.vector.tensor_scalar(
                                hx4v[:, hq:gq, :], psv[:, hq:gq, 0:W1R],
                                0.0, None, op0=OP.add)
                        gb = base + q0 * P
                        gcnt = min(4 * P, cnt - q0 * P)

                        def wr(a, b):   # node range [a, b) within group
                            if a >= b:
                                return
                            ra = gb + a + (1 if gb + a >= LO1 else 0)
                            dv = HX1[ra:ra + (b - a), :]
                            qa, pa = divmod(a, P)
                            qb, pb_ = divmod(b - 1, P)
                            if qa == qb:
                                nc.sync.dma_start(dv, hx4v[pa:pb_ + 1, qa, :])
                                return
                            if pa == 0 and pb_ == P - 1:
                                nc.sync.dma_start(
                                    dv.rearrange("(q p) w -> p q w", p=P),
                                    hx4v[:, qa:qb + 1, :])
                                return
                            n0 = P - pa
                            nc.sync.dma_start(dv[0:n0, :], hx4v[pa:P, qa, :])
                            off = n0
                            for q in range(qa + 1, qb):
                                nc.sync.dma_start(dv[off:off + P, :],
                                                  hx4v[0:P, q, :])
                                off += P
                            nc.sync.dma_start(dv[off:, :],
                                              hx4v[0:pb_ + 1, qb, :])

                        if gb < LO1 < gb + gcnt:
                            wr(0, LO1 - gb)
                            wr(LO1 - gb, gcnt)
                        else:
                            wr(0, gcnt)

            if phase == "p0":
                nc.sync.dma_start(dbg[0:128, 0:W1R].bitcast(F16),
                                  HX1[0:128, :])

            # ---------------- L1 + H2 + chunked AllGather ----------------
            _l2r_cm = tc.tile_pool(name="l2r", bufs=1)
            l2r = _l2r_cm.__enter__()
            IT2 = l2r.tile([128, C2], I16, tag="it2")
            nc.sync.dma_start(IT2[:, :], idx2[:, :])
            with (
                tc.tile_pool(name="l1r", bufs=1) as l1r,
                tc.tile_pool(name="l1", bufs=3) as l1,
                tc.tile_pool(name="l1b", bufs=4) as l1b,
                tc.tile_pool(name="l1ps", bufs=2, space="PSUM") as l1ps,
            ):
                IT1 = l1r.tile([128, C1], I16, tag="it1")
                nc.sync.dma_start(IT1[:, :], idx1[:, :])
                sd4 = l1r.tile([P, NT * HEADS], F16, tag="sd4")
                with tc.tile_pool(name="sdp", bufs=1) as sdp:
                    sdl = sdp.tile([P, NT * 128], F16, tag="sdl")
                    sdh = sdp.tile([P, NT * 128], F16, tag="sdh")
                    nc.gpsimd.dma_gather(
                        sdl[:, :].rearrange("p (t w) -> p t w", t=NT),
                        hx1_lo[:, IN_DIM:IN_DIM + 128], IT1[:, 0:NT * 8],
                        NT * P, NT * P, 128, elem_step=W1R,
                        single_packet=False)
                    nc.gpsimd.dma_gather(
                        sdh[:, :].rearrange("p (t w) -> p t w", t=NT),
                        hx1_hi[:, IN_DIM:IN_DIM + 128], IT1[:, NT * 8:NT * 16],
                        NT * P, NT * P, 128, elem_step=W1R,
                        single_packet=False)
                    sdlv = sdl[:, :].rearrange("p (t w) -> p t w", t=NT)
                    sdhv = sdh[:, :].rearrange("p (t w) -> p t w", t=NT)
                    nc.vector.tensor_tensor(
                        sd4[:, :].rearrange("p (t h) -> p t h", t=NT),
                        sdlv[:, :, HEADS:2 * HEADS],
                        sdhv[:, :, HEADS:2 * HEADS], op=OP.add)

                col = [2 * NT * 8]
                for t in (range(NT) if phase != "p0" else range(0)):
                    num = l1b.tile([P, NH2], F16, tag="num")
                    den = l1b.tile([P, HEADS], F32, tag="den")
                    def l1_mac(hg, hgv, w, kv, first):
                        dv = den if first else l1b.tile([P, HEADS], F32,
                                                        tag="dv")
                        nc.vector.tensor_reduce(
                            dv[:, :],
                            w[:, :].rearrange("p (j h) -> p h j", j=kv),
                            axis=AX.X, op=OP.add)
                        if not first:
                            nc.vector.tensor_tensor(den[:, :], den[:, :],
                                                    dv[:, :], op=OP.add)
                        nc.vector.tensor_tensor(
                            hgv[:, :, 0:NH2].rearrange(
                                "p j (c h) -> p j c h", h=HEADS),
                            hgv[:, :, 0:NH2].rearrange(
                                "p j (c h) -> p j c h", h=HEADS),
                            w[:, :].rearrange("p (j h) -> p j h", j=kv)
                            .unsqueeze(2).broadcast_to([P, kv, HID, HEADS]),
                            op=OP.mult)
                        n = kv
                        while n > 1:
                            if n % 2 == 1:
                                nc.vector.tensor_tensor(
                                    hgv[:, 0, 0:NH2], hgv[:, 0, 0:NH2],
                                    hgv[:, n - 1, 0:NH2], op=OP.add)
                                n -= 1
                            pairs = n // 2
                            hp = hg[:, 0:pairs * 2 * W1R].rearrange(
                                "p (j two w) -> p j two w", two=2, w=W1R)
                            nc.vector.tensor_tensor(
                                hgv[:, 0:pairs, 0:NH2],
                                hp[:, :, 0, 0:NH2], hp[:, :, 1, 0:NH2],
                                op=OP.add)
                            n = pairs
                        if first:
                            nc.vector.tensor_scalar(
                                num[:, :], hgv[:, 0, 0:NH2], 0.0, None,
                                op0=OP.add)
                        else:
                            nc.vector.tensor_tensor(num[:, :], num[:, :],
                                                    hgv[:, 0, 0:NH2],
                                                    op=OP.add)

                    pend = None
                    for v, (half, kv) in enumerate(vt1[t]):
                        itv = IT1[:, col[0]:col[0] + kv * 8]
                        col[0] += kv * 8
                        hg = l1.tile([P, kv * W1R], F16, tag="hg")
                        nc.gpsimd.dma_gather(
                            hg[:, :].rearrange("p (j w) -> p j w", j=kv),
                            (hx1_lo if half == 0 else hx1_hi)[:, :],
                            itv, P * kv, P * kv, W1R, single_packet=False)
                        hgv = hg[:, :].rearrange("p (j w) -> p j w", j=kv)
                        s = l1b.tile([P, kv * HEADS], F16, tag="s")
                        nc.vector.tensor_tensor(
                            s[:, :].rearrange("p (j h) -> p j h", j=kv),
                            hgv[:, :, IN_DIM:IN_DIM + HEADS],
                            sd4[:, t * HEADS:(t + 1) * HEADS].unsqueeze(1)
                            .broadcast_to([P, kv, HEADS]), op=OP.add)
                        w0 = l1b.tile([P, kv * HEADS], F16, tag="w0")
                        nc.vector.scalar_tensor_tensor(
                            w0[:, :], s[:, :], NEG_SLOPE, s[:, :],
                            op0=OP.mult, op1=OP.max)
                        w = l1b.tile([P, kv * HEADS], F16, tag="w")
                        nc.scalar.activation(w[:, :], w0[:, :], AF.Exp,
                                             bias=C("nshift"))
                        if pend is not None:
                            l1_mac(*pend)
                        pend = (hg, hgv, w, kv, v == 0)
                    if pend is not None:
                        l1_mac(*pend)
                    dinv = l1b.tile([P, HEADS], F32, tag="dinv")
                    nc.vector.tensor_scalar_max(dinv[:, :], den[:, :], 1e-6)
                    nc.vector.reciprocal(dinv[:, :], dinv[:, :])
                    o = l1b.tile([P, NH2], F16, tag="o")
                    nc.vector.tensor_tensor(
                        o[:, :].rearrange("p (c h) -> p c h", h=HEADS),
                        num[:, :].rearrange("p (c h) -> p c h", h=HEADS),
                        dinv[:, :].unsqueeze(1).broadcast_to(
                            [P, HID, HEADS]), op=OP.mult)
                    nc.vector.tensor_tensor(o[:, :], o[:, :], C("b1r"),
                                            op=OP.add)
                    # elu
                    m0 = l1b.tile([P, NH2], F16, tag="m0")
                    nc.vector.tensor_scalar_min(m0[:, :], o[:, :], 0.0)
                    em = l1b.tile([P, NH2], F16, tag="em")
                    nc.scalar.activation(em[:, :], m0[:, :], AF.Exp)
                    p1 = l1b.tile([P, NH2], F16, tag="p1")
                    nc.vector.tensor_scalar(p1[:, :], o[:, :], 0.0, -1.0,
                                            op0=OP.max, op1=OP.add)
                    eo = l1b.tile([P, NH2], F16, tag="eo")
                    nc.vector.tensor_tensor(eo[:, :], em[:, :], p1[:, :],
                                            op=OP.add)
                    # transpose + H2 matmul
                    pt = l1ps.tile([P, NH2], F16, tag="pt")
                    nc.tensor.transpose(pt[:, 0:P], eo[:, 0:P], C("ident"))
                    nc.tensor.transpose(pt[:, P:NH2], eo[:, P:NH2],
                                        C("ident"))
                    o1T = l1b.tile([P, NH2], F16, tag="o1T")
                    nc.scalar.copy(o1T[:, :], pt[:, :])
                    h2p = l1ps.tile([P, W2R], F32, tag="h2p")
                    nc.tensor.matmul(h2p[:, :], o1T[:, 0:P], C("w2x0"),
                                     start=True, stop=False)
                    nc.tensor.matmul(h2p[:, :], o1T[:, P:NH2], C("w2x1"),
                                     start=False, stop=True)
                    sh2 = l1b.tile([P, W2R], F16, tag="sh2")
                    nc.scalar.copy(sh2[:, :], h2p[:, :])
                    nc.sync.dma_start(SH2[t * P:(t + 1) * P, :], sh2[:, :])

                    if (t + 1) % CHT == 0 and phase == "full":
                        c = t // CHT
                        base = c * CH_ALL
                        if sim_collective:
                            for r in range(NC):
                                nc.sync.dma_start(
                                    HX2[base + r * CHROWS:
                                        base + (r + 1) * CHROWS, :],
                                    SH2[c * CHROWS:(c + 1) * CHROWS, :])
                        else:
                            nc.gpsimd.collective_compute(
                                "AllGather", OP.bypass,
                                replica_groups=[list(range(NC))],
                                ins=[SH2[c * CHROWS:(c + 1) * CHROWS, :].opt()],
                                outs=[HX2[base:base + CH_ALL, :].opt()],
                            )

            if phase == "l1":
                nc.sync.dma_start(dbg[0:128, 0:W2R].bitcast(F16),
                                  SH2[0:128, :])
            # ---------------- L2 + MLP + normalize ----------------------
            with (
                tc.tile_pool(name="l2", bufs=4) as l2,
                tc.tile_pool(name="l2b", bufs=4) as l2b,
                tc.tile_pool(name="l2ps", bufs=4, space="PSUM") as l2ps,
            ):
                sd2g = l2r.tile([P, NT * 128], F16, tag="sd2g")
                nc.gpsimd.dma_gather(
                    sd2g[:, :].rearrange("p (t w) -> p t w", t=NT),
                    SH2[:, OUT_DIM:OUT_DIM + 128], IT2[:, 0:NT * 8],
                    NT * P, NT * P, 128, elem_step=W2R, single_packet=False)
                sd2v = sd2g[:, :].rearrange("p (t w) -> p t w", t=NT)
                sd2f = l2r.tile([P, NT], F32, tag="sd2f")
                nc.vector.tensor_scalar(sd2f[:, :].unsqueeze(2),
                                        sd2v[:, :, 1:2], 0.0, None,
                                        op0=OP.add)

                o2a = l2r.tile([P, NT * OUT_DIM], F16, tag="o2a")
                magic = l2r.tile([P, 1], I32, tag="magic")
                nc.vector.memset(magic[:, :], 0x5F3759DF)
                col2 = [NT * 8]
                for t in (range(NT) if phase == "full" else range(0)):
                    num2 = l2b.tile([P, OUT_DIM], F16, tag="num2")
                    den2 = l2b.tile([P, 1], F32, tag="den2")
                    def l2_mac(hg, hgv, w, dv, kv, first):
                        if not first:
                            nc.vector.tensor_tensor(den2[:, :], den2[:, :],
                                                    dv[:, :], op=OP.add)
                        for j in range(kv):
                            nc.vector.tensor_scalar(
                                hgv[:, j, 0:OUT_DIM], hgv[:, j, 0:OUT_DIM],
                                w[:, j:j + 1], None, op0=OP.mult)
                        n = kv
                        while n > 1:
                            if n % 2 == 1:
                                nc.vector.tensor_tensor(
                                    hgv[:, 0, 0:OUT_DIM], hgv[:, 0, 0:OUT_DIM],
                                    hgv[:, n - 1, 0:OUT_DIM], op=OP.add)
                                n -= 1
                            pairs = n // 2
                            hp = hg[:, 0:pairs * 2 * W2R].rearrange(
                                "p (j two w) -> p j two w", two=2, w=W2R)
                            nc.vector.tensor_tensor(
                                hgv[:, 0:pairs, 0:OUT_DIM],
                                hp[:, :, 0, 0:OUT_DIM], hp[:, :, 1, 0:OUT_DIM],
                                op=OP.add)
                            n = pairs
                        if first:
                            nc.vector.tensor_scalar(
                                num2[:, :], hgv[:, 0, 0:OUT_DIM], 0.0, None,
                                op0=OP.add)
                        else:
                            nc.vector.tensor_tensor(num2[:, :], num2[:, :],
                                                    hgv[:, 0, 0:OUT_DIM],
                                                    op=OP.add)

                    pend = None
                    for v, (half, kv) in enumerate(vt2[t]):
                        itv = IT2[:, col2[0]:col2[0] + kv * 8]
                        col2[0] += kv * 8
                        ib = IT2[:, col2[0]:col2[0] + kv].bitcast(F16)
                        col2[0] += kv
                        hg = l2.tile([P, kv * W2R], F16, tag="hg2")
                        nc.gpsimd.dma_gather(
                            hg[:, :].rearrange("p (j w) -> p j w", j=kv),
                            (hx2_lo if half == 0 else hx2_hi)[:, :],
                            itv, P * kv, P * kv, W2R, single_packet=False)
                        hgv = hg[:, :].rearrange("p (j w) -> p j w", j=kv)
                        s = l2b.tile([P, kv], F16, tag="s2")
                        nc.vector.tensor_scalar(
                            s[:, :].unsqueeze(2),
                            hgv[:, :, OUT_DIM:OUT_DIM + 1],
                            sd2f[:, t:t + 1], None, op0=OP.add)
                        nc.vector.tensor_tensor(s[:, :], s[:, :], ib,
                                                op=OP.add)
                        w0 = l2b.tile([P, kv], F16, tag="w02")
                        nc.vector.scalar_tensor_tensor(
                            w0[:, :], s[:, :], NEG_SLOPE, s[:, :],
                            op0=OP.mult, op1=OP.max)
                        w = l2b.tile([P, kv], F32, tag="w2")
                        dv = den2 if v == 0 else l2b.tile([P, 1], F32,
                                                          tag="dv2")
                        nc.scalar.activation(w[:, :], w0[:, :], AF.Exp,
                                             bias=C("nshift"),
                                             accum_out=dv[:, :])
                        if pend is not None:
                            l2_mac(*pend)
                        pend = (hg, hgv, w, dv, kv, v == 0)
                    if pend is not None:
                        l2_mac(*pend)
                    dinv = l2b.tile([P, 1], F32, tag="dinv2")
                    nc.vector.tensor_scalar_max(dinv[:, :], den2[:, :], 1e-6)
                    nc.vector.reciprocal(dinv[:, :], dinv[:, :])
                    o2 = o2a[:, t * OUT_DIM:(t + 1) * OUT_DIM]
                    nc.vector.tensor_scalar(o2, num2[:, :],
                                            dinv[:, 0:1], None, op0=OP.mult)
                    nc.vector.tensor_tensor(o2, o2, C("b2r"), op=OP.add)

                for t in (range(NT) if phase == "full" else range(0)):
                    o2 = o2a[:, t * OUT_DIM:(t + 1) * OUT_DIM]
                    # MLP (all PSUM packed into one bank per tile)
                    mlp = l2ps.tile([P, 512], F32, tag="mlp")
                    pt2 = mlp[:, 0:64].bitcast(F16)
                    nc.tensor.transpose(pt2, o2, C("ident"))
                    o2T = l2b.tile([P, P], F16, tag="o2T")
                    nc.scalar.copy(o2T[:, :], pt2)
                    h3p = mlp[:, 64:128]
                    nc.tensor.matmul(h3p, o2T[:, :], C("wm1"),
                                     start=True, stop=True)
                    h3 = l2b.tile([P, HID], F16, tag="h3")
                    nc.vector.tensor_tensor(h3[:, :], h3p, C("bm1r"),
                                            op=OP.add)
                    nc.vector.tensor_scalar_max(h3[:, :], h3[:, :], 0.0)
                    pt3 = mlp[0:HID, 128:192].bitcast(F16)
                    nc.tensor.transpose(pt3, h3[:, :], C("ident"))
                    h3T = l2b.tile([HID, P], F16, tag="h3T")
                    nc.scalar.copy(h3T[:, :], pt3)
                    h4p = mlp[:, 256:384]
                    nc.tensor.matmul(h4p, h3T[0:HID, :],
                                     C("wm2")[0:HID, :], start=True, stop=True)
                    h4 = l2b.tile([P, OUT_DIM], F32, tag="h4")
                    nc.vector.tensor_tensor(h4[:, :], h4p, C("bm2r"),
                                            op=OP.add)
                    hsq = l2b.tile([P, OUT_DIM], F32, tag="hsq")
                    nc.vector.tensor_tensor(hsq[:, :], h4[:, :], h4[:, :],
                                            op=OP.mult)
                    n2 = l2b.tile([P, 1], F32, tag="n2")
                    nc.vector.tensor_reduce(n2[:, :], hsq[:, :], axis=AX.X,
                                            op=OP.add)
                    nin = l2b.tile([P, 1], F32, tag="nin")
                    nc.vector.tensor_scalar_max(nin[:, :], n2[:, :], 1e-12)
                    # Newton rsqrt (keeps Sqrt off the Act engine: avoids
                    # act-table thrash between Exp and Sqrt function sets)
                    yr = l2b.tile([P, 1], F32, tag="yr")
                    tn = l2b.tile([P, 1], F32, tag="tn")
                    nc.vector.tensor_scalar(yr[:, :].bitcast(I32),
                                            nin[:, :].bitcast(I32), 1, None,
                                            op0=OP.arith_shift_right)
                    nc.vector.tensor_tensor(yr[:, :].bitcast(I32),
                                            magic[:, :],
                                            yr[:, :].bitcast(I32),
                                            op=OP.subtract)
                    for _ in range(3):
                        nc.vector.tensor_tensor(tn[:, :], yr[:, :], yr[:, :],
                                                op=OP.mult)
                        nc.vector.tensor_tensor(tn[:, :], tn[:, :], nin[:, :],
                                                op=OP.mult)
                        nc.vector.tensor_scalar(tn[:, :], tn[:, :], -0.5, 1.5,
                                                op0=OP.mult, op1=OP.add)
                        nc.vector.tensor_tensor(yr[:, :], yr[:, :], tn[:, :],
                                                op=OP.mult)
                    nin = yr
                    of = l2b.tile([P, OUT_DIM], F32, tag="of")
                    nc.vector.tensor_scalar(of[:, :], h4[:, :], nin[:, 0:1],
                                            None, op0=OP.mult)
                    nc.sync.dma_start(out[t * P:(t + 1) * P, :], of[:, :])
            _l2r_cm.__exit__(None, None, None)

    nc.compile()
    return nc


# ------------------------------------------------------------------ driver

def run(cfg, inputs, trace=False, phase="full", sim_collective=False,
        prep=None):
    x = np.asarray(inputs["x"], dtype=np.float32)
    edge_index = np.asarray(inputs["edge_index"])
    if prep is None:
        prep = _prep(cfg, edge_index)
    consts, cblocks, dums = _pack_consts(
        cfg, *[np.asarray(inputs[k], dtype=np.float32) for k in
               ("W1", "a1_src", "a1_dst", "b1", "W2", "a2_src", "a2_dst",
                "b2", "Wm1", "bm1", "Wm2", "bm2")])
    nc = _build(cfg, prep, cblocks, consts.shape[1], phase=phase,
                sim_collective=sim_collective)
    xT = np.ascontiguousarray(x.T.astype(np.float16))
    in_maps = []
    for k in range(cfg["NC"]):
        in_maps.append({
            "xT": xT, "consts": consts, "dums": dums,
            "idx1": np.ascontiguousarray(prep["blobs1"][k]),
            "idx2": np.ascontiguousarray(prep["blobs2"][k]),
        })
    res = run_bass_kernel_spmd(nc, in_maps, list(range(cfg["NC"])),
                               trace=trace)
    N, NC, SHARD = cfg["N"], cfg["NC"], cfg["SHARD"]
    full = np.zeros((N, cfg["OUT_DIM"]), dtype=np.float32)
    for k in range(NC):
        o = res.results[k]["out"]
        perm2 = prep["cores"][k]["perm2"]
        real = perm2 >= 0
        full[k * SHARD + perm2[real]] = o[real]
    return full, res


def kernel(**inputs):
    cfg = make_cfg()
    full, _ = run(cfg, inputs, trace=False)
    return full


# revision 20
# speedup vs baseline: 2.9559x; 1.5052x over previous
"""Trainium2 Bass kernel for a 2-layer GAT + MLP (nn_MemoryGNN).

Strategy (8 NeuronCores, SPMD):
  - Destination-node partition: core k owns dst nodes [k*6250, (k+1)*6250).
  - Every core computes the FULL HX1 table (fp16): row n -> [h1 (256,
    head-minor c*4+h) | ssrc (4) | sdst (4) | pad] = 384 cols (768B rows),
    so layer-1 message gathers are purely local.
  - Per-edge gathers use gpsimd.dma_gather from per-dst-tile slot tables
    (host-precomputed int16 index blobs, degree-sorted tiles).
  - Softmax is computed unnormalized with a global exp-shift of -2 (folded
    into the Act-engine exp bias); the shift cancels in num/den.
  - fp16 everywhere in the hot loops (DVE 2x eligible); accumulation of the
    weighted message sum is an fp16 add-chain (err ~0.1% << 2e-2 tol).
  - Layer 2 (fp16 512B rows) needs remote H2 rows: a chunked AllGather of
    per-core SH2 shards into HX2 (Shared) overlaps with layer-1 compute.
  - Output rows are produced in a degree-sorted permuted order; the host
    applies the inverse permutation (free).
"""

import sys
import numpy as np

for _p in ("/opt/trn_rl_repo", "/root/.axon_site/_ro/trn_rl_repo"):
    if _p not in sys.path:
        sys.path.insert(0, _p)

import concourse.bass as bass
import concourse.bacc as bacc
import concourse.mybir as mybir
import concourse.tile as tile
from concourse import library_config
from concourse.bass_utils import run_bass_kernel_spmd

F32 = mybir.dt.float32
F16 = mybir.dt.float16
I16 = mybir.dt.int16
I32 = mybir.dt.int32
AF = mybir.ActivationFunctionType
OP = mybir.AluOpType
AX = mybir.AxisListType

NEG_SLOPE = 0.2
SHIFT = 2.0          # global exp shift (cancels in num/den)
PAD_BIAS = -30000.0  # fp16-safe "minus infinity" for masked slots


def make_cfg(N=50000, E=1000000, IN_DIM=256, HID=64, HEADS=4, OUT_DIM=128,
             NC=8, CHT=7, KCAP1=48, KCAP2=64):
    cfg = dict(N=N, E=E, IN_DIM=IN_DIM, HID=HID, HEADS=HEADS, OUT_DIM=OUT_DIM,
               NC=NC, CHT=CHT, KCAP1=KCAP1, KCAP2=KCAP2)
    cfg["SHARD"] = N // NC
    assert N % NC == 0
    TP = 128
    cfg["TP"] = TP
    NT = -(-cfg["SHARD"] // TP)
    cfg["NT"] = NT
    assert NT % CHT == 0, (NT, CHT)
    cfg["NCH"] = NT // CHT
    cfg["ROWS"] = NT * TP
    cfg["CHROWS"] = CHT * TP          # SH2 rows per AllGather chunk
    # layer-1 table: node-order rows (AllGather output); masked pads
    cfg["D1"] = IN_DIM + 2 * HEADS
    cfg["W1R"] = 384
    assert cfg["W1R"] * 2 % 256 == 0
    cfg["LO1"] = 32768
    assert N - cfg["LO1"] <= 32767
    cfg["HX1_ROWS"] = N
    # layer-2 table (chunk-major, no dummies; masked pads instead)
    cfg["D2"] = OUT_DIM + 2
    cfg["W2R"] = 256
    CH_ALL = cfg["CHROWS"] * NC       # global rows per chunk
    cfg["CH_ALL"] = CH_ALL
    LOCH = 32768 // CH_ALL            # chunks fully addressable as lo
    LOCH = max(1, min(cfg["NCH"] - 1, LOCH))
    cfg["LOCH"] = LOCH
    cfg["LO2ROWS"] = LOCH * CH_ALL
    assert cfg["LO2ROWS"] <= 32767
    assert (cfg["NCH"] - LOCH) * CH_ALL <= 32767
    cfg["HX2_ROWS"] = cfg["NCH"] * CH_ALL
    return cfg


# ----------------------------------------------------------------- host prep

def _wrap16(flat):
    """flat int array (len divisible by 16) -> wrapped [128, n/16] int16."""
    w = flat.reshape(-1, 16).T.astype(np.int16)
    return np.tile(w, (8, 1))


def _chunks(total, cap):
    out = []
    off = 0
    while off < total:
        kv = min(cap, total - off)
        out.append(kv)
        off += kv
    return out


def _sort_core(cfg, lo_cnt, hi_cnt):
    """Degree-sort local dsts into tiles; return perm and per-tile maxima."""
    SHARD, TP, NT = cfg["SHARD"], cfg["TP"], cfg["NT"]
    order = np.lexsort((-hi_cnt, -(lo_cnt // 6)))
    perm = np.full(NT * TP, -1, dtype=np.int64)
    perm[:SHARD] = order
    kl_t = np.zeros(NT, dtype=np.int64)
    kh_t = np.zeros(NT, dtype=np.int64)
    for t in range(NT):
        rows = perm[t * TP:(t + 1) * TP]
        real = rows[rows >= 0]
        if len(real):
            kl_t[t] = lo_cnt[real].max()
            kh_t[t] = hi_cnt[real].max()
    return perm, kl_t, kh_t


def _prep(cfg, edge_index):
    """Host preprocessing (structure only). Returns per-core blobs + meta."""
    N, NC, SHARD = cfg["N"], cfg["NC"], cfg["SHARD"]
    TP, NT, ROWS = cfg["TP"], cfg["NT"], cfg["ROWS"]
    KCAP1, KCAP2 = cfg["KCAP1"], cfg["KCAP2"]
    LO1 = cfg["LO1"]
    src = np.concatenate([np.asarray(edge_index[0]),
                          np.arange(N)]).astype(np.int64)
    dst = np.concatenate([np.asarray(edge_index[1]),
                          np.arange(N)]).astype(np.int64)
    order = np.argsort(dst, kind="stable")
    src_s = src[order]
    deg = np.bincount(dst, minlength=N)
    rp = np.zeros(N + 1, dtype=np.int64)
    np.cumsum(deg, out=rp[1:])

    # L1 table indexing: lo rows = node n (< LO1); hi rows = n - LO1.
    # Pads point at row 0 and are killed by a PAD_BIAS score mask.

    cores = []
    for k in range(NC):
        g0 = k * SHARD
        srcs = [src_s[rp[g0 + r]:rp[g0 + r + 1]] for r in range(SHARD)]
        lo_cnt = np.array([int((s < LO1).sum()) for s in srcs])
        hi_cnt = np.array([len(s) for s in srcs]) - lo_cnt
        perm1, kl1, kh1 = _sort_core(cfg, lo_cnt, hi_cnt)
        cores.append(dict(srcs=srcs, perm1=perm1, kl1=kl1, kh1=kh1))

    KL1 = np.max([c["kl1"] for c in cores], axis=0)
    KH1 = np.max([c["kh1"] for c in cores], axis=0)
    vt1 = [[(0, kv) for kv in _chunks(int(KL1[t]), KCAP1)] +
           [(1, kv) for kv in _chunks(int(KH1[t]), KCAP1)] for t in range(NT)]

    # position of each global node in its owner's tile order
    pos1 = np.empty(N, dtype=np.int64)
    for k in range(NC):
        p = cores[k]["perm1"]
        q = np.arange(ROWS)
        real = p >= 0
        pos1[k * SHARD + p[real]] = q[real]
    CHROWS, CH_ALL = cfg["CHROWS"], cfg["CH_ALL"]
    c_of = pos1 // CHROWS
    r_of = pos1 % CHROWS
    owner = np.arange(N) // SHARD
    cm = c_of * CH_ALL + owner * CHROWS + r_of      # HX2 row of node
    LO2 = cfg["LO2ROWS"]

    for k in range(NC):
        c = cores[k]
        rows2 = [cm[s] for s in c["srcs"]]
        lo2 = np.array([int((r < LO2).sum()) for r in rows2])
        hi2 = np.array([len(r) for r in rows2]) - lo2
        perm2, kl2, kh2 = _sort_core(cfg, lo2, hi2)
        c.update(rows2=rows2, perm2=perm2, kl2=kl2, kh2=kh2)

    KL2 = np.max([c["kl2"] for c in cores], axis=0)
    KH2 = np.max([c["kh2"] for c in cores], axis=0)
    vt2 = [[(0, kv) for kv in _chunks(int(KL2[t]), KCAP2)] +
           [(1, kv) for kv in _chunks(int(KH2[t]), KCAP2)] for t in range(NT)]

    pad_bits = np.float16(PAD_BIAS).view(np.int16)
    HEADS_C = cfg["HEADS"]

    blobs1, blobs2 = [], []
    for k in range(NC):
        c = cores[k]
        g0 = k * SHARD
        # ---- blob1: [sd idx (own-table rows) | per-tile slot idx+bias] ----
        cols = []
        for t in range(NT):
            rows = c["perm1"][t * TP:(t + 1) * TP]
            d1 = np.where(rows >= 0, rows, 0)
            cols.append(_wrap16(d1))
        sd_cols = [np.concatenate(cols, axis=1)]
        tile_cols = []
        tile_off1 = []
        for t in range(NT):
            rows = c["perm1"][t * TP:(t + 1) * TP]
            la, ha = int(KL1[t]), int(KH1[t])
            lo_mat = np.zeros((TP, max(1, la)), dtype=np.int64)
            hi_mat = np.zeros((TP, max(1, ha)), dtype=np.int64)
            lo_msk = np.full((TP, max(1, la)), pad_bits, dtype=np.int16)
            hi_msk = np.full((TP, max(1, ha)), pad_bits, dtype=np.int16)
            for p, r in enumerate(rows):
                if r >= 0:
                    s = c["srcs"][r]
                    lo = s[s < LO1]
                    hi = s[s >= LO1] - LO1
                    lo_mat[p, :len(lo)] = lo
                    hi_mat[p, :len(hi)] = hi
                    lo_msk[p, :len(lo)] = 0
                    hi_msk[p, :len(hi)] = 0
            off_l = off_h = 0
            for half, kv in vt1[t]:
                if half == 0:
                    m, b = lo_mat[:, off_l:off_l + kv], lo_msk[:, off_l:off_l + kv]
                    off_l += kv
                else:
                    m, b = hi_mat[:, off_h:off_h + kv], hi_msk[:, off_h:off_h + kv]
                    off_h += kv
                tile_cols.append(_wrap16(m.T.reshape(-1)))
                tile_cols.append(np.repeat(b, HEADS_C, axis=1))  # bias4
        blobs1.append(np.concatenate(sd_cols + tile_cols, axis=1))

        # ---- blob2: [sd2 idx (SH2 rows) | per-tile (slot idx + bias)] ----
        pos1_local = np.zeros(SHARD, dtype=np.int64)
        p1 = c["perm1"]
        for q in range(ROWS):
            if p1[q] >= 0:
                pos1_local[p1[q]] = q
        first_real = int(c["perm2"][0])
        cols = []
        for t in range(NT):
            rows = c["perm2"][t * TP:(t + 1) * TP]
            d2 = np.empty(TP, dtype=np.int64)
            for p, r in enumerate(rows):
                d2[p] = pos1_local[r if r >= 0 else first_real]
            cols.append(_wrap16(d2))
        tile_cols = [np.concatenate(cols, axis=1)]
        for t in range(NT):
            rows = c["perm2"][t * TP:(t + 1) * TP]
            la, ha = int(KL2[t]), int(KH2[t])
            lo_mat = np.zeros((TP, max(1, la)), dtype=np.int64)
            hi_mat = np.zeros((TP, max(1, ha)), dtype=np.int64)
            lo_msk = np.zeros((TP, max(1, la)), dtype=np.int16)
            hi_msk = np.zeros((TP, max(1, ha)), dtype=np.int16)
            lo_msk[:] = pad_bits
            hi_msk[:] = pad_bits
            for p, r in enumerate(rows):
                if r >= 0:
                    s = c["rows2"][r]
                    lo = s[s < LO2]
                    hi = s[s >= LO2] - LO2
                    lo_mat[p, :len(lo)] = lo
                    hi_mat[p, :len(hi)] = hi
                    lo_msk[p, :len(lo)] = 0
                    hi_msk[p, :len(hi)] = 0
            off_l = off_h = 0
            for half, kv in vt2[t]:
                if half == 0:
                    m, b = lo_mat[:, off_l:off_l + kv], lo_msk[:, off_l:off_l + kv]
                    off_l += kv
                else:
                    m, b = hi_mat[:, off_h:off_h + kv], hi_msk[:, off_h:off_h + kv]
                    off_h += kv
                tile_cols.append(_wrap16(m.T.reshape(-1)))
                tile_cols.append(b.astype(np.int16))   # [128, kv] plain layout
            # bias bits are fp16 PAD_BIAS for masked slots, 0 for real
        blobs2.append(np.concatenate(tile_cols, axis=1))

    return dict(cores=cores, blobs1=blobs1, blobs2=blobs2,
                vt1=vt1, vt2=vt2, KL1=KL1, KH1=KH1, KL2=KL2, KH2=KH2)


def _hm(mat, HEADS, HID):
    """Reorder columns from head-major (h*HID+c) to head-minor (c*HEADS+h)."""
    n = mat.shape[1] if mat.ndim == 2 else mat.shape[0]
    assert n == HEADS * HID
    idx = np.empty(n, dtype=np.int64)
    for h in range(HEADS):
        for c in range(HID):
            idx[c * HEADS + h] = h * HID + c
    return mat[:, idx] if mat.ndim == 2 else mat[idx]


def _pack_consts(cfg, W1, a1_src, a1_dst, b1, W2, a2_src, a2_dst, b2,
                 Wm1, bm1, Wm2, bm2):
    IN_DIM, HID, HEADS, OUT_DIM = (cfg["IN_DIM"], cfg["HID"], cfg["HEADS"],
                                   cfg["OUT_DIM"])
    W1R, W2R = cfg["W1R"], cfg["W2R"]
    U1 = np.einsum("khc,hc->kh", W1.reshape(IN_DIM, HEADS, HID), a1_src)
    V1 = np.einsum("khc,hc->kh", W1.reshape(IN_DIM, HEADS, HID), a1_dst)
    W1X = np.zeros((IN_DIM, W1R), dtype=np.float32)
    W1X[:, :IN_DIM] = _hm(W1, HEADS, HID)
    W1X[:, IN_DIM:IN_DIM + HEADS] = U1
    W1X[:, IN_DIM + HEADS:IN_DIM + 2 * HEADS] = V1
    u2 = W2 @ a2_src[0]
    v2 = W2 @ a2_dst[0]
    # rows of W2 permuted to head-minor order (o1T rows are head-minor)
    W2X = np.zeros((HEADS * HID, W2R), dtype=np.float32)
    W2X[:, :OUT_DIM] = W2
    W2X[:, OUT_DIM] = u2
    W2X[:, OUT_DIM + 1] = v2
    W2Xp = _hm(W2X.T, HEADS, HID).T
    P = 128
    blocks = {}
    cols = 0

    def add(name, arr):
        nonlocal cols
        a = np.zeros((P, arr.shape[1]), dtype=np.float16)
        a[:arr.shape[0]] = arr.astype(np.float16)
        blocks[name] = (cols, arr.shape[1])
        cols += arr.shape[1]
        return a

    parts = []
    parts.append(add("w1x0", W1X[0:P]))
    parts.append(add("w1x1", W1X[P:2 * P]))
    parts.append(add("w2x0", W2Xp[0:P]))
    parts.append(add("w2x1", W2Xp[P:2 * P]))
    parts.append(add("wm1", Wm1.astype(np.float32)))
    parts.append(add("wm2", Wm2.astype(np.float32)))
    parts.append(add("ident", np.eye(P, dtype=np.float32)))
    parts.append(add("b1r", np.tile(_hm(b1.astype(np.float32)[None, :],
                                        HEADS, HID), (P, 1))))
    parts.append(add("b2r", np.tile(b2.astype(np.float32), (P, 1))))
    parts.append(add("bm1r", np.tile(bm1.astype(np.float32), (P, 1))))
    parts.append(add("bm2r", np.tile(bm2.astype(np.float32), (P, 1))))
    parts.append(add("nshift", np.full((P, 1), -SHIFT, dtype=np.float32)))
    consts = np.concatenate(parts, axis=1)
    return consts, blocks


# ------------------------------------------------------------- device build

def _build(cfg, prep, cblocks, CW, phase="full", sim_collective=False):
    N, NC = cfg["N"], cfg["NC"]
    IN_DIM, HID, HEADS, OUT_DIM = (cfg["IN_DIM"], cfg["HID"], cfg["HEADS"],
                                   cfg["OUT_DIM"])
    TP, NT, ROWS, SHARD = cfg["TP"], cfg["NT"], cfg["ROWS"], cfg["SHARD"]
    W1R, LO1 = cfg["W1R"], cfg["LO1"]
    W2R, LO2 = cfg["W2R"], cfg["LO2ROWS"]
    CHT, NCH, CHROWS, CH_ALL = (cfg["CHT"], cfg["NCH"], cfg["CHROWS"],
                                cfg["CH_ALL"])
    HX1R, HX2R = cfg["HX1_ROWS"], cfg["HX2_ROWS"]
    vt1, vt2 = prep["vt1"], prep["vt2"]
    C1 = prep["blobs1"][0].shape[1]
    C2 = prep["blobs2"][0].shape[1]
    NH2 = HEADS * HID
    P = 128

    nc = bacc.Bacc("TRN2", target_bir_lowering=False, debug=False,
                   num_devices=NC)
    xT = nc.dram_tensor("xT", [IN_DIM, SHARD], F16, kind="ExternalInput")
    consts = nc.dram_tensor("consts", [128, CW], F16, kind="ExternalInput")
    idx1 = nc.dram_tensor("idx1", [128, C1], I16, kind="ExternalInput")
    idx2 = nc.dram_tensor("idx2", [128, C2], I16, kind="ExternalInput")
    out = nc.dram_tensor("out", [ROWS, OUT_DIM], F32, kind="ExternalOutput")
    dbg = nc.dram_tensor("dbg", [128, 512], F32, kind="ExternalOutput")

    OWN = nc.dram_tensor("OWN", [SHARD, W1R], F16)
    HX1 = nc.dram_tensor("HX1", [HX1R, W1R], F16, addr_space="Shared")
    SH2 = nc.dram_tensor("SH2", [ROWS, W2R], F16)
    HX2 = nc.dram_tensor("HX2", [HX2R, W2R], F16, addr_space="Shared")

    hx1_lo = HX1[0:LO1, :]
    hx1_hi = HX1[LO1:HX1R, :]
    hx2_lo = HX2[0:LO2, :]
    hx2_hi = HX2[LO2:HX2R, :]

    with tile.TileContext(nc) as tc:
        nc.gpsimd.load_library(library_config.mlp)
        with tc.tile_pool(name="cp", bufs=1) as cp:
            cb = cp.tile([128, CW], F16, tag="consts")
            nc.sync.dma_start(cb[:, :], consts[:, :])

            def C(name):
                off, w = cblocks[name]
                return cb[:, off:off + w]

            # ------------- P0: own-shard table + AllGather ---------------
            SB = 1024
            nsb = -(-SHARD // SB)
            with (
                tc.tile_pool(name="p0", bufs=3) as p0,
                tc.tile_pool(name="p0ps", bufs=1, space="PSUM") as p0ps,
            ):
                for sb in range(nsb):
                    base = sb * SB
                    cnt = min(SB, SHARD - base)
                    nq = -(-cnt // P)
                    xt = p0.tile([P, 2 * cnt], F16, tag="xt")
                    xtv = xt[:, :].rearrange("p (q c) -> p q c", q=2)
                    nc.sync.dma_start(
                        xtv,
                        xT[:, base:base + cnt].rearrange(
                            "(q p) c -> p q c", p=P))
                    for g in range(2):
                        q0 = g * 4
                        gq = min(4, nq - q0)
                        if gq <= 0:
                            continue
                        ps = p0ps.tile([P, 4 * 512], F32, tag=f"ps{g}")
                        psv = ps[:, :].rearrange("p (q c) -> p q c", q=4)
                        for qi in range(gq):
                            q = q0 + qi
                            pb = min(P, cnt - q * P)
                            nc.tensor.matmul(psv[0:pb, qi, 0:W1R],
                                             xtv[:, 0, q * P:q * P + pb],
                                             C("w1x0"), start=True, stop=False)
                            nc.tensor.matmul(psv[0:pb, qi, 0:W1R],
                                             xtv[:, 1, q * P:q * P + pb],
                                             C("w1x1"), start=False, stop=True)
                        hx4 = p0.tile([P, gq * W1R], F16, tag=f"hx4{g}")
                        hx4v = hx4[:, :].rearrange("p (q w) -> p q w", q=gq)
                        hq = (gq + 1) // 2
                        nc.scalar.copy(hx4v[:, 0:hq, :], psv[:, 0:hq, 0:W1R])
                        if gq > hq:
                            nc.vector.tensor_scalar(
                                hx4v[:, hq:gq, :], psv[:, hq:gq, 0:W1R],
                                0.0, None, op0=OP.add)
                        gb = base + q0 * P
                        gcnt = min(4 * P, cnt - q0 * P)

                        def wr(a, b):   # row range [a, b) within group
                            if a >= b:
                                return
                            ra = gb + a
                            dv = OWN[ra:ra + (b - a), :]
                            qa, pa = divmod(a, P)
                            qb, pb_ = divmod(b - 1, P)
                            if qa == qb:
                                nc.sync.dma_start(dv, hx4v[pa:pb_ + 1, qa, :])
                                return
                            if pa == 0 and pb_ == P - 1:
                                nc.sync.dma_start(
                                    dv.rearrange("(q p) w -> p q w", p=P),
                                    hx4v[:, qa:qb + 1, :])
                                return
                            n0 = P - pa
                            nc.sync.dma_start(dv[0:n0, :], hx4v[pa:P, qa, :])
                            off = n0
                            for q in range(qa + 1, qb):
                                nc.sync.dma_start(dv[off:off + P, :],
                                                  hx4v[0:P, q, :])
                                off += P
                            nc.sync.dma_start(dv[off:, :],
                                              hx4v[0:pb_ + 1, qb, :])

                        wr(0, gcnt)

            if sim_collective:
                for r in range(NC):
                    nc.sync.dma_start(HX1[r * SHARD:(r + 1) * SHARD, :],
                                      OWN[0:SHARD, :])
            else:
                nc.gpsimd.collective_compute(
                    "AllGather", OP.bypass,
                    replica_groups=[list(range(NC))],
                    ins=[OWN[0:SHARD, :].opt()],
                    outs=[HX1[0:N, :].opt()],
                )

            if phase == "p0":
                nc.sync.dma_start(dbg[0:128, 0:W1R].bitcast(F16),
                                  HX1[0:128, :])

            # ---------------- L1 + H2 + chunked AllGather ----------------
            _l2r_cm = tc.tile_pool(name="l2r", bufs=1)
            l2r = _l2r_cm.__enter__()
            IT2 = l2r.tile([128, C2], I16, tag="it2")
            nc.sync.dma_start(IT2[:, :], idx2[:, :])
            with (
                tc.tile_pool(name="l1r", bufs=1) as l1r,
                tc.tile_pool(name="l1", bufs=3) as l1,
                tc.tile_pool(name="l1b", bufs=4) as l1b,
                tc.tile_pool(name="l1ps", bufs=2, space="PSUM") as l1ps,
            ):
                ITS = l1r.tile([128, NT * 8], I16, tag="its")
                nc.sync.dma_start(ITS[:, :], idx1[:, 0:NT * 8])
                sdg = l1r.tile([P, NT * 128], F16, tag="sdg")
                nc.gpsimd.dma_gather(
                    sdg[:, :].rearrange("p (t w) -> p t w", t=NT),
                    OWN[:, IN_DIM:IN_DIM + 128], ITS[:, :],
                    NT * P, NT * P, 128, elem_step=W1R,
                    single_packet=False)
                sd4 = sdg[:, :].rearrange("p (t w) -> p t w", t=NT)

                col = [NT * 8]
                for t in (range(NT) if phase != "p0" else range(0)):
                    num = l1b.tile([P, NH2], F16, tag="num")
                    den = l1b.tile([P, HEADS], F32, tag="den")
                    def l1_mac(hg, hgv, w, kv, first):
                        dv = den if first else l1b.tile([P, HEADS], F32,
                                                        tag="dv")
                        nc.vector.tensor_reduce(
                            dv[:, :],
                            w[:, :].rearrange("p (j h) -> p h j", j=kv),
                            axis=AX.X, op=OP.add)
                        if not first:
                            nc.vector.tensor_tensor(den[:, :], den[:, :],
                                                    dv[:, :], op=OP.add)
                        nc.vector.tensor_tensor(
                            hgv[:, :, 0:NH2].rearrange(
                                "p j (c h) -> p j c h", h=HEADS),
                            hgv[:, :, 0:NH2].rearrange(
                                "p j (c h) -> p j c h", h=HEADS),
                            w[:, :].rearrange("p (j h) -> p j h", j=kv)
                            .unsqueeze(2).broadcast_to([P, kv, HID, HEADS]),
                            op=OP.mult)
                        n = kv
                        while n > 1:
                            if n % 2 == 1:
                                nc.vector.tensor_tensor(
                                    hgv[:, 0, 0:NH2], hgv[:, 0, 0:NH2],
                                    hgv[:, n - 1, 0:NH2], op=OP.add)
                                n -= 1
                            pairs = n // 2
                            hp = hg[:, 0:pairs * 2 * W1R].rearrange(
                                "p (j two w) -> p j two w", two=2, w=W1R)
                            nc.vector.tensor_tensor(
                                hgv[:, 0:pairs, 0:NH2],
                                hp[:, :, 0, 0:NH2], hp[:, :, 1, 0:NH2],
                                op=OP.add)
                            n = pairs
                        if first:
                            nc.vector.tensor_scalar(
                                num[:, :], hgv[:, 0, 0:NH2], 0.0, None,
                                op0=OP.add)
                        else:
                            nc.vector.tensor_tensor(num[:, :], num[:, :],
                                                    hgv[:, 0, 0:NH2],
                                                    op=OP.add)

                    tcols = sum(kv * (8 + HEADS) for _, kv in vt1[t])
                    IT1 = l1b.tile([128, max(16, tcols)], I16, tag="it1")
                    nc.sync.dma_start(IT1[:, 0:tcols],
                                      idx1[:, col[0]:col[0] + tcols])
                    col[0] += tcols
                    tc0 = 0
                    pend = None
                    for v, (half, kv) in enumerate(vt1[t]):
                        itv = IT1[:, tc0:tc0 + kv * 8]
                        tc0 += kv * 8
                        ib4 = IT1[:, tc0:tc0 + kv * HEADS].bitcast(F16)
                        tc0 += kv * HEADS
                        hg = l1.tile([P, kv * W1R], F16, tag="hg")
                        nc.gpsimd.dma_gather(
                            hg[:, :].rearrange("p (j w) -> p j w", j=kv),
                            (hx1_lo if half == 0 else hx1_hi)[:, :],
                            itv, P * kv, P * kv, W1R, single_packet=False)
                        hgv = hg[:, :].rearrange("p (j w) -> p j w", j=kv)
                        s = l1b.tile([P, kv * HEADS], F16, tag="s")
                        nc.vector.tensor_tensor(
                            s[:, :], hgv[:, :, IN_DIM:IN_DIM + HEADS],
                            ib4, op=OP.add)
                        nc.vector.tensor_tensor(
                            s[:, :].rearrange("p (j h) -> p j h", j=kv),
                            s[:, :].rearrange("p (j h) -> p j h", j=kv),
                            sd4[:, t, HEADS:2 * HEADS].unsqueeze(1)
                            .broadcast_to([P, kv, HEADS]), op=OP.add)
                        w0 = l1b.tile([P, kv * HEADS], F16, tag="w0")
                        nc.vector.scalar_tensor_tensor(
                            w0[:, :], s[:, :], NEG_SLOPE, s[:, :],
                            op0=OP.mult, op1=OP.max)
                        w = l1b.tile([P, kv * HEADS], F16, tag="w")
                        nc.scalar.activation(w[:, :], w0[:, :], AF.Exp,
                                             bias=C("nshift"))
                        if pend is not None:
                            l1_mac(*pend)
                        pend = (hg, hgv, w, kv, v == 0)
                    if pend is not None:
                        l1_mac(*pend)
                    dinv = l1b.tile([P, HEADS], F32, tag="dinv")
                    nc.vector.tensor_scalar_max(dinv[:, :], den[:, :], 1e-6)
                    nc.vector.reciprocal(dinv[:, :], dinv[:, :])
                    o = l1b.tile([P, NH2], F16, tag="o")
                    nc.vector.tensor_tensor(
                        o[:, :].rearrange("p (c h) -> p c h", h=HEADS),
                        num[:, :].rearrange("p (c h) -> p c h", h=HEADS),
                        dinv[:, :].unsqueeze(1).broadcast_to(
                            [P, HID, HEADS]), op=OP.mult)
                    nc.vector.tensor_tensor(o[:, :], o[:, :], C("b1r"),
                                            op=OP.add)
                    # elu
                    m0 = l1b.tile([P, NH2], F16, tag="m0")
                    nc.vector.tensor_scalar_min(m0[:, :], o[:, :], 0.0)
                    em = l1b.tile([P, NH2], F16, tag="em")
                    nc.scalar.activation(em[:, :], m0[:, :], AF.Exp)
                    p1 = l1b.tile([P, NH2], F16, tag="p1")
                    nc.vector.tensor_scalar(p1[:, :], o[:, :], 0.0, -1.0,
                                            op0=OP.max, op1=OP.add)
                    eo = l1b.tile([P, NH2], F16, tag="eo")
                    nc.vector.tensor_tensor(eo[:, :], em[:, :], p1[:, :],
                                            op=OP.add)
                    # transpose + H2 matmul
                    pt = l1ps.tile([P, NH2], F16, tag="pt")
                    nc.tensor.transpose(pt[:, 0:P], eo[:, 0:P], C("ident"))
                    nc.tensor.transpose(pt[:, P:NH2], eo[:, P:NH2],
                                        C("ident"))
                    o1T = l1b.tile([P, NH2], F16, tag="o1T")
                    nc.scalar.copy(o1T[:, :], pt[:, :])
                    h2p = l1ps.tile([P, W2R], F32, tag="h2p")
                    nc.tensor.matmul(h2p[:, :], o1T[:, 0:P], C("w2x0"),
                                     start=True, stop=False)
                    nc.tensor.matmul(h2p[:, :], o1T[:, P:NH2], C("w2x1"),
                                     start=False, stop=True)
                    sh2 = l1b.tile([P, W2R], F16, tag="sh2")
                    nc.scalar.copy(sh2[:, :], h2p[:, :])
                    nc.sync.dma_start(SH2[t * P:(t + 1) * P, :], sh2[:, :])

                    if (t + 1) % CHT == 0 and phase == "full":
                        c = t // CHT
                        base = c * CH_ALL
                        if sim_collective:
                            for r in range(NC):
                                nc.sync.dma_start(
                                    HX2[base + r * CHROWS:
                                        base + (r + 1) * CHROWS, :],
                                    SH2[c * CHROWS:(c + 1) * CHROWS, :])
                        else:
                            nc.gpsimd.collective_compute(
                                "AllGather", OP.bypass,
                                replica_groups=[list(range(NC))],
                                ins=[SH2[c * CHROWS:(c + 1) * CHROWS, :].opt()],
                                outs=[HX2[base:base + CH_ALL, :].opt()],
                            )

            if phase == "l1":
                nc.sync.dma_start(dbg[0:128, 0:W2R].bitcast(F16),
                                  SH2[0:128, :])
            # ---------------- L2 + MLP + normalize ----------------------
            with (
                tc.tile_pool(name="l2", bufs=4) as l2,
                tc.tile_pool(name="l2b", bufs=4) as l2b,
                tc.tile_pool(name="l2ps", bufs=4, space="PSUM") as l2ps,
            ):
                sd2g = l2r.tile([P, NT * 128], F16, tag="sd2g")
                nc.gpsimd.dma_gather(
                    sd2g[:, :].rearrange("p (t w) -> p t w", t=NT),
                    SH2[:, OUT_DIM:OUT_DIM + 128], IT2[:, 0:NT * 8],
                    NT * P, NT * P, 128, elem_step=W2R, single_packet=False)
                sd2v = sd2g[:, :].rearrange("p (t w) -> p t w", t=NT)
                sd2f = l2r.tile([P, NT], F32, tag="sd2f")
                nc.vector.tensor_scalar(sd2f[:, :].unsqueeze(2),
                                        sd2v[:, :, 1:2], 0.0, None,
                                        op0=OP.add)

                o2a = l2r.tile([P, NT * OUT_DIM], F16, tag="o2a")
                magic = l2r.tile([P, 1], I32, tag="magic")
                nc.vector.memset(magic[:, :], 0x5F3759DF)
                col2 = [NT * 8]
                for t in (range(NT) if phase == "full" else range(0)):
                    num2 = l2b.tile([P, OUT_DIM], F16, tag="num2")
                    den2 = l2b.tile([P, 1], F32, tag="den2")
                    def l2_mac(hg, hgv, w, dv, kv, first):
                        if not first:
                            nc.vector.tensor_tensor(den2[:, :], den2[:, :],
                                                    dv[:, :], op=OP.add)
                        for j in range(kv):
                            nc.vector.tensor_scalar(
                                hgv[:, j, 0:OUT_DIM], hgv[:, j, 0:OUT_DIM],
                                w[:, j:j + 1], None, op0=OP.mult)
                        n = kv
                        while n > 1:
                            if n % 2 == 1:
                                nc.vector.tensor_tensor(
                                    hgv[:, 0, 0:OUT_DIM], hgv[:, 0, 0:OUT_DIM],
                                    hgv[:, n - 1, 0:OUT_DIM], op=OP.add)
                                n -= 1
                            pairs = n // 2
                            hp = hg[:, 0:pairs * 2 * W2R].rearrange(
                                "p (j two w) -> p j two w", two=2, w=W2R)
                            nc.vector.tensor_tensor(
                                hgv[:, 0:pairs, 0:OUT_DIM],
                                hp[:, :, 0, 0:OUT_DIM], hp[:, :, 1, 0:OUT_DIM],
                                op=OP.add)
                            n = pairs
                        if first:
                            nc.vector.tensor_scalar(
                                num2[:, :], hgv[:, 0, 0:OUT_DIM], 0.0, None,
                                op0=OP.add)
                        else:
                            nc.vector.tensor_tensor(num2[:, :], num2[:, :],
                                                    hgv[:, 0, 0:OUT_DIM],
                                                    op=OP.add)

                    pend = None
                    for v, (half, kv) in enumerate(vt2[t]):
                        itv = IT2[:, col2[0]:col2[0] + kv * 8]
                        col2[0] += kv * 8
                        ib = IT2[:, col2[0]:col2[0] + kv].bitcast(F16)
                        col2[0] += kv
                        hg = l2.tile([P, kv * W2R], F16, tag="hg2")
                        nc.gpsimd.dma_gather(
                            hg[:, :].rearrange("p (j w) -> p j w", j=kv),
                            (hx2_lo if half == 0 else hx2_hi)[:, :],
                            itv, P * kv, P * kv, W2R, single_packet=False)
                        hgv = hg[:, :].rearrange("p (j w) -> p j w", j=kv)
                        s = l2b.tile([P, kv], F16, tag="s2")
                        nc.vector.tensor_scalar(
                            s[:, :].unsqueeze(2),
                            hgv[:, :, OUT_DIM:OUT_DIM + 1],
                            sd2f[:, t:t + 1], None, op0=OP.add)
                        nc.vector.tensor_tensor(s[:, :], s[:, :], ib,
                                                op=OP.add)
                        w0 = l2b.tile([P, kv], F16, tag="w02")
                        nc.vector.scalar_tensor_tensor(
                            w0[:, :], s[:, :], NEG_SLOPE, s[:, :],
                            op0=OP.mult, op1=OP.max)
                        w = l2b.tile([P, kv], F32, tag="w2")
                        dv = den2 if v == 0 else l2b.tile([P, 1], F32,
                                                          tag="dv2")
                        nc.scalar.activation(w[:, :], w0[:, :], AF.Exp,
                                             bias=C("nshift"),
                                             accum_out=dv[:, :])
                        if pend is not None:
                            l2_mac(*pend)
                        pend = (hg, hgv, w, dv, kv, v == 0)
                    if pend is not None:
                        l2_mac(*pend)
                    dinv = l2b.tile([P, 1], F32, tag="dinv2")
                    nc.vector.tensor_scalar_max(dinv[:, :], den2[:, :], 1e-6)
                    nc.vector.reciprocal(dinv[:, :], dinv[:, :])
                    o2 = o2a[:, t * OUT_DIM:(t + 1) * OUT_DIM]
                    nc.vector.tensor_scalar(o2, num2[:, :],
                                            dinv[:, 0:1], None, op0=OP.mult)
                    nc.vector.tensor_tensor(o2, o2, C("b2r"), op=OP.add)

                for t in (range(NT) if phase == "full" else range(0)):
                    o2 = o2a[:, t * OUT_DIM:(t + 1) * OUT_DIM]
                    # MLP (all PSUM packed into one bank per tile)
                    mlp = l2ps.tile([P, 512], F32, tag="mlp")
                    pt2 = mlp[:, 0:64].bitcast(F16)
                    nc.tensor.transpose(pt2, o2, C("ident"))
                    o2T = l2b.tile([P, P], F16, tag="o2T")
                    nc.scalar.copy(o2T[:, :], pt2)
                    h3p = mlp[:, 64:128]
                    nc.tensor.matmul(h3p, o2T[:, :], C("wm1"),
                                     start=True, stop=True)
                    h3 = l2b.tile([P, HID], F16, tag="h3")
                    nc.vector.tensor_tensor(h3[:, :], h3p, C("bm1r"),
                                            op=OP.add)
                    nc.vector.tensor_scalar_max(h3[:, :], h3[:, :], 0.0)
                    pt3 = mlp[0:HID, 128:192].bitcast(F16)
                    nc.tensor.transpose(pt3, h3[:, :], C("ident"))
                    h3T = l2b.tile([HID, P], F16, tag="h3T")
                    nc.scalar.copy(h3T[:, :], pt3)
                    h4p = mlp[:, 256:384]
                    nc.tensor.matmul(h4p, h3T[0:HID, :],
                                     C("wm2")[0:HID, :], start=True, stop=True)
                    h4 = l2b.tile([P, OUT_DIM], F32, tag="h4")
                    nc.vector.tensor_tensor(h4[:, :], h4p, C("bm2r"),
                                            op=OP.add)
                    hsq = l2b.tile([P, OUT_DIM], F32, tag="hsq")
                    nc.vector.tensor_tensor(hsq[:, :], h4[:, :], h4[:, :],
                                            op=OP.mult)
                    n2 = l2b.tile([P, 1], F32, tag="n2")
                    nc.vector.tensor_reduce(n2[:, :], hsq[:, :], axis=AX.X,
                                            op=OP.add)
                    nin = l2b.tile([P, 1], F32, tag="nin")
                    nc.vector.tensor_scalar_max(nin[:, :], n2[:, :], 1e-12)
                    # Newton rsqrt (keeps Sqrt off the Act engine: avoids
                    # act-table thrash between Exp and Sqrt function sets)
                    yr = l2b.tile([P, 1], F32, tag="yr")
                    tn = l2b.tile([P, 1], F32, tag="tn")
                    nc.vector.tensor_scalar(yr[:, :].bitcast(I32),
                                            nin[:, :].bitcast(I32), 1, None,
                                            op0=OP.arith_shift_right)
                    nc.vector.tensor_tensor(yr[:, :].bitcast(I32),
                                            magic[:, :],
                                            yr[:, :].bitcast(I32),
                                            op=OP.subtract)
                    for _ in range(3):
                        nc.vector.tensor_tensor(tn[:, :], yr[:, :], yr[:, :],
                                                op=OP.mult)
                        nc.vector.tensor_tensor(tn[:, :], tn[:, :], nin[:, :],
                                                op=OP.mult)
                        nc.vector.tensor_scalar(tn[:, :], tn[:, :], -0.5, 1.5,
                                                op0=OP.mult, op1=OP.add)
                        nc.vector.tensor_tensor(yr[:, :], yr[:, :], tn[:, :],
                                                op=OP.mult)
                    nin = yr
                    of = l2b.tile([P, OUT_DIM], F32, tag="of")
                    nc.vector.tensor_scalar(of[:, :], h4[:, :], nin[:, 0:1],
                                            None, op0=OP.mult)
                    nc.sync.dma_start(out[t * P:(t + 1) * P, :], of[:, :])
            _l2r_cm.__exit__(None, None, None)

    nc.compile()
    return nc


# ------------------------------------------------------------------ driver

def run(cfg, inputs, trace=False, phase="full", sim_collective=False,
        prep=None):
    x = np.asarray(inputs["x"], dtype=np.float32)
    edge_index = np.asarray(inputs["edge_index"])
    if prep is None:
        prep = _prep(cfg, edge_index)
    consts, cblocks = _pack_consts(
        cfg, *[np.asarray(inputs[k], dtype=np.float32) for k in
               ("W1", "a1_src", "a1_dst", "b1", "W2", "a2_src", "a2_dst",
                "b2", "Wm1", "bm1", "Wm2", "bm2")])
    nc = _build(cfg, prep, cblocks, consts.shape[1], phase=phase,
                sim_collective=sim_collective)
    xT = np.ascontiguousarray(x.T.astype(np.float16))
    SH = cfg["SHARD"]
    in_maps = []
    for k in range(cfg["NC"]):
        in_maps.append({
            "xT": np.ascontiguousarray(xT[:, k * SH:(k + 1) * SH]),
            "consts": consts,
            "idx1": np.ascontiguousarray(prep["blobs1"][k]),
            "idx2": np.ascontiguousarray(prep["blobs2"][k]),
        })
    res = run_bass_kernel_spmd(nc, in_maps, list(range(cfg["NC"])),
                               trace=trace)
    N, NC, SHARD = cfg["N"], cfg["NC"], cfg["SHARD"]
    full = np.zeros((N, cfg["OUT_DIM"]), dtype=np.float32)
    for k in range(NC):
        o = res.results[k]["out"]
        perm2 = prep["cores"][k]["perm2"]
        real = perm2 >= 0
        full[k * SHARD + perm2[real]] = o[real]
    return full, res


def kernel(**inputs):
    cfg = make_cfg()
    full, _ = run(cfg, inputs, trace=False)
    return full
